# revision 13
# baseline (speedup 1.0000x reference)
"""Trainium2 Bass kernel for DeepConvGraphEncoderDownstream.

Model (per reference):
  4-layer GCN (shared dense 24x24 graph operator) applied per (batch, timestep)
  frame -> node-mean -> per sliding window (W=32, stride 2, 113 windows):
  BiLSTM(H=256) -> concat(h_fwd[-1], h_bwd[0]) @ Wfc + bfc.

Key algebraic restructurings:
  * gcn_norm folded into one dense Ahat[24,24] on host.
  * GCN runs ONCE over all 256 timesteps (the reference recomputes it ~14x
    across overlapping windows).
  * backward LSTM: only hb[:, 0] is used => exactly ONE step, no recurrence.
  * forward LSTM: all 113 windows batched into one 904-row recurrence per
    core; input transforms U precomputed from node-mean features.

Sharding: data-parallel over batch, 8 batches/core on 8 cores; output
slices are independent (no collectives).

v2 performance structure:
  * GCN processed in 16 half-chunk units (28/24 gb groups) for cross-unit
    pipelining; pools sized for ~1 unit of overlap.
  * DMA transposes round-robin over the two HWDGE rings (sync 5/6,
    scalar 1/6); x0 loads on gpsimd SWDGE.
  * PSUM->SBUF casts and bias+ReLU rotate between vector and scalar.
  * x0 packed at c=8 (6 real channels) => mix L1 is 224 cols, not 3584;
    L1->L2 transpose narrow (64 channels).
  * node-sum via single strided tensor_reduce per (kt, unit).
  * LSTM: no identity matmuls -- U+bias pre-written into PSUM by vector,
    Whh matmuls accumulate on top (start=False); activations batched per
    gate pair; elementwise state update on gpsimd.

Layouts (per core, per chunk = one local batch = 256 timesteps padded to
260 = 52 blocks * 5):
  A-layout [c_part, free=(gb, blk:128)], blk = n*5+g5 (120:128 pad),
           timestep t = 5*gb + g5.
  B-layout [blk partitions = 128, free=(gb, c)]
  A->B / B->A are single XBAR DMA-transpose instructions per c-block:
  HW semantics out[p, b, c] = in[c, b*128 + p].
  Node mixing = matmul with zero-padded stationary kron(Ahat^T, I5) [128,128].
"""

import itertools
import os
import sys
import numpy as np

try:
    import concourse.bass as bass
except ImportError:
    sys.path.insert(0, "/opt/trn_rl_repo")
    import concourse.bass as bass

import concourse.bacc as bacc
import concourse.tile as tile
from concourse import mybir
from concourse import bass_utils

F16 = mybir.dt.float16
F32 = mybir.dt.float32
AF = mybir.ActivationFunctionType
ALU = mybir.AluOpType

B, T, N, FIN = 64, 256, 24, 6
H, EMB = 256, 128
WIN = 32
NW = (T - WIN) // 2 + 1               # 113
NCORES = 8
BL = B // NCORES                      # 8
G5 = 5
GBLK = 52                             # ceil(260/5): 52*5 = 260 t-slots
TP = GBLK * G5                        # 260 padded timesteps
NCH = BL
ROWS = BL * NW                        # 904
HROWS = ROWS // 2                     # 452
FTOT = BL * TP                        # 2080 F columns
HALVES = [(0, 28), (28, 24)]          # (gb offset, gb count) per unit

PREWRITE = os.environ.get("K_PREWRITE", "1") == "1"

_CACHE = {}


def _chunks(nf, step):
    return [(i, min(step, nf - i)) for i in range(0, nf, step)]


def _kernel_body(tc, io):
    nc = tc.nc
    from contextlib import ExitStack
    ctx = ExitStack()

    cons = ctx.enter_context(tc.tile_pool(name="cons", bufs=1))
    fpool = ctx.enter_context(tc.tile_pool(name="fpool", bufs=1))

    def load_const(name, shape, dt=F16):
        t = cons.tile(shape, dt, name=name)
        nc.sync.dma_start(t[:], io[name][:])
        return t

    mixM = load_const("mixM", [128, 128])
    w1 = load_const("w1", [FIN, 64])
    w2 = load_const("w2", [64, 128])
    w3 = load_const("w3", [128, 256])
    b1 = load_const("b1", [64, 1], F32)
    b2 = load_const("b2", [128, 1], F32)
    b3 = load_const("b3", [128, 2], F32)
    b4 = load_const("b4", [128, 2], F32)
    w4k = []
    for kt in range(2):
        t = cons.tile([128, 256], F16, name=f"w4k{kt}")
        nc.sync.dma_start(t[:], io["w4"][kt * 128:(kt + 1) * 128, :])
        w4k.append(t)

    def load_ktiles(name):
        ts = []
        for kt in range(2):
            t = cons.tile([128, 1024], F16, name=f"{name}{kt}")
            nc.sync.dma_start(t[:], io[name][kt * 128:(kt + 1) * 128, :])
            ts.append(t)
        return ts

    lxf = load_ktiles("lxf")
    lhf = load_ktiles("lhf")
    lxb = load_ktiles("lxb")
    bgf = load_const("bgf", [128, 8], F32)
    bgb = load_const("bgb", [128, 8], F32)
    wfct = []
    for qt in range(4):
        t = cons.tile([128, 128], F16, name=f"wfct{qt}")
        nc.sync.dma_start(t[:], io["wfc"][qt * 128:(qt + 1) * 128, :])
        wfct.append(t)
    bfc = load_const("bfc", [128, 1], F32)
    ident = load_const("ident", [128, 128])

    F0 = fpool.tile([128, FTOT], F16, name="F0")
    F1 = fpool.tile([128, FTOT], F16, name="F1")
    Fts = [F0, F1]

    # persistent zero-padded B-layout staging tiles (pad cols stay 0)
    y1pads = [cons.tile([128, 28 * 128], F16, name=f"y1p{i}") for i in range(2)]
    y2pads = [cons.tile([128, 28 * 128], F16, name=f"y2p{i}") for i in range(2)]
    for i in range(2):
        nc.vector.memset(y1pads[i][:], 0.0)
        nc.gpsimd.memset(y2pads[i][:], 0.0)

    # engine rotations
    cast_cycle = itertools.cycle([nc.vector, nc.vector, nc.scalar])
    relu_cycle = itertools.cycle([nc.scalar, nc.vector, nc.vector])
    # concurrent XBAR transposes on the two HWDGE rings corrupt each other
    # (shared xbar S2M state) -- keep ALL transposes on the sync ring.
    tp_cycle = itertools.cycle([nc.sync])

    def cast_rot(dst, src):
        e = next(cast_cycle)
        if e is nc.scalar:
            nc.scalar.copy(dst, src)
        else:
            e.tensor_copy(dst, src)

    def relu_rot(dst, src, bias_ap):
        e = next(relu_cycle)
        if e is nc.scalar:
            nc.scalar.activation(dst, src, AF.Relu, bias=bias_ap, scale=1.0)
        else:
            e.tensor_scalar(dst, src, bias_ap, 0.0, ALU.add, ALU.max)

    def tpose(out_ap, in_ap):
        next(tp_cycle).dma_start(out_ap, in_ap, transpose=True)

    # ================= Phase 1: GCN =================
    with tc.tile_pool(name="gcnS", bufs=3) as gpS, \
         tc.tile_pool(name="gcnA", bufs=9) as gpA, \
         tc.tile_pool(name="gcnB", bufs=5) as gpB, \
         tc.tile_pool(name="gcnBn", bufs=3) as gpBn, \
         tc.tile_pool(name="gcnBig", bufs=2) as gpBig, \
         tc.tile_pool(name="gps", bufs=2, space="PSUM") as ps_t, \
         tc.tile_pool(name="gpsm", bufs=2, space="PSUM") as ps_m:

        def mix(src_ap, n_free, cast_fn):
            """node-mix src_ap [128, n_free] via mixM; cast_fn(c0, cw, ps_ap)
            moves each fp32 psum chunk into its fp16 destination."""
            for c0, cw in _chunks(n_free, 1024):
                ps = ps_m.tile([128, 1024], F32, tag="mps", name="mps")
                for s0, sw in _chunks(cw, 512):
                    nc.tensor.matmul(ps[:, s0:s0 + sw], mixM[:],
                                     src_ap[:, c0 + s0:c0 + s0 + sw],
                                     start=True, stop=True)
                cast_fn(c0, cw, ps[:, 0:cw])

        def transform(rhs_list, wslices, bias, cout, out_tiles, FH):
            """dense channel transform: rhs_list [rows,FH] fp16 tiles;
            out c-tiles get bias+ReLU fused in the PSUM->SBUF move."""
            nkt = len(rhs_list)
            nmt = max(cout // 128, 1)
            mp = min(cout, 128)
            for mt in range(nmt):
                for f0, fw in _chunks(FH, 1024):
                    ps = ps_t.tile([128, 1024], F32, tag="tps", name="tps")
                    for s0, sw in _chunks(fw, 512):
                        g0 = f0 + s0
                        for kt in range(nkt):
                            w = wslices[kt]
                            wap = w[:, mt * 128:(mt + 1) * 128] if cout > 128 \
                                else w
                            nc.tensor.matmul(ps[0:mp, s0:s0 + sw], wap,
                                             rhs_list[kt][:, g0:g0 + sw],
                                             start=(kt == 0),
                                             stop=(kt == nkt - 1))
                    relu_rot(out_tiles[mt][0:mp, f0:f0 + fw],
                             ps[0:mp, 0:fw], bias[0:mp, mt:mt + 1])

        for uidx in range(NCH * 2):
            k, hf_i = uidx // 2, uidx % 2
            g0gb, gh = HALVES[hf_i]
            FH = gh * 128

            # --- L1: packed-c8 input, mix at c=8, narrow transform 6->64
            x0t = gpS.tile([128, 28 * 8], F16, tag="x0", name="x0t")
            nc.gpsimd.dma_start(x0t[:, 0:gh * 8],
                                io["x0"][k][:, g0gb * 8:(g0gb + gh) * 8])
            y1p = y1pads[uidx % 2]

            def cast_y1(c0, cw, ps_ap, _y1p=y1p, _gh=gh):
                cast_rot(_y1p[:, 0:_gh * 128]
                         .rearrange("p (gb c) -> p gb c", c=128)[:, :, 0:8],
                         ps_ap.rearrange("p (gb c) -> p gb c", c=8))

            mix(x0t[:, 0:gh * 8], gh * 8, cast_y1)
            y1a = gpA.tile([128, 28 * 128], F16, tag="cA", name="y1a")
            tpose(y1a[:, 0:FH].rearrange("c (gb p) -> c gb p", p=128),
                  y1p[:, 0:FH])
            x1a = gpA.tile([128, 28 * 128], F16, tag="cA", name="x1a")
            transform([y1a[0:FIN, 0:FH]], [w1[:]], b1, 64, [x1a], FH)

            # --- L2: narrow A->B (64 c), mix at 64, transform 64->128
            x2b = gpBn.tile([128, 28 * 64], F16, tag="cBn", name="x2b")
            tpose(x2b[:, 0:gh * 64].rearrange("p (gb c) -> p gb c", c=64),
                  x1a[0:64, 0:FH])
            y2p = y2pads[uidx % 2]

            def cast_y2(c0, cw, ps_ap, _y2p=y2p, _gh=gh):
                cast_rot(_y2p[:, 0:_gh * 128]
                         .rearrange("p (gb c) -> p gb c", c=128)
                         [:, c0 // 64:(c0 + cw) // 64, 0:64],
                         ps_ap.rearrange("p (gb c) -> p gb c", c=64))

            mix(x2b[:, 0:gh * 64], gh * 64, cast_y2)
            y2a = gpA.tile([128, 28 * 128], F16, tag="cA", name="y2a")
            tpose(y2a[:, 0:FH].rearrange("c (gb p) -> c gb p", p=128),
                  y2p[:, 0:FH])
            x2a = gpA.tile([128, 28 * 128], F16, tag="cA", name="x2a")
            transform([y2a[0:64, 0:FH]], [w2[:]], b2, 128, [x2a], FH)

            # --- L3: mix@128 -> transform 128->256
            x3b = gpB.tile([128, 28 * 128], F16, tag="cB", name="x3b")
            tpose(x3b[:, 0:FH].rearrange("p (gb c) -> p gb c", c=128),
                  x2a[:, 0:FH])
            y3b = gpB.tile([128, 28 * 128], F16, tag="cB", name="y3b")

            def cast_y3(c0, cw, ps_ap, _y3b=y3b):
                cast_rot(_y3b[:, c0:c0 + cw], ps_ap)

            mix(x3b[:, 0:FH], FH, cast_y3)
            y3a = gpA.tile([128, 28 * 128], F16, tag="cA", name="y3a")
            tpose(y3a[:, 0:FH].rearrange("c (gb p) -> c gb p", p=128),
                  y3b[:, 0:FH])
            x3a0 = gpA.tile([128, 28 * 128], F16, tag="cA", name="x3a0")
            x3a1 = gpA.tile([128, 28 * 128], F16, tag="cA", name="x3a1")
            transform([y3a[:, 0:FH]], [w3[:]], b3, 256, [x3a0, x3a1], FH)

            # --- L4: mix@256 -> transform 256->256
            x4b = gpBig.tile([128, 28 * 256], F16, tag="big", name="x4b")
            x4bv = x4b[:, 0:gh * 256].rearrange("p (gb c) -> p gb c", c=256)
            tpose(x4bv[:, :, 0:128], x3a0[:, 0:FH])
            tpose(x4bv[:, :, 128:256], x3a1[:, 0:FH])
            ylo = gpB.tile([128, 28 * 128], F16, tag="cB", name="ylo")
            yhi = gpB.tile([128, 28 * 128], F16, tag="cB", name="yhi")

            def cast_y4(c0, cw, ps_ap, _ylo=ylo, _yhi=yhi):
                psv = ps_ap.rearrange("p (g c) -> p g c", c=256)
                g = cw // 256
                for hf in range(2):
                    dv = (_ylo, _yhi)[hf][:].rearrange(
                        "p (gb c) -> p gb c", c=128)
                    cast_rot(dv[:, c0 // 256:c0 // 256 + g, :],
                             psv[:, :, hf * 128:(hf + 1) * 128])

            mix(x4b[:, 0:gh * 256], gh * 256, cast_y4)
            y4a0 = gpA.tile([128, 28 * 128], F16, tag="cA", name="y4a0")
            y4a1 = gpA.tile([128, 28 * 128], F16, tag="cA", name="y4a1")
            tpose(y4a0[:, 0:FH].rearrange("c (gb p) -> c gb p", p=128),
                  ylo[:, 0:FH])
            tpose(y4a1[:, 0:FH].rearrange("c (gb p) -> c gb p", p=128),
                  yhi[:, 0:FH])
            x4a0 = gpA.tile([128, 28 * 128], F16, tag="cA", name="x4a0")
            x4a1 = gpA.tile([128, 28 * 128], F16, tag="cA", name="x4a1")
            transform([y4a0[:, 0:FH], y4a1[:, 0:FH]],
                      [w4k[0][:], w4k[1][:]], b4, 256, [x4a0, x4a1], FH)

            if uidx == 1 and "sdbg0" in io:
                nc.sync.dma_start(io["sdbg0"][:], x4b[:, 0:3584])
                nc.sync.dma_start(io["sdbg1"][:, 0:gh * 256 - 3584],
                                  x4b[:, 3584:gh * 256])
                for si, st in enumerate((ylo, yhi, y4a0, y4a1, x4a0),
                                        start=2):
                    nc.sync.dma_start(io[f"sdbg{si}"][:, 0:FH],
                                      st[:, 0:FH])

            # node-sum into F: F[:, k*TP + t], t = 5*(g0gb+gb) + g5
            for ct, xt in enumerate((x4a0, x4a1)):
                xv = (xt[:, 0:FH]
                      .rearrange("p (gb blk) -> p gb blk", blk=128)
                      [:, :, 0:120]
                      .rearrange("p gb (n g5) -> p gb g5 n", g5=G5))
                dstv = (Fts[ct][:, k * TP + g0gb * G5:
                                k * TP + (g0gb + gh) * G5]
                        .rearrange("p (gb g5) -> p gb g5", g5=G5))
                with nc.allow_low_precision("node-sum in fp16, as baseline"):
                    nc.vector.tensor_reduce(dstv, xv, mybir.AxisListType.X,
                                            ALU.add)

    # ================= Phase 2: U = F @ (Wih_f/24)^T =================
    upool = ctx.enter_context(tc.tile_pool(name="upool", bufs=1))
    Umt = []
    with tc.tile_pool(name="ups", bufs=3, space="PSUM") as ps_u:
        for mt in range(8):
            u = upool.tile([128, FTOT], F16, name=f"U{mt}")
            for f0, fw in _chunks(FTOT, 1024):
                ps = ps_u.tile([128, 1024], F32, tag="ups", name="ups")
                for s0, sw in _chunks(fw, 512):
                    g0 = f0 + s0
                    for kt in range(2):
                        nc.tensor.matmul(ps[:, s0:s0 + sw],
                                         lxf[kt][:, mt * 128:(mt + 1) * 128],
                                         Fts[kt][:, g0:g0 + sw],
                                         start=(kt == 0), stop=(kt == 1))
                cast_rot(u[:, f0:f0 + fw], ps[:, 0:fw])
            Umt.append(u)

    # ================= Phase 3: forward LSTM =================
    lp = ctx.enter_context(tc.tile_pool(name="lstm", bufs=1))
    Hf = lp.tile([128, 2 * ROWS], F16, name="Hf")
    Cf = lp.tile([128, 2 * ROWS], F16, name="Cf")
    nc.vector.memset(Hf[:], 0.0)
    nc.gpsimd.memset(Cf[:], 0.0)
    gi = lp.tile([128, 2 * ROWS], F16, name="gi")
    gf = lp.tile([128, 2 * ROWS], F16, name="gf")
    go = lp.tile([128, 2 * ROWS], F16, name="go")
    tg = lp.tile([128, 2 * ROWS], F16, name="tg")
    tcl = lp.tile([128, 2 * ROWS], F16, name="tcl")
    tmp = lp.tile([128, 2 * ROWS], F16, name="tmp")
    PAIRS = [(0, 1, gi, AF.Sigmoid), (2, 3, gf, AF.Sigmoid),
             (4, 5, go, AF.Sigmoid), (6, 7, tg, AF.Tanh)]

    with tc.tile_pool(name="lps", bufs=2, space="PSUM") as ps_l:
        for s in range(WIN):
            k0, par = s // 2, s % 2
            for ma, mb, gdst, fn in PAIRS:
                ps = ps_l.tile([128, 2048], F32, tag="lp", name="lp")
                for j, mt in ((0, ma), (1, mb)):
                    uv = Umt[mt][:].rearrange("p (b k two) -> p b k two",
                                              b=BL, two=2)
                    if PREWRITE:
                        # pre-write U + gate bias into PSUM (vector engine)
                        src = (uv[:, :, k0:k0 + NW, par]
                               .rearrange("p (hh b) k -> p hh b k", hh=2))
                        dst = (ps[:, j * 1024:(j + 1) * 1024]
                               .rearrange("p (hh x) -> p hh x", hh=2)
                               [:, :, 0:HROWS]
                               .rearrange("p hh (b k) -> p hh b k", k=NW))
                        nc.vector.tensor_scalar(dst, src, bgf[:, mt:mt + 1],
                                                None, ALU.add)
                    for hh in range(2):
                        pslice = ps[:, j * 1024 + hh * 512:
                                    j * 1024 + hh * 512 + HROWS]
                        b0 = hh * (BL // 2)
                        if not PREWRITE:
                            nc.tensor.matmul(
                                pslice, ident[:],
                                uv[:, b0:b0 + BL // 2, k0:k0 + NW, par],
                                start=True, stop=False)
                        for kt in range(2):
                            nc.tensor.matmul(
                                pslice, lhf[kt][:, mt * 128:(mt + 1) * 128],
                                Hf[:, kt * ROWS + hh * HROWS:
                                   kt * ROWS + (hh + 1) * HROWS],
                                start=False, stop=(kt == 1),
                                skip_group_check=True)
                psq = ps[:].rearrange("p (q x) -> p q x", q=4)[:, :, 0:HROWS]
                gv = gdst[:].rearrange("p (q r) -> p q r", q=4)
                if PREWRITE:
                    nc.scalar.activation(gv, psq, fn, scale=1.0)
                else:
                    gva = gdst[:].rearrange("p (m x) -> p m x", m=2)
                    psa = ps[:].rearrange("p (m x) -> p m x", m=2)
                    for j, mt in ((0, ma), (1, mb)):
                        nc.scalar.activation(
                            gva[:, j, :].rearrange("p (hh r) -> p hh r",
                                                   hh=2),
                            psa[:, j, :].rearrange("p (hh x) -> p hh x",
                                                   hh=2)[:, :, 0:HROWS],
                            fn, bias=bgf[:, mt:mt + 1], scale=1.0)
            nc.gpsimd.tensor_tensor(tmp[:], gi[:], tg[:], ALU.mult)
            nc.gpsimd.tensor_tensor(Cf[:], gf[:], Cf[:], ALU.mult)
            nc.gpsimd.tensor_tensor(Cf[:], Cf[:], tmp[:], ALU.add)
            nc.scalar.activation(tcl[:], Cf[:], AF.Tanh)
            nc.gpsimd.tensor_tensor(Hf[:], go[:], tcl[:], ALU.mult)

        # ===== Phase 4: backward LSTM single step (only hb[:,0] used) =====
        Hb = lp.tile([128, 2 * ROWS], F16, name="Hb")
        kb = (WIN - 2) // 2
        BPAIRS = [(0, 1, gi, AF.Sigmoid), (4, 5, go, AF.Sigmoid),
                  (6, 7, tg, AF.Tanh)]
        for ma, mb, gdst, fn in BPAIRS:
            ps = ps_l.tile([128, 2048], F32, tag="lp", name="lpb")
            for j, mt in ((0, ma), (1, mb)):
                for hh in range(2):
                    pslice = ps[:, j * 1024 + hh * 512:
                                j * 1024 + hh * 512 + HROWS]
                    b0 = hh * (BL // 2)
                    for kt in range(2):
                        fv = Fts[kt][:].rearrange("p (b k two) -> p b k two",
                                                  b=BL, two=2)
                        nc.tensor.matmul(
                            pslice, lxb[kt][:, mt * 128:(mt + 1) * 128],
                            fv[:, b0:b0 + BL // 2, kb:kb + NW, 1],
                            start=(kt == 0), stop=(kt == 1))
                psj = (ps[:, j * 1024:(j + 1) * 1024]
                       .rearrange("p (hh x) -> p hh x", hh=2)[:, :, 0:HROWS])
                gvj = (gdst[:, j * ROWS:(j + 1) * ROWS]
                       .rearrange("p (hh r) -> p hh r", hh=2))
                nc.scalar.activation(gvj, psj, fn,
                                     bias=bgb[:, mt:mt + 1], scale=1.0)
        nc.gpsimd.tensor_tensor(tmp[:], gi[:], tg[:], ALU.mult)
        nc.scalar.activation(tcl[:], tmp[:], AF.Tanh)
        nc.gpsimd.tensor_tensor(Hb[:], go[:], tcl[:], ALU.mult)

        # ===== Phase 5: FC head =====
        ps = ps_l.tile([128, 2048], F32, tag="lp", name="lpf")
        rhs4 = [Hf[:, 0:ROWS], Hf[:, ROWS:2 * ROWS],
                Hb[:, 0:ROWS], Hb[:, ROWS:2 * ROWS]]
        for hh in range(2):
            for qt in range(4):
                nc.tensor.matmul(ps[:, hh * 512:hh * 512 + HROWS],
                                 wfct[qt][:],
                                 rhs4[qt].rearrange("p (h r) -> p h r",
                                                    h=2)[:, hh, :],
                                 start=(qt == 0), stop=(qt == 3))
        ob = lp.tile([EMB, ROWS], F32, name="ob")
        obv = ob[:].rearrange("p (h r) -> p h r", h=2)
        psv = (ps[:, 0:1024].rearrange("p (h x) -> p h x", h=2)
               [:, :, 0:HROWS])
        nc.scalar.activation(obv, psv, AF.Identity,
                             bias=bfc[:, 0:1], scale=1.0)
        nc.sync.dma_start(io["out_d"][:], ob[:])

    if "fdbg0" in io:
        nc.sync.dma_start(io["fdbg0"][:], F0[:])
        nc.sync.dma_start(io["fdbg1"][:], F1[:])
    ctx.close()


def _build_program():
    nc = bacc.Bacc("TRN2", target_bir_lowering=False, debug=False,
                   num_devices=NCORES)

    def din(name, shape, dt=F16):
        return nc.dram_tensor(name, shape, dt, kind="ExternalInput").ap()

    io = dict(
        x0=din("x0", [NCH, 128, GBLK * 8]),
        mixM=din("mixM", [128, 128]),
        w1=din("w1", [FIN, 64]), w2=din("w2", [64, 128]),
        w3=din("w3", [128, 256]), w4=din("w4", [256, 256]),
        b1=din("b1", [64, 1], F32), b2=din("b2", [128, 1], F32),
        b3=din("b3", [128, 2], F32), b4=din("b4", [128, 2], F32),
        lxf=din("lxf", [256, 1024]), lhf=din("lhf", [256, 1024]),
        lxb=din("lxb", [256, 1024]),
        bgf=din("bgf", [128, 8], F32), bgb=din("bgb", [128, 8], F32),
        wfc=din("wfc", [512, 128]), bfc=din("bfc", [128, 1], F32),
        ident=din("ident", [128, 128]),
        out_d=nc.dram_tensor("out", [EMB, ROWS], F32,
                             kind="ExternalOutput").ap(),
    )
    if os.environ.get("K_FDBG", "0") == "1":
        io["fdbg0"] = nc.dram_tensor("fdbg0", [128, FTOT], F16,
                                     kind="ExternalOutput").ap()
        io["fdbg1"] = nc.dram_tensor("fdbg1", [128, FTOT], F16,
                                     kind="ExternalOutput").ap()
    if os.environ.get("K_SDBG", "0") == "1":
        for si in range(7):
            io[f"sdbg{si}"] = nc.dram_tensor(f"sdbg{si}", [128, 28 * 128],
                                             F16,
                                             kind="ExternalOutput").ap()
    with tile.TileContext(nc) as tc:
        _kernel_body(tc, io)
    nc.compile()
    return nc


def _host_prep(inputs):
    f16 = np.float16
    data = np.asarray(inputs["data"], np.float32)
    ei = np.asarray(inputs["edge_index"]).astype(np.int64)

    src = np.concatenate([ei[0], np.arange(N)])
    dst = np.concatenate([ei[1], np.arange(N)])
    deg = np.zeros(N, np.float32)
    np.add.at(deg, dst, 1.0)
    dinv = np.where(deg > 0, deg ** -0.5, 0.0).astype(np.float32)
    Ahat = np.zeros((N, N), np.float32)
    np.add.at(Ahat, (dst, src), dinv[src] * dinv[dst])
    mixM = np.zeros((128, 128), np.float32)
    mixM[0:N * G5, 0:N * G5] = np.kron(Ahat.T, np.eye(G5, dtype=np.float32))
    mixM = mixM.astype(f16)

    # x0: [core][chunk b][blk = n*5+g5 (120:128 zero)][gb*8 + c],
    # t = 5*gb+g5, channels 6:8 zero
    d = data.reshape(NCORES, BL, T, N, FIN)
    x0 = np.zeros((NCORES, BL, 128, GBLK, 8), np.float32)
    dpad = np.zeros((NCORES, BL, TP, N, FIN), np.float32)
    dpad[:, :, :T] = d
    dv = dpad.reshape(NCORES, BL, GBLK, G5, N, FIN)
    # [core, b, n, g5, gb, c]
    dv = dv.transpose(0, 1, 4, 3, 2, 5).reshape(NCORES, BL, N * G5, GBLK, FIN)
    x0[:, :, 0:N * G5, :, 0:FIN] = dv
    x0 = np.ascontiguousarray(
        x0.reshape(NCORES, BL, 128, GBLK * 8)).astype(f16)

    perm = np.concatenate([np.arange(0, H), np.arange(H, 2 * H),
                           np.arange(3 * H, 4 * H), np.arange(2 * H, 3 * H)])

    def prep_dir(wih, whh, bih, bhh):
        wihp = np.asarray(wih, np.float32)[perm] / N
        whhp = np.asarray(whh, np.float32)[perm]
        bg = (np.asarray(bih, np.float32) + np.asarray(bhh, np.float32))[perm]
        return (np.ascontiguousarray(wihp.T).astype(f16),
                np.ascontiguousarray(whhp.T).astype(f16),
                np.ascontiguousarray(bg.reshape(8, 128).T).astype(np.float32))

    lxf, lhf, bgf = prep_dir(inputs["lstm_Wih_f"], inputs["lstm_Whh_f"],
                             inputs["lstm_bih_f"], inputs["lstm_bhh_f"])
    lxb, _lhb, bgb = prep_dir(inputs["lstm_Wih_b"], inputs["lstm_Whh_b"],
                              inputs["lstm_bih_b"], inputs["lstm_bhh_b"])

    com = {
        "mixM": mixM,
        "w1": np.asarray(inputs["W1"], np.float32).astype(f16),
        "w2": np.asarray(inputs["W2"], np.float32).astype(f16),
        "w3": np.asarray(inputs["W3"], np.float32).astype(f16),
        "w4": np.asarray(inputs["W4"], np.float32).astype(f16),
        "b1": np.asarray(inputs["b1"], np.float32).reshape(64, 1),
        "b2": np.asarray(inputs["b2"], np.float32).reshape(128, 1),
        "b3": np.ascontiguousarray(
            np.asarray(inputs["b3"], np.float32).reshape(2, 128).T),
        "b4": np.ascontiguousarray(
            np.asarray(inputs["b4"], np.float32).reshape(2, 128).T),
        "lxf": lxf, "lhf": lhf, "lxb": lxb, "bgf": bgf, "bgb": bgb,
        "wfc": np.asarray(inputs["Wfc"], np.float32).astype(f16),
        "bfc": np.asarray(inputs["bfc"], np.float32).reshape(128, 1),
        "ident": np.eye(128, dtype=f16),
    }
    return [dict(com, x0=x0[c]) for c in range(NCORES)]


TRACE = False          # set by test harness to capture an NTFF profile


def kernel(**inputs) -> np.ndarray:
    if "nc" not in _CACHE:
        _CACHE["nc"] = _build_program()
    nc = _CACHE["nc"]
    in_maps = _host_prep(inputs)
    res = bass_utils.run_bass_kernel_spmd(nc, in_maps,
                                          core_ids=list(range(NCORES)),
                                          trace=TRACE)
    _CACHE["last_res"] = res
    outs = []
    for c in range(NCORES):
        o = res.results[c]["out"]                       # [128, 904]
        outs.append(o.reshape(EMB, BL, NW).transpose(1, 2, 0))
    return np.concatenate(outs, 0).astype(np.float32)   # [64, 113, 128]


if __name__ == "__main__":
    import reference
    ins = {k: np.asarray(v) for k, v in reference.setup_inputs().items()}
    out = kernel(**ins)
    print("kernel out", out.shape, out.dtype, float(np.abs(out).max()))


# revision 19
# speedup vs baseline: 1.3448x; 1.3448x over previous
"""Trainium2 Bass kernel for DeepConvGraphEncoderDownstream.

Model (per reference):
  4-layer GCN (shared dense 24x24 graph operator) applied per (batch, timestep)
  frame -> node-mean -> per sliding window (W=32, stride 2, 113 windows):
  BiLSTM(H=256) -> concat(h_fwd[-1], h_bwd[0]) @ Wfc + bfc.

Key algebraic restructurings:
  * gcn_norm folded into one dense Ahat[24,24] on host.
  * GCN runs ONCE over all 256 timesteps (the reference recomputes it ~14x
    across overlapping windows).
  * backward LSTM: only hb[:, 0] is used => exactly ONE step, no recurrence.
  * forward LSTM: all 113 windows batched into one 904-row recurrence per
    core; input transforms U precomputed from node-mean features.

Sharding: data-parallel over batch, 8 batches/core on 8 cores; output
slices are independent (no collectives).

v2 performance structure:
  * GCN processed in 16 half-chunk units (28/24 gb groups) for cross-unit
    pipelining; pools sized for ~1 unit of overlap.
  * DMA transposes round-robin over the two HWDGE rings (sync 5/6,
    scalar 1/6); x0 loads on gpsimd SWDGE.
  * PSUM->SBUF casts and bias+ReLU rotate between vector and scalar.
  * x0 packed at c=8 (6 real channels) => mix L1 is 224 cols, not 3584;
    L1->L2 transpose narrow (64 channels).
  * node-sum via single strided tensor_reduce per (kt, unit).
  * LSTM: no identity matmuls -- U+bias pre-written into PSUM by vector,
    Whh matmuls accumulate on top (start=False); activations batched per
    gate pair; elementwise state update on gpsimd.

Layouts (per core, per chunk = one local batch = 256 timesteps padded to
260 = 52 blocks * 5):
  A-layout [c_part, free=(gb, blk:128)], blk = n*5+g5 (120:128 pad),
           timestep t = 5*gb + g5.
  B-layout [blk partitions = 128, free=(gb, c)]
  A->B / B->A are single XBAR DMA-transpose instructions per c-block:
  HW semantics out[p, b, c] = in[c, b*128 + p].
  Node mixing = matmul with zero-padded stationary kron(Ahat^T, I5) [128,128].
"""

import itertools
import os
import sys
import numpy as np

try:
    import concourse.bass as bass
except ImportError:
    sys.path.insert(0, "/opt/trn_rl_repo")
    import concourse.bass as bass

import concourse.bacc as bacc
import concourse.tile as tile
from concourse import mybir
from concourse import bass_utils

F16 = mybir.dt.float16
F32 = mybir.dt.float32
AF = mybir.ActivationFunctionType
ALU = mybir.AluOpType

B, T, N, FIN = 64, 256, 24, 6
H, EMB = 256, 128
WIN = 32
NW = (T - WIN) // 2 + 1               # 113
NCORES = 8
BL = B // NCORES                      # 8
G5 = 5
GBLK = 52                             # ceil(260/5): 52*5 = 260 t-slots
TP = GBLK * G5                        # 260 padded timesteps
NCH = BL
ROWS = BL * NW                        # 904
HROWS = ROWS // 2                     # 452
FTOT = BL * TP                        # 2080 F columns
HALVES = [(0, 28), (28, 24)]          # (gb offset, gb count) per unit

PREWRITE = os.environ.get("K_PREWRITE", "1") == "1"

_CACHE = {}


def _chunks(nf, step):
    return [(i, min(step, nf - i)) for i in range(0, nf, step)]


def _kernel_body(tc, io):
    nc = tc.nc
    from contextlib import ExitStack
    ctx = ExitStack()

    cons = ctx.enter_context(tc.tile_pool(name="cons", bufs=1))
    fpool = ctx.enter_context(tc.tile_pool(name="fpool", bufs=1))

    def load_const(name, shape, dt=F16):
        t = cons.tile(shape, dt, name=name)
        nc.sync.dma_start(t[:], io[name][:])
        return t

    mixM = load_const("mixM", [128, 128])
    w1 = load_const("w1", [FIN, 64])
    w2 = load_const("w2", [64, 128])
    w3 = load_const("w3", [128, 256])
    b1 = load_const("b1", [64, 1], F32)
    b2 = load_const("b2", [128, 1], F32)
    b3 = load_const("b3", [128, 2], F32)
    b4 = load_const("b4", [128, 2], F32)
    w4k = []
    for kt in range(2):
        t = cons.tile([128, 256], F16, name=f"w4k{kt}")
        nc.sync.dma_start(t[:], io["w4"][kt * 128:(kt + 1) * 128, :])
        w4k.append(t)

    def load_ktiles(name):
        ts = []
        for kt in range(2):
            t = cons.tile([128, 1024], F16, name=f"{name}{kt}")
            nc.sync.dma_start(t[:], io[name][kt * 128:(kt + 1) * 128, :])
            ts.append(t)
        return ts

    lxf = load_ktiles("lxf")
    lhf = load_ktiles("lhf")
    lxb = load_ktiles("lxb")
    bgf = load_const("bgf", [128, 8], F32)
    bgb = load_const("bgb", [128, 8], F32)
    wfct = []
    for qt in range(4):
        t = cons.tile([128, 128], F16, name=f"wfct{qt}")
        nc.sync.dma_start(t[:], io["wfc"][qt * 128:(qt + 1) * 128, :])
        wfct.append(t)
    bfc = load_const("bfc", [128, 1], F32)
    ident = load_const("ident", [128, 128])

    F0 = fpool.tile([128, FTOT], F16, name="F0")
    F1 = fpool.tile([128, FTOT], F16, name="F1")
    Fts = [F0, F1]

    # persistent zero-padded B-layout staging tiles (pad cols stay 0)
    y1pads = [cons.tile([128, 28 * 128], F16, name=f"y1p{i}") for i in range(2)]
    y2pads = [cons.tile([128, 28 * 128], F16, name=f"y2p{i}") for i in range(2)]
    for i in range(2):
        nc.vector.memset(y1pads[i][:], 0.0)
        nc.gpsimd.memset(y2pads[i][:], 0.0)

    # engine rotations
    cast_cycle = itertools.cycle([nc.vector, nc.vector, nc.scalar])
    relu_cycle = itertools.cycle([nc.scalar, nc.vector, nc.vector])
    # concurrent XBAR transposes on the two HWDGE rings corrupt each other
    # (shared xbar S2M state) -- keep ALL transposes on the sync ring.
    tp_cycle = itertools.cycle([nc.sync])

    def cast_rot(dst, src):
        e = next(cast_cycle)
        if e is nc.scalar:
            nc.scalar.copy(dst, src)
        else:
            e.tensor_copy(dst, src)

    def relu_rot(dst, src, bias_ap):
        e = next(relu_cycle)
        if e is nc.scalar:
            nc.scalar.activation(dst, src, AF.Relu, bias=bias_ap, scale=1.0)
        else:
            e.tensor_scalar(dst, src, bias_ap, 0.0, ALU.add, ALU.max)

    def tpose(out_ap, in_ap):
        next(tp_cycle).dma_start(out_ap, in_ap, transpose=True)

    # ================= Phase 1: GCN =================
    # Units are emitted stage-interleaved (software pipeline) so pool-buffer
    # rotation reuse targets recently-freed buffers instead of coupling each
    # unit's first stage to the previous unit's last.
    with tc.tile_pool(name="gcnS", bufs=4) as gpS, \
         tc.tile_pool(name="gcnA", bufs=9) as gpA, \
         tc.tile_pool(name="gcnB", bufs=5) as gpB, \
         tc.tile_pool(name="gcnBn", bufs=3) as gpBn, \
         tc.tile_pool(name="gcnBig", bufs=2) as gpBig, \
         tc.tile_pool(name="gps", bufs=4, space="PSUM") as ps_g:

        def mix(src_ap, n_free, cast_fn):
            """node-mix src_ap [128, n_free] via mixM; cast_fn(c0, cw, ps_ap)
            moves each fp32 psum chunk into its fp16 destination."""
            for c0, cw in _chunks(n_free, 1024):
                ps = ps_g.tile([128, 1024], F32, tag="ps", name="mps")
                for s0, sw in _chunks(cw, 512):
                    nc.tensor.matmul(ps[:, s0:s0 + sw], mixM[:],
                                     src_ap[:, c0 + s0:c0 + s0 + sw],
                                     start=True, stop=True)
                cast_fn(c0, cw, ps[:, 0:cw])

        def transform(rhs_list, wslices, bias, cout, out_tiles, FH, mtr):
            """dense channel transform for output c-tile mtr; bias+ReLU fused
            in the PSUM->SBUF move."""
            nkt = len(rhs_list)
            mp = min(cout, 128)
            for f0, fw in _chunks(FH, 1024):
                ps = ps_g.tile([128, 1024], F32, tag="ps", name="tps")
                for s0, sw in _chunks(fw, 512):
                    g0 = f0 + s0
                    for kt in range(nkt):
                        w = wslices[kt]
                        wap = w[:, mtr * 128:(mtr + 1) * 128] if cout > 128 \
                            else w
                        nc.tensor.matmul(ps[0:mp, s0:s0 + sw], wap,
                                         rhs_list[kt][:, g0:g0 + sw],
                                         start=(kt == 0),
                                         stop=(kt == nkt - 1))
                relu_rot(out_tiles[mtr][0:mp, f0:f0 + fw],
                         ps[0:mp, 0:fw], bias[0:mp, mtr:mtr + 1])

        def unit_stages(uidx):
            k, hf_i = uidx // 2, uidx % 2
            g0gb, gh = HALVES[hf_i]
            FH = gh * 128

            # --- L1: packed-c8 input, mix at c=8, narrow transform 6->64
            x0t = gpS.tile([128, 28 * 8], F16, tag="x0", name="x0t")
            nc.gpsimd.dma_start(x0t[:, 0:gh * 8],
                                io["x0"][k][:, g0gb * 8:(g0gb + gh) * 8])
            y1p = y1pads[uidx % 2]

            def cast_y1(c0, cw, ps_ap):
                cast_rot(y1p[:, 0:gh * 128]
                         .rearrange("p (gb c) -> p gb c", c=128)[:, :, 0:8],
                         ps_ap.rearrange("p (gb c) -> p gb c", c=8))

            mix(x0t[:, 0:gh * 8], gh * 8, cast_y1)
            yield
            y1a = gpA.tile([128, 28 * 128], F16, tag="cA", name="y1a")
            tpose(y1a[:, 0:FH].rearrange("c (gb p) -> c gb p", p=128),
                  y1p[:, 0:FH])
            yield
            x1a = gpA.tile([128, 28 * 128], F16, tag="cA", name="x1a")
            transform([y1a[0:FIN, 0:FH]], [w1[:]], b1, 64, [x1a], FH, 0)
            yield

            # --- L2: narrow A->B (64 c), mix at 64, transform 64->128
            x2b = gpBn.tile([128, 28 * 64], F16, tag="cBn", name="x2b")
            tpose(x2b[:, 0:gh * 64].rearrange("p (gb c) -> p gb c", c=64),
                  x1a[0:64, 0:FH])
            yield
            y2p = y2pads[uidx % 2]

            def cast_y2(c0, cw, ps_ap):
                cast_rot(y2p[:, 0:gh * 128]
                         .rearrange("p (gb c) -> p gb c", c=128)
                         [:, c0 // 64:(c0 + cw) // 64, 0:64],
                         ps_ap.rearrange("p (gb c) -> p gb c", c=64))

            mix(x2b[:, 0:gh * 64], gh * 64, cast_y2)
            yield
            y2a = gpA.tile([128, 28 * 128], F16, tag="cA", name="y2a")
            tpose(y2a[:, 0:FH].rearrange("c (gb p) -> c gb p", p=128),
                  y2p[:, 0:FH])
            yield
            x2a = gpA.tile([128, 28 * 128], F16, tag="cA", name="x2a")
            transform([y2a[0:64, 0:FH]], [w2[:]], b2, 128, [x2a], FH, 0)
            yield

            # --- L3: mix@128 -> transform 128->256
            x3b = gpB.tile([128, 28 * 128], F16, tag="cB", name="x3b")
            tpose(x3b[:, 0:FH].rearrange("p (gb c) -> p gb c", c=128),
                  x2a[:, 0:FH])
            yield
            y3b = gpB.tile([128, 28 * 128], F16, tag="cB", name="y3b")

            def cast_y3(c0, cw, ps_ap):
                cast_rot(y3b[:, c0:c0 + cw], ps_ap)

            mix(x3b[:, 0:FH], FH, cast_y3)
            yield
            y3a = gpA.tile([128, 28 * 128], F16, tag="cA", name="y3a")
            tpose(y3a[:, 0:FH].rearrange("c (gb p) -> c gb p", p=128),
                  y3b[:, 0:FH])
            yield
            x3a0 = gpA.tile([128, 28 * 128], F16, tag="cA", name="x3a0")
            x3a1 = gpA.tile([128, 28 * 128], F16, tag="cA", name="x3a1")
            transform([y3a[:, 0:FH]], [w3[:]], b3, 256, [x3a0, x3a1], FH, 0)
            yield
            transform([y3a[:, 0:FH]], [w3[:]], b3, 256, [x3a0, x3a1], FH, 1)
            yield

            # --- L4: mix@256 -> transform 256->256
            x4b = gpBig.tile([128, 28 * 256], F16, tag="big", name="x4b")
            x4bv = x4b[:, 0:gh * 256].rearrange("p (gb c) -> p gb c", c=256)
            tpose(x4bv[:, :, 0:128], x3a0[:, 0:FH])
            yield
            tpose(x4bv[:, :, 128:256], x3a1[:, 0:FH])
            yield
            ylo = gpB.tile([128, 28 * 128], F16, tag="cB", name="ylo")
            yhi = gpB.tile([128, 28 * 128], F16, tag="cB", name="yhi")

            def cast_y4(c0, cw, ps_ap):
                psv = ps_ap.rearrange("p (g c) -> p g c", c=256)
                g = cw // 256
                for hf in range(2):
                    dv = (ylo, yhi)[hf][:].rearrange(
                        "p (gb c) -> p gb c", c=128)
                    cast_rot(dv[:, c0 // 256:c0 // 256 + g, :],
                             psv[:, :, hf * 128:(hf + 1) * 128])

            mix(x4b[:, 0:gh * 256], gh * 256, cast_y4)
            yield
            y4a0 = gpA.tile([128, 28 * 128], F16, tag="cA", name="y4a0")
            y4a1 = gpA.tile([128, 28 * 128], F16, tag="cA", name="y4a1")
            tpose(y4a0[:, 0:FH].rearrange("c (gb p) -> c gb p", p=128),
                  ylo[:, 0:FH])
            yield
            tpose(y4a1[:, 0:FH].rearrange("c (gb p) -> c gb p", p=128),
                  yhi[:, 0:FH])
            yield
            x4a0 = gpA.tile([128, 28 * 128], F16, tag="cA", name="x4a0")
            x4a1 = gpA.tile([128, 28 * 128], F16, tag="cA", name="x4a1")
            transform([y4a0[:, 0:FH], y4a1[:, 0:FH]],
                      [w4k[0][:], w4k[1][:]], b4, 256, [x4a0, x4a1], FH, 0)
            yield
            transform([y4a0[:, 0:FH], y4a1[:, 0:FH]],
                      [w4k[0][:], w4k[1][:]], b4, 256, [x4a0, x4a1], FH, 1)
            yield

            # node-sum into F: F[:, k*TP + t], t = 5*(g0gb+gb) + g5
            for ct, xt in enumerate((x4a0, x4a1)):
                xv = (xt[:, 0:FH]
                      .rearrange("p (gb blk) -> p gb blk", blk=128)
                      [:, :, 0:120]
                      .rearrange("p gb (n g5) -> p gb g5 n", g5=G5))
                dstv = (Fts[ct][:, k * TP + g0gb * G5:
                                k * TP + (g0gb + gh) * G5]
                        .rearrange("p (gb g5) -> p gb g5", g5=G5))
                with nc.allow_low_precision("node-sum in fp16, as baseline"):
                    nc.vector.tensor_reduce(dstv, xv, mybir.AxisListType.X,
                                            ALU.add)

        # skewed round-robin driver: admit the next unit once the newest
        # active one is SKEW stages in; emit one stage per active unit.
        SKEW = 9
        gens = [unit_stages(u) for u in range(NCH * 2)]
        active, nxt, prog = [], 0, {}
        while active or nxt < len(gens):
            if nxt < len(gens) and (not active or prog[active[-1]] >= SKEW):
                active.append(nxt)
                prog[nxt] = 0
                nxt += 1
            for u in list(active):
                try:
                    next(gens[u])
                    prog[u] += 1
                except StopIteration:
                    active.remove(u)

    # ================= Phase 2: U = F @ (Wih_f/24)^T =================
    # U stored par-major: col = par*(BL*130) + b*130 + kk, where the source
    # F column is b*260 + 2*kk + par.  This gives the LSTM pre-writes a
    # unit-stride inner dim.
    upool = ctx.enter_context(tc.tile_pool(name="upool", bufs=1))
    KK = TP // 2                                  # 130
    Umt = []
    with tc.tile_pool(name="ups", bufs=4, space="PSUM") as ps_u:
        for mt in range(8):
            u = upool.tile([128, FTOT], F16, name=f"U{mt}")
            uview = u[:].rearrange("p (par b kk) -> p b kk par",
                                   par=2, kk=KK)
            for b in range(BL):
                ps = ps_u.tile([128, 512], F32, tag="ups", name="ups")
                for kt in range(2):
                    nc.tensor.matmul(ps[:, 0:TP],
                                     lxf[kt][:, mt * 128:(mt + 1) * 128],
                                     Fts[kt][:, b * TP:(b + 1) * TP],
                                     start=(kt == 0), stop=(kt == 1))
                cast_rot(uview[:, b],
                         ps[:, 0:TP].rearrange("p (kk par) -> p kk par",
                                               par=2))
            Umt.append(u)

    # ================= Phase 3: forward LSTM =================
    lp = ctx.enter_context(tc.tile_pool(name="lstm", bufs=1))
    Hf = lp.tile([128, 2 * ROWS], F16, name="Hf")
    Cf = lp.tile([128, 2 * ROWS], F16, name="Cf")
    nc.vector.memset(Hf[:], 0.0)
    nc.gpsimd.memset(Cf[:], 0.0)
    gi = lp.tile([128, 2 * ROWS], F16, name="gi")
    gf = lp.tile([128, 2 * ROWS], F16, name="gf")
    go = lp.tile([128, 2 * ROWS], F16, name="go")
    tg = lp.tile([128, 2 * ROWS], F16, name="tg")
    tcl = lp.tile([128, 2 * ROWS], F16, name="tcl")
    tmp = lp.tile([128, 2 * ROWS], F16, name="tmp")
    PAIRS = [(0, 1, gi, AF.Sigmoid), (2, 3, gf, AF.Sigmoid),
             (4, 5, go, AF.Sigmoid), (6, 7, tg, AF.Tanh)]

    with tc.tile_pool(name="lps", bufs=2, space="PSUM") as ps_l:
        for s in range(WIN):
            k0, par = s // 2, s % 2
            for ma, mb, gdst, fn in PAIRS:
                ps = ps_l.tile([128, 2048], F32, tag="lp", name="lp")
                for j, mt in ((0, ma), (1, mb)):
                    uv = Umt[mt][:].rearrange("p (par b kk) -> p par b kk",
                                              par=2, kk=KK)
                    if PREWRITE:
                        # pre-write U + gate bias into PSUM (vector engine)
                        src = (uv[:, par, :, k0:k0 + NW]
                               .rearrange("p (hh b) k -> p hh b k", hh=2))
                        dst = (ps[:, j * 1024:(j + 1) * 1024]
                               .rearrange("p (hh x) -> p hh x", hh=2)
                               [:, :, 0:HROWS]
                               .rearrange("p hh (b k) -> p hh b k", k=NW))
                        nc.vector.tensor_scalar(dst, src, bgf[:, mt:mt + 1],
                                                None, ALU.add)
                    for hh in range(2):
                        pslice = ps[:, j * 1024 + hh * 512:
                                    j * 1024 + hh * 512 + HROWS]
                        b0 = hh * (BL // 2)
                        if not PREWRITE:
                            nc.tensor.matmul(
                                pslice, ident[:],
                                uv[:, par, b0:b0 + BL // 2, k0:k0 + NW],
                                start=True, stop=False)
                        for kt in range(2):
                            nc.tensor.matmul(
                                pslice, lhf[kt][:, mt * 128:(mt + 1) * 128],
                                Hf[:, kt * ROWS + hh * HROWS:
                                   kt * ROWS + (hh + 1) * HROWS],
                                start=False, stop=(kt == 1),
                                skip_group_check=True)
                psq = ps[:].rearrange("p (q x) -> p q x", q=4)[:, :, 0:HROWS]
                gv = gdst[:].rearrange("p (q r) -> p q r", q=4)
                if PREWRITE:
                    nc.scalar.activation(gv, psq, fn, scale=1.0)
                else:
                    gva = gdst[:].rearrange("p (m x) -> p m x", m=2)
                    psa = ps[:].rearrange("p (m x) -> p m x", m=2)
                    for j, mt in ((0, ma), (1, mb)):
                        nc.scalar.activation(
                            gva[:, j, :].rearrange("p (hh r) -> p hh r",
                                                   hh=2),
                            psa[:, j, :].rearrange("p (hh x) -> p hh x",
                                                   hh=2)[:, :, 0:HROWS],
                            fn, bias=bgf[:, mt:mt + 1], scale=1.0)
            nc.gpsimd.tensor_tensor(tmp[:], gi[:], tg[:], ALU.mult)
            nc.vector.tensor_tensor(Cf[:], gf[:], Cf[:], ALU.mult)
            nc.vector.tensor_tensor(Cf[:], Cf[:], tmp[:], ALU.add)
            nc.scalar.activation(tcl[:], Cf[:], AF.Tanh)
            nc.vector.tensor_tensor(Hf[:], go[:], tcl[:], ALU.mult)

        # ===== Phase 4: backward LSTM single step (only hb[:,0] used) =====
        Hb = lp.tile([128, 2 * ROWS], F16, name="Hb")
        kb = (WIN - 2) // 2
        BPAIRS = [(0, 1, gi, AF.Sigmoid), (4, 5, go, AF.Sigmoid),
                  (6, 7, tg, AF.Tanh)]
        for ma, mb, gdst, fn in BPAIRS:
            ps = ps_l.tile([128, 2048], F32, tag="lp", name="lpb")
            for j, mt in ((0, ma), (1, mb)):
                for hh in range(2):
                    pslice = ps[:, j * 1024 + hh * 512:
                                j * 1024 + hh * 512 + HROWS]
                    b0 = hh * (BL // 2)
                    for kt in range(2):
                        fv = Fts[kt][:].rearrange("p (b k two) -> p b k two",
                                                  b=BL, two=2)
                        nc.tensor.matmul(
                            pslice, lxb[kt][:, mt * 128:(mt + 1) * 128],
                            fv[:, b0:b0 + BL // 2, kb:kb + NW, 1],
                            start=(kt == 0), stop=(kt == 1))
                psj = (ps[:, j * 1024:(j + 1) * 1024]
                       .rearrange("p (hh x) -> p hh x", hh=2)[:, :, 0:HROWS])
                gvj = (gdst[:, j * ROWS:(j + 1) * ROWS]
                       .rearrange("p (hh r) -> p hh r", hh=2))
                nc.scalar.activation(gvj, psj, fn,
                                     bias=bgb[:, mt:mt + 1], scale=1.0)
        nc.gpsimd.tensor_tensor(tmp[:], gi[:], tg[:], ALU.mult)
        nc.scalar.activation(tcl[:], tmp[:], AF.Tanh)
        nc.gpsimd.tensor_tensor(Hb[:], go[:], tcl[:], ALU.mult)

        # ===== Phase 5: FC head =====
        ps = ps_l.tile([128, 2048], F32, tag="lp", name="lpf")
        rhs4 = [Hf[:, 0:ROWS], Hf[:, ROWS:2 * ROWS],
                Hb[:, 0:ROWS], Hb[:, ROWS:2 * ROWS]]
        for hh in range(2):
            for qt in range(4):
                nc.tensor.matmul(ps[:, hh * 512:hh * 512 + HROWS],
                                 wfct[qt][:],
                                 rhs4[qt].rearrange("p (h r) -> p h r",
                                                    h=2)[:, hh, :],
                                 start=(qt == 0), stop=(qt == 3))
        ob = lp.tile([EMB, ROWS], F32, name="ob")
        obv = ob[:].rearrange("p (h r) -> p h r", h=2)
        psv = (ps[:, 0:1024].rearrange("p (h x) -> p h x", h=2)
               [:, :, 0:HROWS])
        nc.scalar.activation(obv, psv, AF.Identity,
                             bias=bfc[:, 0:1], scale=1.0)
        nc.sync.dma_start(io["out_d"][:], ob[:])

    if "fdbg0" in io:
        nc.sync.dma_start(io["fdbg0"][:], F0[:])
        nc.sync.dma_start(io["fdbg1"][:], F1[:])
    ctx.close()


def _build_program():
    nc = bacc.Bacc("TRN2", target_bir_lowering=False, debug=False,
                   num_devices=NCORES)

    def din(name, shape, dt=F16):
        return nc.dram_tensor(name, shape, dt, kind="ExternalInput").ap()

    io = dict(
        x0=din("x0", [NCH, 128, GBLK * 8]),
        mixM=din("mixM", [128, 128]),
        w1=din("w1", [FIN, 64]), w2=din("w2", [64, 128]),
        w3=din("w3", [128, 256]), w4=din("w4", [256, 256]),
        b1=din("b1", [64, 1], F32), b2=din("b2", [128, 1], F32),
        b3=din("b3", [128, 2], F32), b4=din("b4", [128, 2], F32),
        lxf=din("lxf", [256, 1024]), lhf=din("lhf", [256, 1024]),
        lxb=din("lxb", [256, 1024]),
        bgf=din("bgf", [128, 8], F32), bgb=din("bgb", [128, 8], F32),
        wfc=din("wfc", [512, 128]), bfc=din("bfc", [128, 1], F32),
        ident=din("ident", [128, 128]),
        out_d=nc.dram_tensor("out", [EMB, ROWS], F32,
                             kind="ExternalOutput").ap(),
    )
    if os.environ.get("K_FDBG", "0") == "1":
        io["fdbg0"] = nc.dram_tensor("fdbg0", [128, FTOT], F16,
                                     kind="ExternalOutput").ap()
        io["fdbg1"] = nc.dram_tensor("fdbg1", [128, FTOT], F16,
                                     kind="ExternalOutput").ap()

    with tile.TileContext(nc) as tc:
        _kernel_body(tc, io)
    nc.compile()
    return nc


def _host_prep(inputs):
    f16 = np.float16
    data = np.asarray(inputs["data"], np.float32)
    ei = np.asarray(inputs["edge_index"]).astype(np.int64)

    src = np.concatenate([ei[0], np.arange(N)])
    dst = np.concatenate([ei[1], np.arange(N)])
    deg = np.zeros(N, np.float32)
    np.add.at(deg, dst, 1.0)
    dinv = np.where(deg > 0, deg ** -0.5, 0.0).astype(np.float32)
    Ahat = np.zeros((N, N), np.float32)
    np.add.at(Ahat, (dst, src), dinv[src] * dinv[dst])
    mixM = np.zeros((128, 128), np.float32)
    mixM[0:N * G5, 0:N * G5] = np.kron(Ahat.T, np.eye(G5, dtype=np.float32))
    mixM = mixM.astype(f16)

    # x0: [core][chunk b][blk = n*5+g5 (120:128 zero)][gb*8 + c],
    # t = 5*gb+g5, channels 6:8 zero
    d = data.reshape(NCORES, BL, T, N, FIN)
    x0 = np.zeros((NCORES, BL, 128, GBLK, 8), np.float32)
    dpad = np.zeros((NCORES, BL, TP, N, FIN), np.float32)
    dpad[:, :, :T] = d
    dv = dpad.reshape(NCORES, BL, GBLK, G5, N, FIN)
    # [core, b, n, g5, gb, c]
    dv = dv.transpose(0, 1, 4, 3, 2, 5).reshape(NCORES, BL, N * G5, GBLK, FIN)
    x0[:, :, 0:N * G5, :, 0:FIN] = dv
    x0 = np.ascontiguousarray(
        x0.reshape(NCORES, BL, 128, GBLK * 8)).astype(f16)

    perm = np.concatenate([np.arange(0, H), np.arange(H, 2 * H),
                           np.arange(3 * H, 4 * H), np.arange(2 * H, 3 * H)])

    def prep_dir(wih, whh, bih, bhh):
        wihp = np.asarray(wih, np.float32)[perm] / N
        whhp = np.asarray(whh, np.float32)[perm]
        bg = (np.asarray(bih, np.float32) + np.asarray(bhh, np.float32))[perm]
        return (np.ascontiguousarray(wihp.T).astype(f16),
                np.ascontiguousarray(whhp.T).astype(f16),
                np.ascontiguousarray(bg.reshape(8, 128).T).astype(np.float32))

    lxf, lhf, bgf = prep_dir(inputs["lstm_Wih_f"], inputs["lstm_Whh_f"],
                             inputs["lstm_bih_f"], inputs["lstm_bhh_f"])
    lxb, _lhb, bgb = prep_dir(inputs["lstm_Wih_b"], inputs["lstm_Whh_b"],
                              inputs["lstm_bih_b"], inputs["lstm_bhh_b"])

    com = {
        "mixM": mixM,
        "w1": np.asarray(inputs["W1"], np.float32).astype(f16),
        "w2": np.asarray(inputs["W2"], np.float32).astype(f16),
        "w3": np.asarray(inputs["W3"], np.float32).astype(f16),
        "w4": np.asarray(inputs["W4"], np.float32).astype(f16),
        "b1": np.asarray(inputs["b1"], np.float32).reshape(64, 1),
        "b2": np.asarray(inputs["b2"], np.float32).reshape(128, 1),
        "b3": np.ascontiguousarray(
            np.asarray(inputs["b3"], np.float32).reshape(2, 128).T),
        "b4": np.ascontiguousarray(
            np.asarray(inputs["b4"], np.float32).reshape(2, 128).T),
        "lxf": lxf, "lhf": lhf, "lxb": lxb, "bgf": bgf, "bgb": bgb,
        "wfc": np.asarray(inputs["Wfc"], np.float32).astype(f16),
        "bfc": np.asarray(inputs["bfc"], np.float32).reshape(128, 1),
        "ident": np.eye(128, dtype=f16),
    }
    return [dict(com, x0=x0[c]) for c in range(NCORES)]


TRACE = False          # set by test harness to capture an NTFF profile


def kernel(**inputs) -> np.ndarray:
    if "nc" not in _CACHE:
        _CACHE["nc"] = _build_program()
    nc = _CACHE["nc"]
    in_maps = _host_prep(inputs)
    res = bass_utils.run_bass_kernel_spmd(nc, in_maps,
                                          core_ids=list(range(NCORES)),
                                          trace=TRACE)
    _CACHE["last_res"] = res
    outs = []
    for c in range(NCORES):
        o = res.results[c]["out"]                       # [128, 904]
        outs.append(o.reshape(EMB, BL, NW).transpose(1, 2, 0))
    return np.concatenate(outs, 0).astype(np.float32)   # [64, 113, 128]


if __name__ == "__main__":
    import reference
    ins = {k: np.asarray(v) for k, v in reference.setup_inputs().items()}
    out = kernel(**ins)
    print("kernel out", out.shape, out.dtype, float(np.abs(out).max()))


# revision 21
# speedup vs baseline: 1.4824x; 1.1024x over previous
"""Trainium2 Bass kernel for DeepConvGraphEncoderDownstream.

Model (per reference):
  4-layer GCN (shared dense 24x24 graph operator) applied per (batch, timestep)
  frame -> node-mean -> per sliding window (W=32, stride 2, 113 windows):
  BiLSTM(H=256) -> concat(h_fwd[-1], h_bwd[0]) @ Wfc + bfc.

Key algebraic restructurings:
  * gcn_norm folded into one dense Ahat[24,24] on host.
  * GCN runs ONCE over all 256 timesteps (the reference recomputes it ~14x
    across overlapping windows).
  * backward LSTM: only hb[:, 0] is used => exactly ONE step, no recurrence.
  * forward LSTM: all 113 windows batched into one 904-row recurrence per
    core; input transforms U precomputed from node-mean features.

Sharding: data-parallel over batch, 8 batches/core on 8 cores; output
slices are independent (no collectives).

v2 performance structure:
  * GCN processed in 16 half-chunk units (28/24 gb groups) for cross-unit
    pipelining; pools sized for ~1 unit of overlap.
  * DMA transposes round-robin over the two HWDGE rings (sync 5/6,
    scalar 1/6); x0 loads on gpsimd SWDGE.
  * PSUM->SBUF casts and bias+ReLU rotate between vector and scalar.
  * x0 packed at c=8 (6 real channels) => mix L1 is 224 cols, not 3584;
    L1->L2 transpose narrow (64 channels).
  * node-sum via single strided tensor_reduce per (kt, unit).
  * LSTM: no identity matmuls -- U+bias pre-written into PSUM by vector,
    Whh matmuls accumulate on top (start=False); activations batched per
    gate pair; elementwise state update on gpsimd.

Layouts (per core, per chunk = one local batch = 256 timesteps padded to
260 = 52 blocks * 5):
  A-layout [c_part, free=(gb, blk:128)], blk = n*5+g5 (120:128 pad),
           timestep t = 5*gb + g5.
  B-layout [blk partitions = 128, free=(gb, c)]
  A->B / B->A are single XBAR DMA-transpose instructions per c-block:
  HW semantics out[p, b, c] = in[c, b*128 + p].
  Node mixing = matmul with zero-padded stationary kron(Ahat^T, I5) [128,128].
"""

import itertools
import os
import sys
import numpy as np

try:
    import concourse.bass as bass
except ImportError:
    sys.path.insert(0, "/opt/trn_rl_repo")
    import concourse.bass as bass

import concourse.bacc as bacc
import concourse.tile as tile
from concourse import mybir
from concourse import bass_utils

F16 = mybir.dt.float16
F32 = mybir.dt.float32
AF = mybir.ActivationFunctionType
ALU = mybir.AluOpType

B, T, N, FIN = 64, 256, 24, 6
H, EMB = 256, 128
WIN = 32
NW = (T - WIN) // 2 + 1               # 113
NCORES = 8
BL = B // NCORES                      # 8
G5 = 5
GBLK = 52                             # ceil(260/5): 52*5 = 260 t-slots
TP = GBLK * G5                        # 260 padded timesteps
NCH = BL
ROWS = BL * NW                        # 904
HROWS = ROWS // 2                     # 452
FTOT = BL * TP                        # 2080 F columns
HALVES = [(0, 28), (28, 24)]          # (gb offset, gb count) per unit

PREWRITE = os.environ.get("K_PREWRITE", "1") == "1"

_CACHE = {}


def _chunks(nf, step):
    return [(i, min(step, nf - i)) for i in range(0, nf, step)]


def _kernel_body(tc, io):
    nc = tc.nc
    from contextlib import ExitStack
    ctx = ExitStack()

    cons = ctx.enter_context(tc.tile_pool(name="cons", bufs=1))
    fpool = ctx.enter_context(tc.tile_pool(name="fpool", bufs=1))

    def load_const(name, shape, dt=F16):
        t = cons.tile(shape, dt, name=name)
        nc.sync.dma_start(t[:], io[name][:])
        return t

    mixM = load_const("mixM", [128, 128])
    w1 = load_const("w1", [FIN, 64])
    w2 = load_const("w2", [64, 128])
    w3 = load_const("w3", [128, 256])
    b1 = load_const("b1", [64, 1], F32)
    b2 = load_const("b2", [128, 1], F32)
    b3 = load_const("b3", [128, 2], F32)
    b4 = load_const("b4", [128, 2], F32)
    w4k = []
    for kt in range(2):
        t = cons.tile([128, 256], F16, name=f"w4k{kt}")
        nc.sync.dma_start(t[:], io["w4"][kt * 128:(kt + 1) * 128, :])
        w4k.append(t)

    def load_ktiles(name):
        ts = []
        for kt in range(2):
            t = cons.tile([128, 1024], F16, name=f"{name}{kt}")
            nc.sync.dma_start(t[:], io[name][kt * 128:(kt + 1) * 128, :])
            ts.append(t)
        return ts

    lxf = load_ktiles("lxf")
    lhf = load_ktiles("lhf")
    lxb = load_ktiles("lxb")
    bgf = load_const("bgf", [128, 8], F32)
    bgb = load_const("bgb", [128, 8], F32)
    wfct = []
    for qt in range(4):
        t = cons.tile([128, 128], F16, name=f"wfct{qt}")
        nc.sync.dma_start(t[:], io["wfc"][qt * 128:(qt + 1) * 128, :])
        wfct.append(t)
    bfc = load_const("bfc", [128, 1], F32)
    ident = load_const("ident", [128, 128])

    F0 = fpool.tile([128, FTOT], F16, name="F0")
    F1 = fpool.tile([128, FTOT], F16, name="F1")
    Fts = [F0, F1]

    # persistent zero-padded B-layout staging tiles (pad cols stay 0)
    y1pads = [cons.tile([128, 28 * 128], F16, name=f"y1p{i}") for i in range(2)]
    y2pads = [cons.tile([128, 28 * 128], F16, name=f"y2p{i}") for i in range(2)]
    for i in range(2):
        nc.vector.memset(y1pads[i][:], 0.0)
        nc.gpsimd.memset(y2pads[i][:], 0.0)

    # engine rotations
    cast_cycle = itertools.cycle([nc.vector, nc.scalar])
    relu_cycle = itertools.cycle([nc.scalar, nc.vector])
    # concurrent XBAR transposes on the two HWDGE rings corrupt each other
    # (shared xbar S2M state) -- keep ALL transposes on the sync ring.
    tp_cycle = itertools.cycle([nc.sync])

    def cast_rot(dst, src):
        e = next(cast_cycle)
        if e is nc.scalar:
            nc.scalar.copy(dst, src)
        else:
            e.tensor_copy(dst, src)

    def relu_rot(dst, src, bias_ap):
        e = next(relu_cycle)
        if e is nc.scalar:
            nc.scalar.activation(dst, src, AF.Relu, bias=bias_ap, scale=1.0)
        else:
            e.tensor_scalar(dst, src, bias_ap, 0.0, ALU.add, ALU.max)

    def tpose(out_ap, in_ap):
        next(tp_cycle).dma_start(out_ap, in_ap, transpose=True)

    # ================= Phase 1: GCN =================
    # Units are emitted stage-interleaved (software pipeline) so pool-buffer
    # rotation reuse targets recently-freed buffers instead of coupling each
    # unit's first stage to the previous unit's last.
    with tc.tile_pool(name="gcnS", bufs=4) as gpS, \
         tc.tile_pool(name="gcnA", bufs=9) as gpA, \
         tc.tile_pool(name="gcnB", bufs=5) as gpB, \
         tc.tile_pool(name="gcnBn", bufs=3) as gpBn, \
         tc.tile_pool(name="gcnBig", bufs=2) as gpBig, \
         tc.tile_pool(name="gps", bufs=4, space="PSUM") as ps_g:

        def mix(src_ap, n_free, cast_fn):
            """node-mix src_ap [128, n_free] via mixM; cast_fn(c0, cw, ps_ap)
            moves each fp32 psum chunk into its fp16 destination."""
            for c0, cw in _chunks(n_free, 1024):
                ps = ps_g.tile([128, 1024], F32, tag="ps", name="mps")
                for s0, sw in _chunks(cw, 512):
                    nc.tensor.matmul(ps[:, s0:s0 + sw], mixM[:],
                                     src_ap[:, c0 + s0:c0 + s0 + sw],
                                     start=True, stop=True)
                cast_fn(c0, cw, ps[:, 0:cw])

        def transform(rhs_list, wslices, bias, cout, out_tiles, FH, mtr):
            """dense channel transform for output c-tile mtr; bias+ReLU fused
            in the PSUM->SBUF move."""
            nkt = len(rhs_list)
            mp = min(cout, 128)
            for f0, fw in _chunks(FH, 1024):
                ps = ps_g.tile([128, 1024], F32, tag="ps", name="tps")
                for s0, sw in _chunks(fw, 512):
                    g0 = f0 + s0
                    for kt in range(nkt):
                        w = wslices[kt]
                        wap = w[:, mtr * 128:(mtr + 1) * 128] if cout > 128 \
                            else w
                        nc.tensor.matmul(ps[0:mp, s0:s0 + sw], wap,
                                         rhs_list[kt][:, g0:g0 + sw],
                                         start=(kt == 0),
                                         stop=(kt == nkt - 1))
                relu_rot(out_tiles[mtr][0:mp, f0:f0 + fw],
                         ps[0:mp, 0:fw], bias[0:mp, mtr:mtr + 1])

        def unit_stages(uidx):
            k, hf_i = uidx // 2, uidx % 2
            g0gb, gh = HALVES[hf_i]
            FH = gh * 128

            # --- L1: packed-c8 input, mix at c=8, narrow transform 6->64
            x0t = gpS.tile([128, 28 * 8], F16, tag="x0", name="x0t")
            nc.gpsimd.dma_start(x0t[:, 0:gh * 8],
                                io["x0"][k][:, g0gb * 8:(g0gb + gh) * 8])
            y1p = y1pads[uidx % 2]

            def cast_y1(c0, cw, ps_ap):
                cast_rot(y1p[:, 0:gh * 128]
                         .rearrange("p (gb c) -> p gb c", c=128)[:, :, 0:8],
                         ps_ap.rearrange("p (gb c) -> p gb c", c=8))

            mix(x0t[:, 0:gh * 8], gh * 8, cast_y1)
            yield
            y1a = gpA.tile([128, 28 * 128], F16, tag="cA", name="y1a")
            tpose(y1a[:, 0:FH].rearrange("c (gb p) -> c gb p", p=128),
                  y1p[:, 0:FH])
            yield
            x1a = gpA.tile([128, 28 * 128], F16, tag="cA", name="x1a")
            transform([y1a[0:FIN, 0:FH]], [w1[:]], b1, 64, [x1a], FH, 0)
            yield

            # --- L2: narrow A->B (64 c), mix at 64, transform 64->128
            x2b = gpBn.tile([128, 28 * 64], F16, tag="cBn", name="x2b")
            tpose(x2b[:, 0:gh * 64].rearrange("p (gb c) -> p gb c", c=64),
                  x1a[0:64, 0:FH])
            yield
            y2p = y2pads[uidx % 2]

            def cast_y2(c0, cw, ps_ap):
                cast_rot(y2p[:, 0:gh * 128]
                         .rearrange("p (gb c) -> p gb c", c=128)
                         [:, c0 // 64:(c0 + cw) // 64, 0:64],
                         ps_ap.rearrange("p (gb c) -> p gb c", c=64))

            mix(x2b[:, 0:gh * 64], gh * 64, cast_y2)
            yield
            y2a = gpA.tile([128, 28 * 128], F16, tag="cA", name="y2a")
            tpose(y2a[:, 0:FH].rearrange("c (gb p) -> c gb p", p=128),
                  y2p[:, 0:FH])
            yield
            x2a = gpA.tile([128, 28 * 128], F16, tag="cA", name="x2a")
            transform([y2a[0:64, 0:FH]], [w2[:]], b2, 128, [x2a], FH, 0)
            yield

            # --- L3: mix@128 -> transform 128->256
            x3b = gpB.tile([128, 28 * 128], F16, tag="cB", name="x3b")
            tpose(x3b[:, 0:FH].rearrange("p (gb c) -> p gb c", c=128),
                  x2a[:, 0:FH])
            yield
            y3b = gpB.tile([128, 28 * 128], F16, tag="cB", name="y3b")

            def cast_y3(c0, cw, ps_ap):
                cast_rot(y3b[:, c0:c0 + cw], ps_ap)

            mix(x3b[:, 0:FH], FH, cast_y3)
            yield
            y3a = gpA.tile([128, 28 * 128], F16, tag="cA", name="y3a")
            tpose(y3a[:, 0:FH].rearrange("c (gb p) -> c gb p", p=128),
                  y3b[:, 0:FH])
            yield
            x3a0 = gpA.tile([128, 28 * 128], F16, tag="cA", name="x3a0")
            x3a1 = gpA.tile([128, 28 * 128], F16, tag="cA", name="x3a1")
            transform([y3a[:, 0:FH]], [w3[:]], b3, 256, [x3a0, x3a1], FH, 0)
            yield
            transform([y3a[:, 0:FH]], [w3[:]], b3, 256, [x3a0, x3a1], FH, 1)
            yield

            # --- L4: mix@256 -> transform 256->256
            x4b = gpBig.tile([128, 28 * 256], F16, tag="big", name="x4b")
            x4bv = x4b[:, 0:gh * 256].rearrange("p (gb c) -> p gb c", c=256)
            tpose(x4bv[:, :, 0:128], x3a0[:, 0:FH])
            yield
            tpose(x4bv[:, :, 128:256], x3a1[:, 0:FH])
            yield
            ylo = gpB.tile([128, 28 * 128], F16, tag="cB", name="ylo")
            yhi = gpB.tile([128, 28 * 128], F16, tag="cB", name="yhi")

            def cast_y4(c0, cw, ps_ap):
                psv = ps_ap.rearrange("p (g c) -> p g c", c=256)
                g = cw // 256
                for hf in range(2):
                    dv = (ylo, yhi)[hf][:].rearrange(
                        "p (gb c) -> p gb c", c=128)
                    cast_rot(dv[:, c0 // 256:c0 // 256 + g, :],
                             psv[:, :, hf * 128:(hf + 1) * 128])

            mix(x4b[:, 0:gh * 256], gh * 256, cast_y4)
            yield
            y4a0 = gpA.tile([128, 28 * 128], F16, tag="cA", name="y4a0")
            y4a1 = gpA.tile([128, 28 * 128], F16, tag="cA", name="y4a1")
            tpose(y4a0[:, 0:FH].rearrange("c (gb p) -> c gb p", p=128),
                  ylo[:, 0:FH])
            yield
            tpose(y4a1[:, 0:FH].rearrange("c (gb p) -> c gb p", p=128),
                  yhi[:, 0:FH])
            yield
            x4a0 = gpA.tile([128, 28 * 128], F16, tag="cA", name="x4a0")
            x4a1 = gpA.tile([128, 28 * 128], F16, tag="cA", name="x4a1")
            transform([y4a0[:, 0:FH], y4a1[:, 0:FH]],
                      [w4k[0][:], w4k[1][:]], b4, 256, [x4a0, x4a1], FH, 0)
            yield
            transform([y4a0[:, 0:FH], y4a1[:, 0:FH]],
                      [w4k[0][:], w4k[1][:]], b4, 256, [x4a0, x4a1], FH, 1)
            yield

            # node-sum into F: F[:, k*TP + t], t = 5*(g0gb+gb) + g5
            for ct, xt in enumerate((x4a0, x4a1)):
                xv = (xt[:, 0:FH]
                      .rearrange("p (gb blk) -> p gb blk", blk=128)
                      [:, :, 0:120]
                      .rearrange("p gb (n g5) -> p gb g5 n", g5=G5))
                dstv = (Fts[ct][:, k * TP + g0gb * G5:
                                k * TP + (g0gb + gh) * G5]
                        .rearrange("p (gb g5) -> p gb g5", g5=G5))
                with nc.allow_low_precision("node-sum in fp16, as baseline"):
                    nc.vector.tensor_reduce(dstv, xv, mybir.AxisListType.X,
                                            ALU.add)

        # skewed round-robin driver: admit the next unit once the newest
        # active one is SKEW stages in; emit one stage per active unit.
        SKEW = 9
        gens = [unit_stages(u) for u in range(NCH * 2)]
        active, nxt, prog = [], 0, {}
        while active or nxt < len(gens):
            if nxt < len(gens) and (not active or prog[active[-1]] >= SKEW):
                active.append(nxt)
                prog[nxt] = 0
                nxt += 1
            for u in list(active):
                try:
                    next(gens[u])
                    prog[u] += 1
                except StopIteration:
                    active.remove(u)

    # ================= Phase 2: U = F @ (Wih_f/24)^T =================
    # U stored par-major: col = par*(BL*130) + b*130 + kk, where the source
    # F column is b*260 + 2*kk + par.  This gives the LSTM pre-writes a
    # unit-stride inner dim.
    upool = ctx.enter_context(tc.tile_pool(name="upool", bufs=1))
    KK = TP // 2                                  # 130
    Umt = []
    with tc.tile_pool(name="ups", bufs=4, space="PSUM") as ps_u:
        for mt in range(8):
            u = upool.tile([128, FTOT], F16, name=f"U{mt}")
            uview = u[:].rearrange("p (par b kk) -> p b kk par",
                                   par=2, kk=KK)
            for b in range(BL):
                ps = ps_u.tile([128, 512], F32, tag="ups", name="ups")
                for kt in range(2):
                    nc.tensor.matmul(ps[:, 0:TP],
                                     lxf[kt][:, mt * 128:(mt + 1) * 128],
                                     Fts[kt][:, b * TP:(b + 1) * TP],
                                     start=(kt == 0), stop=(kt == 1))
                cast_rot(uview[:, b],
                         ps[:, 0:TP].rearrange("p (kk par) -> p kk par",
                                               par=2))
            Umt.append(u)

    # ================= Phase 3: forward LSTM =================
    lp = ctx.enter_context(tc.tile_pool(name="lstm", bufs=1))
    Hf = lp.tile([128, 2 * ROWS], F16, name="Hf")
    Cf = lp.tile([128, 2 * ROWS], F16, name="Cf")
    nc.vector.memset(Hf[:], 0.0)
    nc.gpsimd.memset(Cf[:], 0.0)
    gi = lp.tile([128, 2 * ROWS], F16, name="gi")
    gf = lp.tile([128, 2 * ROWS], F16, name="gf")
    go = lp.tile([128, 2 * ROWS], F16, name="go")
    tg = lp.tile([128, 2 * ROWS], F16, name="tg")
    tcl = lp.tile([128, 2 * ROWS], F16, name="tcl")
    tmp = lp.tile([128, 2 * ROWS], F16, name="tmp")
    # pair order (g, i, f, o): the c/h elementwise update interleaves with
    # the later pairs instead of forming a serial tail after all four.
    PAIRS = [(6, 7, tg, AF.Tanh), (0, 1, gi, AF.Sigmoid),
             (2, 3, gf, AF.Sigmoid), (4, 5, go, AF.Sigmoid)]

    with tc.tile_pool(name="lps", bufs=2, space="PSUM") as ps_l:
        for s in range(WIN):
            k0, par = s // 2, s % 2
            for pi, (ma, mb, gdst, fn) in enumerate(PAIRS):
                ps = ps_l.tile([128, 2048], F32, tag="lp", name="lp")
                for j, mt in ((0, ma), (1, mb)):
                    uv = Umt[mt][:].rearrange("p (par b kk) -> p par b kk",
                                              par=2, kk=KK)
                    if PREWRITE:
                        # pre-write U + gate bias into PSUM; j1 of odd pairs
                        # goes to scalar to offload vector
                        src = (uv[:, par, :, k0:k0 + NW]
                               .rearrange("p (hh b) k -> p hh b k", hh=2))
                        dst = (ps[:, j * 1024:(j + 1) * 1024]
                               .rearrange("p (hh x) -> p hh x", hh=2)
                               [:, :, 0:HROWS]
                               .rearrange("p hh (b k) -> p hh b k", k=NW))
                        if j == 1 and pi % 2 == 1:
                            nc.scalar.activation(dst, src, AF.Identity,
                                                 bias=bgf[:, mt:mt + 1],
                                                 scale=1.0)
                        else:
                            nc.vector.tensor_scalar(dst, src,
                                                    bgf[:, mt:mt + 1],
                                                    None, ALU.add)
                    for hh in range(2):
                        pslice = ps[:, j * 1024 + hh * 512:
                                    j * 1024 + hh * 512 + HROWS]
                        b0 = hh * (BL // 2)
                        if not PREWRITE:
                            nc.tensor.matmul(
                                pslice, ident[:],
                                uv[:, par, b0:b0 + BL // 2, k0:k0 + NW],
                                start=True, stop=False)
                        for kt in range(2):
                            nc.tensor.matmul(
                                pslice, lhf[kt][:, mt * 128:(mt + 1) * 128],
                                Hf[:, kt * ROWS + hh * HROWS:
                                   kt * ROWS + (hh + 1) * HROWS],
                                start=False, stop=(kt == 1),
                                skip_group_check=True)
                psq = ps[:].rearrange("p (q x) -> p q x", q=4)[:, :, 0:HROWS]
                gv = gdst[:].rearrange("p (q r) -> p q r", q=4)
                if PREWRITE:
                    nc.scalar.activation(gv, psq, fn, scale=1.0)
                else:
                    gva = gdst[:].rearrange("p (m x) -> p m x", m=2)
                    psa = ps[:].rearrange("p (m x) -> p m x", m=2)
                    for j, mt in ((0, ma), (1, mb)):
                        nc.scalar.activation(
                            gva[:, j, :].rearrange("p (hh r) -> p hh r",
                                                   hh=2),
                            psa[:, j, :].rearrange("p (hh x) -> p hh x",
                                                   hh=2)[:, :, 0:HROWS],
                            fn, bias=bgf[:, mt:mt + 1], scale=1.0)
                if pi == 1:
                    nc.gpsimd.tensor_tensor(tmp[:], gi[:], tg[:], ALU.mult)
                elif pi == 2:
                    nc.vector.tensor_tensor(Cf[:], gf[:], Cf[:], ALU.mult)
                    nc.vector.tensor_tensor(Cf[:], Cf[:], tmp[:], ALU.add)
                    nc.scalar.activation(tcl[:], Cf[:], AF.Tanh)
            nc.vector.tensor_tensor(Hf[:], go[:], tcl[:], ALU.mult)

        # ===== Phase 4: backward LSTM single step (only hb[:,0] used) =====
        Hb = lp.tile([128, 2 * ROWS], F16, name="Hb")
        kb = (WIN - 2) // 2
        BPAIRS = [(0, 1, gi, AF.Sigmoid), (4, 5, go, AF.Sigmoid),
                  (6, 7, tg, AF.Tanh)]
        for ma, mb, gdst, fn in BPAIRS:
            ps = ps_l.tile([128, 2048], F32, tag="lp", name="lpb")
            for j, mt in ((0, ma), (1, mb)):
                for hh in range(2):
                    pslice = ps[:, j * 1024 + hh * 512:
                                j * 1024 + hh * 512 + HROWS]
                    b0 = hh * (BL // 2)
                    for kt in range(2):
                        fv = Fts[kt][:].rearrange("p (b k two) -> p b k two",
                                                  b=BL, two=2)
                        nc.tensor.matmul(
                            pslice, lxb[kt][:, mt * 128:(mt + 1) * 128],
                            fv[:, b0:b0 + BL // 2, kb:kb + NW, 1],
                            start=(kt == 0), stop=(kt == 1))
                psj = (ps[:, j * 1024:(j + 1) * 1024]
                       .rearrange("p (hh x) -> p hh x", hh=2)[:, :, 0:HROWS])
                gvj = (gdst[:, j * ROWS:(j + 1) * ROWS]
                       .rearrange("p (hh r) -> p hh r", hh=2))
                nc.scalar.activation(gvj, psj, fn,
                                     bias=bgb[:, mt:mt + 1], scale=1.0)
        nc.gpsimd.tensor_tensor(tmp[:], gi[:], tg[:], ALU.mult)
        nc.scalar.activation(tcl[:], tmp[:], AF.Tanh)
        nc.gpsimd.tensor_tensor(Hb[:], go[:], tcl[:], ALU.mult)

        # ===== Phase 5: FC head =====
        ps = ps_l.tile([128, 2048], F32, tag="lp", name="lpf")
        rhs4 = [Hf[:, 0:ROWS], Hf[:, ROWS:2 * ROWS],
                Hb[:, 0:ROWS], Hb[:, ROWS:2 * ROWS]]
        for hh in range(2):
            for qt in range(4):
                nc.tensor.matmul(ps[:, hh * 512:hh * 512 + HROWS],
                                 wfct[qt][:],
                                 rhs4[qt].rearrange("p (h r) -> p h r",
                                                    h=2)[:, hh, :],
                                 start=(qt == 0), stop=(qt == 3))
        ob = lp.tile([EMB, ROWS], F32, name="ob")
        obv = ob[:].rearrange("p (h r) -> p h r", h=2)
        psv = (ps[:, 0:1024].rearrange("p (h x) -> p h x", h=2)
               [:, :, 0:HROWS])
        nc.scalar.activation(obv, psv, AF.Identity,
                             bias=bfc[:, 0:1], scale=1.0)
        nc.sync.dma_start(io["out_d"][:], ob[:])

    if "fdbg0" in io:
        nc.sync.dma_start(io["fdbg0"][:], F0[:])
        nc.sync.dma_start(io["fdbg1"][:], F1[:])
    ctx.close()


def _build_program():
    nc = bacc.Bacc("TRN2", target_bir_lowering=False, debug=False,
                   num_devices=NCORES)

    def din(name, shape, dt=F16):
        return nc.dram_tensor(name, shape, dt, kind="ExternalInput").ap()

    io = dict(
        x0=din("x0", [NCH, 128, GBLK * 8]),
        mixM=din("mixM", [128, 128]),
        w1=din("w1", [FIN, 64]), w2=din("w2", [64, 128]),
        w3=din("w3", [128, 256]), w4=din("w4", [256, 256]),
        b1=din("b1", [64, 1], F32), b2=din("b2", [128, 1], F32),
        b3=din("b3", [128, 2], F32), b4=din("b4", [128, 2], F32),
        lxf=din("lxf", [256, 1024]), lhf=din("lhf", [256, 1024]),
        lxb=din("lxb", [256, 1024]),
        bgf=din("bgf", [128, 8], F32), bgb=din("bgb", [128, 8], F32),
        wfc=din("wfc", [512, 128]), bfc=din("bfc", [128, 1], F32),
        ident=din("ident", [128, 128]),
        out_d=nc.dram_tensor("out", [EMB, ROWS], F32,
                             kind="ExternalOutput").ap(),
    )
    if os.environ.get("K_FDBG", "0") == "1":
        io["fdbg0"] = nc.dram_tensor("fdbg0", [128, FTOT], F16,
                                     kind="ExternalOutput").ap()
        io["fdbg1"] = nc.dram_tensor("fdbg1", [128, FTOT], F16,
                                     kind="ExternalOutput").ap()

    with tile.TileContext(nc) as tc:
        _kernel_body(tc, io)
    nc.compile()
    return nc


def _host_prep(inputs):
    f16 = np.float16
    data = np.asarray(inputs["data"], np.float32)
    ei = np.asarray(inputs["edge_index"]).astype(np.int64)

    src = np.concatenate([ei[0], np.arange(N)])
    dst = np.concatenate([ei[1], np.arange(N)])
    deg = np.zeros(N, np.float32)
    np.add.at(deg, dst, 1.0)
    dinv = np.where(deg > 0, deg ** -0.5, 0.0).astype(np.float32)
    Ahat = np.zeros((N, N), np.float32)
    np.add.at(Ahat, (dst, src), dinv[src] * dinv[dst])
    mixM = np.zeros((128, 128), np.float32)
    mixM[0:N * G5, 0:N * G5] = np.kron(Ahat.T, np.eye(G5, dtype=np.float32))
    mixM = mixM.astype(f16)

    # x0: [core][chunk b][blk = n*5+g5 (120:128 zero)][gb*8 + c],
    # t = 5*gb+g5, channels 6:8 zero
    d = data.reshape(NCORES, BL, T, N, FIN)
    x0 = np.zeros((NCORES, BL, 128, GBLK, 8), np.float32)
    dpad = np.zeros((NCORES, BL, TP, N, FIN), np.float32)
    dpad[:, :, :T] = d
    dv = dpad.reshape(NCORES, BL, GBLK, G5, N, FIN)
    # [core, b, n, g5, gb, c]
    dv = dv.transpose(0, 1, 4, 3, 2, 5).reshape(NCORES, BL, N * G5, GBLK, FIN)
    x0[:, :, 0:N * G5, :, 0:FIN] = dv
    x0 = np.ascontiguousarray(
        x0.reshape(NCORES, BL, 128, GBLK * 8)).astype(f16)

    perm = np.concatenate([np.arange(0, H), np.arange(H, 2 * H),
                           np.arange(3 * H, 4 * H), np.arange(2 * H, 3 * H)])

    def prep_dir(wih, whh, bih, bhh):
        wihp = np.asarray(wih, np.float32)[perm] / N
        whhp = np.asarray(whh, np.float32)[perm]
        bg = (np.asarray(bih, np.float32) + np.asarray(bhh, np.float32))[perm]
        return (np.ascontiguousarray(wihp.T).astype(f16),
                np.ascontiguousarray(whhp.T).astype(f16),
                np.ascontiguousarray(bg.reshape(8, 128).T).astype(np.float32))

    lxf, lhf, bgf = prep_dir(inputs["lstm_Wih_f"], inputs["lstm_Whh_f"],
                             inputs["lstm_bih_f"], inputs["lstm_bhh_f"])
    lxb, _lhb, bgb = prep_dir(inputs["lstm_Wih_b"], inputs["lstm_Whh_b"],
                              inputs["lstm_bih_b"], inputs["lstm_bhh_b"])

    com = {
        "mixM": mixM,
        "w1": np.asarray(inputs["W1"], np.float32).astype(f16),
        "w2": np.asarray(inputs["W2"], np.float32).astype(f16),
        "w3": np.asarray(inputs["W3"], np.float32).astype(f16),
        "w4": np.asarray(inputs["W4"], np.float32).astype(f16),
        "b1": np.asarray(inputs["b1"], np.float32).reshape(64, 1),
        "b2": np.asarray(inputs["b2"], np.float32).reshape(128, 1),
        "b3": np.ascontiguousarray(
            np.asarray(inputs["b3"], np.float32).reshape(2, 128).T),
        "b4": np.ascontiguousarray(
            np.asarray(inputs["b4"], np.float32).reshape(2, 128).T),
        "lxf": lxf, "lhf": lhf, "lxb": lxb, "bgf": bgf, "bgb": bgb,
        "wfc": np.asarray(inputs["Wfc"], np.float32).astype(f16),
        "bfc": np.asarray(inputs["bfc"], np.float32).reshape(128, 1),
        "ident": np.eye(128, dtype=f16),
    }
    return [dict(com, x0=x0[c]) for c in range(NCORES)]


TRACE = False          # set by test harness to capture an NTFF profile


def kernel(**inputs) -> np.ndarray:
    if "nc" not in _CACHE:
        _CACHE["nc"] = _build_program()
    nc = _CACHE["nc"]
    in_maps = _host_prep(inputs)
    res = bass_utils.run_bass_kernel_spmd(nc, in_maps,
                                          core_ids=list(range(NCORES)),
                                          trace=TRACE)
    _CACHE["last_res"] = res
    outs = []
    for c in range(NCORES):
        o = res.results[c]["out"]                       # [128, 904]
        outs.append(o.reshape(EMB, BL, NW).transpose(1, 2, 0))
    return np.concatenate(outs, 0).astype(np.float32)   # [64, 113, 128]


if __name__ == "__main__":
    import reference
    ins = {k: np.asarray(v) for k, v in reference.setup_inputs().items()}
    out = kernel(**ins)
    print("kernel out", out.shape, out.dtype, float(np.abs(out).max()))


# revision 23
# speedup vs baseline: 1.6740x; 1.1292x over previous
"""Trainium2 Bass kernel for DeepConvGraphEncoderDownstream.

Model (per reference):
  4-layer GCN (shared dense 24x24 graph operator) applied per (batch, timestep)
  frame -> node-mean -> per sliding window (W=32, stride 2, 113 windows):
  BiLSTM(H=256) -> concat(h_fwd[-1], h_bwd[0]) @ Wfc + bfc.

Key algebraic restructurings:
  * gcn_norm folded into one dense Ahat[24,24] on host.
  * GCN runs ONCE over all 256 timesteps (the reference recomputes it ~14x
    across overlapping windows).
  * backward LSTM: only hb[:, 0] is used => exactly ONE step, no recurrence.
  * forward LSTM: all 113 windows batched into one 904-row recurrence per
    core; input transforms U precomputed from node-mean features.

Sharding: data-parallel over batch, 8 batches/core on 8 cores; output
slices are independent (no collectives).

v2 performance structure:
  * GCN processed in 16 half-chunk units (28/24 gb groups) for cross-unit
    pipelining; pools sized for ~1 unit of overlap.
  * DMA transposes round-robin over the two HWDGE rings (sync 5/6,
    scalar 1/6); x0 loads on gpsimd SWDGE.
  * PSUM->SBUF casts and bias+ReLU rotate between vector and scalar.
  * x0 packed at c=8 (6 real channels) => mix L1 is 224 cols, not 3584;
    L1->L2 transpose narrow (64 channels).
  * node-sum via single strided tensor_reduce per (kt, unit).
  * LSTM: no identity matmuls -- U+bias pre-written into PSUM by vector,
    Whh matmuls accumulate on top (start=False); activations batched per
    gate pair; elementwise state update on gpsimd.

Layouts (per core, per chunk = one local batch = 256 timesteps padded to
260 = 52 blocks * 5):
  A-layout [c_part, free=(gb, blk:128)], blk = n*5+g5 (120:128 pad),
           timestep t = 5*gb + g5.
  B-layout [blk partitions = 128, free=(gb, c)]
  A->B / B->A are single XBAR DMA-transpose instructions per c-block:
  HW semantics out[p, b, c] = in[c, b*128 + p].
  Node mixing = matmul with zero-padded stationary kron(Ahat^T, I5) [128,128].
"""

import itertools
import os
import sys
import numpy as np

try:
    import concourse.bass as bass
except ImportError:
    sys.path.insert(0, "/opt/trn_rl_repo")
    import concourse.bass as bass

import concourse.bacc as bacc
import concourse.tile as tile
from concourse import mybir
from concourse import bass_utils

F16 = mybir.dt.float16
F32 = mybir.dt.float32
AF = mybir.ActivationFunctionType
ALU = mybir.AluOpType

B, T, N, FIN = 64, 256, 24, 6
H, EMB = 256, 128
WIN = 32
NW = (T - WIN) // 2 + 1               # 113
NCORES = 8
BL = B // NCORES                      # 8
G5 = 5
GBLK = 52                             # ceil(260/5): 52*5 = 260 t-slots
TP = GBLK * G5                        # 260 padded timesteps
NCH = BL
ROWS = BL * NW                        # 904
HROWS = ROWS // 2                     # 452
FTOT = BL * TP                        # 2080 F columns
HALVES = [(0, 28), (28, 24)]          # (gb offset, gb count) per unit

PREWRITE = os.environ.get("K_PREWRITE", "1") == "1"

_CACHE = {}


def _chunks(nf, step):
    return [(i, min(step, nf - i)) for i in range(0, nf, step)]


def _kernel_body(tc, io):
    nc = tc.nc
    from contextlib import ExitStack
    ctx = ExitStack()

    cons = ctx.enter_context(tc.tile_pool(name="cons", bufs=1))
    fpool = ctx.enter_context(tc.tile_pool(name="fpool", bufs=1))

    def load_const(name, shape, dt=F16):
        t = cons.tile(shape, dt, name=name)
        nc.sync.dma_start(t[:], io[name][:])
        return t

    mixM = load_const("mixM", [128, 128])
    w1 = load_const("w1", [FIN, 64])
    w2 = load_const("w2", [64, 128])
    w3 = load_const("w3", [128, 256])
    b1 = load_const("b1", [64, 1], F32)
    b2 = load_const("b2", [128, 1], F32)
    b3 = load_const("b3", [128, 2], F32)
    b4 = load_const("b4", [128, 2], F32)
    w4k = []
    for kt in range(2):
        t = cons.tile([128, 256], F16, name=f"w4k{kt}")
        nc.sync.dma_start(t[:], io["w4"][kt * 128:(kt + 1) * 128, :])
        w4k.append(t)

    def load_ktiles(name):
        ts = []
        for kt in range(2):
            t = cons.tile([128, 1024], F16, name=f"{name}{kt}")
            nc.sync.dma_start(t[:], io[name][kt * 128:(kt + 1) * 128, :])
            ts.append(t)
        return ts

    lxf = load_ktiles("lxf")
    lhf = load_ktiles("lhf")
    lxb = load_ktiles("lxb")
    bgf = load_const("bgf", [128, 8], F32)
    bgb = load_const("bgb", [128, 8], F32)
    wfct = []
    for qt in range(4):
        t = cons.tile([128, 128], F16, name=f"wfct{qt}")
        nc.sync.dma_start(t[:], io["wfc"][qt * 128:(qt + 1) * 128, :])
        wfct.append(t)
    bfc = load_const("bfc", [128, 1], F32)
    ident = load_const("ident", [128, 128])

    F0 = fpool.tile([128, FTOT], F16, name="F0")
    F1 = fpool.tile([128, FTOT], F16, name="F1")
    Fts = [F0, F1]

    # persistent zero-padded B-layout staging tiles (pad cols stay 0)
    y1pads = [cons.tile([128, 28 * 128], F16, name=f"y1p{i}") for i in range(2)]
    y2pads = [cons.tile([128, 28 * 128], F16, name=f"y2p{i}") for i in range(2)]
    for i in range(2):
        nc.vector.memset(y1pads[i][:], 0.0)
        nc.gpsimd.memset(y2pads[i][:], 0.0)

    # engine rotations
    cast_cycle = itertools.cycle([nc.vector, nc.scalar])
    relu_cycle = itertools.cycle([nc.scalar, nc.vector])
    # concurrent XBAR transposes on the two HWDGE rings corrupt each other
    # (shared xbar S2M state) -- keep ALL transposes on the sync ring.
    tp_cycle = itertools.cycle([nc.sync])

    def cast_rot(dst, src):
        e = next(cast_cycle)
        if e is nc.scalar:
            nc.scalar.copy(dst, src)
        else:
            e.tensor_copy(dst, src)

    def relu_rot(dst, src, bias_ap):
        e = next(relu_cycle)
        if e is nc.scalar:
            nc.scalar.activation(dst, src, AF.Relu, bias=bias_ap, scale=1.0)
        else:
            e.tensor_scalar(dst, src, bias_ap, 0.0, ALU.add, ALU.max)

    def tpose(out_ap, in_ap):
        next(tp_cycle).dma_start(out_ap, in_ap, transpose=True)

    # ================= Phase 1: GCN =================
    # Units are emitted stage-interleaved (software pipeline) so pool-buffer
    # rotation reuse targets recently-freed buffers instead of coupling each
    # unit's first stage to the previous unit's last.
    with tc.tile_pool(name="gcnS", bufs=4) as gpS, \
         tc.tile_pool(name="gcnA", bufs=9) as gpA, \
         tc.tile_pool(name="gcnB", bufs=5) as gpB, \
         tc.tile_pool(name="gcnBn", bufs=3) as gpBn, \
         tc.tile_pool(name="gcnBig", bufs=2) as gpBig, \
         tc.tile_pool(name="gps", bufs=4, space="PSUM") as ps_g:

        def mix(src_ap, n_free, cast_fn):
            """node-mix src_ap [128, n_free] via mixM; cast_fn(c0, cw, ps_ap)
            moves each fp32 psum chunk into its fp16 destination."""
            for c0, cw in _chunks(n_free, 1024):
                ps = ps_g.tile([128, 1024], F32, tag="ps", name="mps")
                for s0, sw in _chunks(cw, 512):
                    nc.tensor.matmul(ps[:, s0:s0 + sw], mixM[:],
                                     src_ap[:, c0 + s0:c0 + s0 + sw],
                                     start=True, stop=True)
                cast_fn(c0, cw, ps[:, 0:cw])

        def transform(rhs_list, wslices, bias, cout, out_tiles, FH, mtr):
            """dense channel transform for output c-tile mtr; bias+ReLU fused
            in the PSUM->SBUF move."""
            nkt = len(rhs_list)
            mp = min(cout, 128)
            for f0, fw in _chunks(FH, 1024):
                ps = ps_g.tile([128, 1024], F32, tag="ps", name="tps")
                for s0, sw in _chunks(fw, 512):
                    g0 = f0 + s0
                    for kt in range(nkt):
                        w = wslices[kt]
                        wap = w[:, mtr * 128:(mtr + 1) * 128] if cout > 128 \
                            else w
                        nc.tensor.matmul(ps[0:mp, s0:s0 + sw], wap,
                                         rhs_list[kt][:, g0:g0 + sw],
                                         start=(kt == 0),
                                         stop=(kt == nkt - 1))
                relu_rot(out_tiles[mtr][0:mp, f0:f0 + fw],
                         ps[0:mp, 0:fw], bias[0:mp, mtr:mtr + 1])

        def unit_stages(uidx):
            k, hf_i = uidx // 2, uidx % 2
            g0gb, gh = HALVES[hf_i]
            FH = gh * 128

            # --- L1: packed-c8 input, mix at c=8, narrow transform 6->64
            x0t = gpS.tile([128, 28 * 8], F16, tag="x0", name="x0t")
            nc.gpsimd.dma_start(x0t[:, 0:gh * 8],
                                io["x0"][k][:, g0gb * 8:(g0gb + gh) * 8])
            y1p = y1pads[uidx % 2]

            def cast_y1(c0, cw, ps_ap):
                cast_rot(y1p[:, 0:gh * 128]
                         .rearrange("p (gb c) -> p gb c", c=128)[:, :, 0:8],
                         ps_ap.rearrange("p (gb c) -> p gb c", c=8))

            mix(x0t[:, 0:gh * 8], gh * 8, cast_y1)
            yield
            y1a = gpA.tile([128, 28 * 128], F16, tag="cA", name="y1a")
            tpose(y1a[:, 0:FH].rearrange("c (gb p) -> c gb p", p=128),
                  y1p[:, 0:FH])
            yield
            x1a = gpA.tile([128, 28 * 128], F16, tag="cA", name="x1a")
            transform([y1a[0:FIN, 0:FH]], [w1[:]], b1, 64, [x1a], FH, 0)
            yield

            # --- L2: narrow A->B (64 c), mix at 64, transform 64->128
            x2b = gpBn.tile([128, 28 * 64], F16, tag="cBn", name="x2b")
            tpose(x2b[:, 0:gh * 64].rearrange("p (gb c) -> p gb c", c=64),
                  x1a[0:64, 0:FH])
            yield
            y2p = y2pads[uidx % 2]

            def cast_y2(c0, cw, ps_ap):
                cast_rot(y2p[:, 0:gh * 128]
                         .rearrange("p (gb c) -> p gb c", c=128)
                         [:, c0 // 64:(c0 + cw) // 64, 0:64],
                         ps_ap.rearrange("p (gb c) -> p gb c", c=64))

            mix(x2b[:, 0:gh * 64], gh * 64, cast_y2)
            yield
            y2a = gpA.tile([128, 28 * 128], F16, tag="cA", name="y2a")
            tpose(y2a[:, 0:FH].rearrange("c (gb p) -> c gb p", p=128),
                  y2p[:, 0:FH])
            yield
            x2a = gpA.tile([128, 28 * 128], F16, tag="cA", name="x2a")
            transform([y2a[0:64, 0:FH]], [w2[:]], b2, 128, [x2a], FH, 0)
            yield

            # --- L3: mix@128 -> transform 128->256
            x3b = gpB.tile([128, 28 * 128], F16, tag="cB", name="x3b")
            tpose(x3b[:, 0:FH].rearrange("p (gb c) -> p gb c", c=128),
                  x2a[:, 0:FH])
            yield
            y3b = gpB.tile([128, 28 * 128], F16, tag="cB", name="y3b")

            def cast_y3(c0, cw, ps_ap):
                cast_rot(y3b[:, c0:c0 + cw], ps_ap)

            mix(x3b[:, 0:FH], FH, cast_y3)
            yield
            y3a = gpA.tile([128, 28 * 128], F16, tag="cA", name="y3a")
            tpose(y3a[:, 0:FH].rearrange("c (gb p) -> c gb p", p=128),
                  y3b[:, 0:FH])
            yield
            x3a0 = gpA.tile([128, 28 * 128], F16, tag="cA", name="x3a0")
            x3a1 = gpA.tile([128, 28 * 128], F16, tag="cA", name="x3a1")
            transform([y3a[:, 0:FH]], [w3[:]], b3, 256, [x3a0, x3a1], FH, 0)
            yield
            transform([y3a[:, 0:FH]], [w3[:]], b3, 256, [x3a0, x3a1], FH, 1)
            yield

            # --- L4: mix@256 -> transform 256->256
            x4b = gpBig.tile([128, 28 * 256], F16, tag="big", name="x4b")
            x4bv = x4b[:, 0:gh * 256].rearrange("p (gb c) -> p gb c", c=256)
            tpose(x4bv[:, :, 0:128], x3a0[:, 0:FH])
            yield
            tpose(x4bv[:, :, 128:256], x3a1[:, 0:FH])
            yield
            ylo = gpB.tile([128, 28 * 128], F16, tag="cB", name="ylo")
            yhi = gpB.tile([128, 28 * 128], F16, tag="cB", name="yhi")

            def cast_y4(c0, cw, ps_ap):
                psv = ps_ap.rearrange("p (g c) -> p g c", c=256)
                g = cw // 256
                for hf in range(2):
                    dv = (ylo, yhi)[hf][:].rearrange(
                        "p (gb c) -> p gb c", c=128)
                    cast_rot(dv[:, c0 // 256:c0 // 256 + g, :],
                             psv[:, :, hf * 128:(hf + 1) * 128])

            mix(x4b[:, 0:gh * 256], gh * 256, cast_y4)
            yield
            y4a0 = gpA.tile([128, 28 * 128], F16, tag="cA", name="y4a0")
            y4a1 = gpA.tile([128, 28 * 128], F16, tag="cA", name="y4a1")
            tpose(y4a0[:, 0:FH].rearrange("c (gb p) -> c gb p", p=128),
                  ylo[:, 0:FH])
            yield
            tpose(y4a1[:, 0:FH].rearrange("c (gb p) -> c gb p", p=128),
                  yhi[:, 0:FH])
            yield
            x4a0 = gpA.tile([128, 28 * 128], F16, tag="cA", name="x4a0")
            x4a1 = gpA.tile([128, 28 * 128], F16, tag="cA", name="x4a1")
            transform([y4a0[:, 0:FH], y4a1[:, 0:FH]],
                      [w4k[0][:], w4k[1][:]], b4, 256, [x4a0, x4a1], FH, 0)
            yield
            transform([y4a0[:, 0:FH], y4a1[:, 0:FH]],
                      [w4k[0][:], w4k[1][:]], b4, 256, [x4a0, x4a1], FH, 1)
            yield

            # node-sum into F: F[:, k*TP + t], t = 5*(g0gb+gb) + g5
            for ct, xt in enumerate((x4a0, x4a1)):
                xv = (xt[:, 0:FH]
                      .rearrange("p (gb blk) -> p gb blk", blk=128)
                      [:, :, 0:120]
                      .rearrange("p gb (n g5) -> p gb g5 n", g5=G5))
                dstv = (Fts[ct][:, k * TP + g0gb * G5:
                                k * TP + (g0gb + gh) * G5]
                        .rearrange("p (gb g5) -> p gb g5", g5=G5))
                with nc.allow_low_precision("node-sum in fp16, as baseline"):
                    nc.vector.tensor_reduce(dstv, xv, mybir.AxisListType.X,
                                            ALU.add)

        # skewed round-robin driver: admit the next unit once the newest
        # active one is SKEW stages in; emit one stage per active unit.
        SKEW = int(os.environ.get("K_SKEW", "6"))
        gens = [unit_stages(u) for u in range(NCH * 2)]
        active, nxt, prog = [], 0, {}
        while active or nxt < len(gens):
            if nxt < len(gens) and (not active or prog[active[-1]] >= SKEW):
                active.append(nxt)
                prog[nxt] = 0
                nxt += 1
            for u in list(active):
                try:
                    next(gens[u])
                    prog[u] += 1
                except StopIteration:
                    active.remove(u)

    # ================= Phase 2: U = F @ (Wih_f/24)^T =================
    # U stored par-major: col = par*(BL*130) + b*130 + kk, where the source
    # F column is b*260 + 2*kk + par.  This gives the LSTM pre-writes a
    # unit-stride inner dim.
    upool = ctx.enter_context(tc.tile_pool(name="upool", bufs=1))
    KK = TP // 2                                  # 130
    Umt = []
    with tc.tile_pool(name="ups", bufs=4, space="PSUM") as ps_u:
        for mt in range(8):
            u = upool.tile([128, FTOT], F16, name=f"U{mt}")
            uview = u[:].rearrange("p (par b kk) -> p b kk par",
                                   par=2, kk=KK)
            for b in range(BL):
                ps = ps_u.tile([128, 512], F32, tag="ups", name="ups")
                for kt in range(2):
                    nc.tensor.matmul(ps[:, 0:TP],
                                     lxf[kt][:, mt * 128:(mt + 1) * 128],
                                     Fts[kt][:, b * TP:(b + 1) * TP],
                                     start=(kt == 0), stop=(kt == 1))
                cast_rot(uview[:, b],
                         ps[:, 0:TP].rearrange("p (kk par) -> p kk par",
                                               par=2))
            Umt.append(u)

    # ================= Phase 3: forward LSTM =================
    lp = ctx.enter_context(tc.tile_pool(name="lstm", bufs=1))
    Hf = lp.tile([128, 2 * ROWS], F16, name="Hf")
    Cf = lp.tile([128, 2 * ROWS], F16, name="Cf")
    nc.vector.memset(Hf[:], 0.0)
    nc.gpsimd.memset(Cf[:], 0.0)
    gi = lp.tile([128, 2 * ROWS], F16, name="gi")
    gf = lp.tile([128, 2 * ROWS], F16, name="gf")
    go = lp.tile([128, 2 * ROWS], F16, name="go")
    tg = lp.tile([128, 2 * ROWS], F16, name="tg")
    tcl = lp.tile([128, 2 * ROWS], F16, name="tcl")
    tmp = lp.tile([128, 2 * ROWS], F16, name="tmp")
    # pair order (g, i, f, o): the c/h elementwise update interleaves with
    # the later pairs instead of forming a serial tail after all four.
    PAIRS = [(6, 7, tg, AF.Tanh), (0, 1, gi, AF.Sigmoid),
             (2, 3, gf, AF.Sigmoid), (4, 5, go, AF.Sigmoid)]

    with tc.tile_pool(name="lps", bufs=2, space="PSUM") as ps_l:
        # software pipeline over the global pair sequence: each pair's PSUM
        # pre-write is emitted right after the act that frees its buffer
        # (2 pairs earlier), so vector/scalar flow without step-tail stalls.
        PAIR_SEQ = [(s, pi) for s in range(WIN) for pi in range(4)]
        ptile = {}

        def emit_pre(P):
            s, pi = PAIR_SEQ[P]
            k0, par = s // 2, s % 2
            ma, mb, _, _ = PAIRS[pi]
            ps = ps_l.tile([128, 2048], F32, tag="lp", name="lp")
            for j, mt in ((0, ma), (1, mb)):
                uv = Umt[mt][:].rearrange("p (par b kk) -> p par b kk",
                                          par=2, kk=KK)
                if PREWRITE:
                    src = (uv[:, par, :, k0:k0 + NW]
                           .rearrange("p (hh b) k -> p hh b k", hh=2))
                    dst = (ps[:, j * 1024:(j + 1) * 1024]
                           .rearrange("p (hh x) -> p hh x", hh=2)
                           [:, :, 0:HROWS]
                           .rearrange("p hh (b k) -> p hh b k", k=NW))
                    if j == 1 and pi % 2 == 1:
                        nc.scalar.activation(dst, src, AF.Identity,
                                             bias=bgf[:, mt:mt + 1],
                                             scale=1.0)
                    else:
                        nc.vector.tensor_scalar(dst, src, bgf[:, mt:mt + 1],
                                                None, ALU.add)
            ptile[P] = ps

        emit_pre(0)
        emit_pre(1)
        for P, (s, pi) in enumerate(PAIR_SEQ):
            k0, par = s // 2, s % 2
            ma, mb, gdst, fn = PAIRS[pi]
            ps = ptile.pop(P)
            for j, mt in ((0, ma), (1, mb)):
                for hh in range(2):
                    pslice = ps[:, j * 1024 + hh * 512:
                                j * 1024 + hh * 512 + HROWS]
                    b0 = hh * (BL // 2)
                    if not PREWRITE:
                        uv = Umt[mt][:].rearrange(
                            "p (par b kk) -> p par b kk", par=2, kk=KK)
                        nc.tensor.matmul(
                            pslice, ident[:],
                            uv[:, par, b0:b0 + BL // 2, k0:k0 + NW],
                            start=True, stop=False)
                    for kt in range(2):
                        nc.tensor.matmul(
                            pslice, lhf[kt][:, mt * 128:(mt + 1) * 128],
                            Hf[:, kt * ROWS + hh * HROWS:
                               kt * ROWS + (hh + 1) * HROWS],
                            start=False, stop=(kt == 1),
                            skip_group_check=True)
            psq = ps[:].rearrange("p (q x) -> p q x", q=4)[:, :, 0:HROWS]
            gv = gdst[:].rearrange("p (q r) -> p q r", q=4)
            if PREWRITE:
                nc.scalar.activation(gv, psq, fn, scale=1.0)
            else:
                gva = gdst[:].rearrange("p (m x) -> p m x", m=2)
                psa = ps[:].rearrange("p (m x) -> p m x", m=2)
                for j, mt in ((0, ma), (1, mb)):
                    nc.scalar.activation(
                        gva[:, j, :].rearrange("p (hh r) -> p hh r", hh=2),
                        psa[:, j, :].rearrange("p (hh x) -> p hh x",
                                               hh=2)[:, :, 0:HROWS],
                        fn, bias=bgf[:, mt:mt + 1], scale=1.0)
            if P + 2 < len(PAIR_SEQ):
                emit_pre(P + 2)
            if pi == 1:
                nc.gpsimd.tensor_tensor(tmp[:], gi[:], tg[:], ALU.mult)
            elif pi == 2:
                nc.vector.tensor_tensor(Cf[:], gf[:], Cf[:], ALU.mult)
                nc.vector.tensor_tensor(Cf[:], Cf[:], tmp[:], ALU.add)
                nc.scalar.activation(tcl[:], Cf[:], AF.Tanh)
            elif pi == 3:
                nc.vector.tensor_tensor(Hf[:], go[:], tcl[:], ALU.mult)

        # ===== Phase 4: backward LSTM single step (only hb[:,0] used) =====
        Hb = lp.tile([128, 2 * ROWS], F16, name="Hb")
        kb = (WIN - 2) // 2
        BPAIRS = [(0, 1, gi, AF.Sigmoid), (4, 5, go, AF.Sigmoid),
                  (6, 7, tg, AF.Tanh)]
        for ma, mb, gdst, fn in BPAIRS:
            ps = ps_l.tile([128, 2048], F32, tag="lp", name="lpb")
            for j, mt in ((0, ma), (1, mb)):
                for hh in range(2):
                    pslice = ps[:, j * 1024 + hh * 512:
                                j * 1024 + hh * 512 + HROWS]
                    b0 = hh * (BL // 2)
                    for kt in range(2):
                        fv = Fts[kt][:].rearrange("p (b k two) -> p b k two",
                                                  b=BL, two=2)
                        nc.tensor.matmul(
                            pslice, lxb[kt][:, mt * 128:(mt + 1) * 128],
                            fv[:, b0:b0 + BL // 2, kb:kb + NW, 1],
                            start=(kt == 0), stop=(kt == 1))
                psj = (ps[:, j * 1024:(j + 1) * 1024]
                       .rearrange("p (hh x) -> p hh x", hh=2)[:, :, 0:HROWS])
                gvj = (gdst[:, j * ROWS:(j + 1) * ROWS]
                       .rearrange("p (hh r) -> p hh r", hh=2))
                nc.scalar.activation(gvj, psj, fn,
                                     bias=bgb[:, mt:mt + 1], scale=1.0)
        nc.gpsimd.tensor_tensor(tmp[:], gi[:], tg[:], ALU.mult)
        nc.scalar.activation(tcl[:], tmp[:], AF.Tanh)
        nc.gpsimd.tensor_tensor(Hb[:], go[:], tcl[:], ALU.mult)

        # ===== Phase 5: FC head =====
        ps = ps_l.tile([128, 2048], F32, tag="lp", name="lpf")
        rhs4 = [Hf[:, 0:ROWS], Hf[:, ROWS:2 * ROWS],
                Hb[:, 0:ROWS], Hb[:, ROWS:2 * ROWS]]
        for hh in range(2):
            for qt in range(4):
                nc.tensor.matmul(ps[:, hh * 512:hh * 512 + HROWS],
                                 wfct[qt][:],
                                 rhs4[qt].rearrange("p (h r) -> p h r",
                                                    h=2)[:, hh, :],
                                 start=(qt == 0), stop=(qt == 3))
        ob = lp.tile([EMB, ROWS], F32, name="ob")
        obv = ob[:].rearrange("p (h r) -> p h r", h=2)
        psv = (ps[:, 0:1024].rearrange("p (h x) -> p h x", h=2)
               [:, :, 0:HROWS])
        nc.scalar.activation(obv, psv, AF.Identity,
                             bias=bfc[:, 0:1], scale=1.0)
        nc.sync.dma_start(io["out_d"][:], ob[:])

    if "fdbg0" in io:
        nc.sync.dma_start(io["fdbg0"][:], F0[:])
        nc.sync.dma_start(io["fdbg1"][:], F1[:])
    ctx.close()


def _build_program():
    nc = bacc.Bacc("TRN2", target_bir_lowering=False, debug=False,
                   num_devices=NCORES)

    def din(name, shape, dt=F16):
        return nc.dram_tensor(name, shape, dt, kind="ExternalInput").ap()

    io = dict(
        x0=din("x0", [NCH, 128, GBLK * 8]),
        mixM=din("mixM", [128, 128]),
        w1=din("w1", [FIN, 64]), w2=din("w2", [64, 128]),
        w3=din("w3", [128, 256]), w4=din("w4", [256, 256]),
        b1=din("b1", [64, 1], F32), b2=din("b2", [128, 1], F32),
        b3=din("b3", [128, 2], F32), b4=din("b4", [128, 2], F32),
        lxf=din("lxf", [256, 1024]), lhf=din("lhf", [256, 1024]),
        lxb=din("lxb", [256, 1024]),
        bgf=din("bgf", [128, 8], F32), bgb=din("bgb", [128, 8], F32),
        wfc=din("wfc", [512, 128]), bfc=din("bfc", [128, 1], F32),
        ident=din("ident", [128, 128]),
        out_d=nc.dram_tensor("out", [EMB, ROWS], F32,
                             kind="ExternalOutput").ap(),
    )
    if os.environ.get("K_FDBG", "0") == "1":
        io["fdbg0"] = nc.dram_tensor("fdbg0", [128, FTOT], F16,
                                     kind="ExternalOutput").ap()
        io["fdbg1"] = nc.dram_tensor("fdbg1", [128, FTOT], F16,
                                     kind="ExternalOutput").ap()

    with tile.TileContext(nc) as tc:
        _kernel_body(tc, io)
    nc.compile()
    return nc


def _host_prep(inputs):
    f16 = np.float16
    data = np.asarray(inputs["data"], np.float32)
    ei = np.asarray(inputs["edge_index"]).astype(np.int64)

    src = np.concatenate([ei[0], np.arange(N)])
    dst = np.concatenate([ei[1], np.arange(N)])
    deg = np.zeros(N, np.float32)
    np.add.at(deg, dst, 1.0)
    dinv = np.where(deg > 0, deg ** -0.5, 0.0).astype(np.float32)
    Ahat = np.zeros((N, N), np.float32)
    np.add.at(Ahat, (dst, src), dinv[src] * dinv[dst])
    mixM = np.zeros((128, 128), np.float32)
    mixM[0:N * G5, 0:N * G5] = np.kron(Ahat.T, np.eye(G5, dtype=np.float32))
    mixM = mixM.astype(f16)

    # x0: [core][chunk b][blk = n*5+g5 (120:128 zero)][gb*8 + c],
    # t = 5*gb+g5, channels 6:8 zero
    d = data.reshape(NCORES, BL, T, N, FIN)
    x0 = np.zeros((NCORES, BL, 128, GBLK, 8), np.float32)
    dpad = np.zeros((NCORES, BL, TP, N, FIN), np.float32)
    dpad[:, :, :T] = d
    dv = dpad.reshape(NCORES, BL, GBLK, G5, N, FIN)
    # [core, b, n, g5, gb, c]
    dv = dv.transpose(0, 1, 4, 3, 2, 5).reshape(NCORES, BL, N * G5, GBLK, FIN)
    x0[:, :, 0:N * G5, :, 0:FIN] = dv
    x0 = np.ascontiguousarray(
        x0.reshape(NCORES, BL, 128, GBLK * 8)).astype(f16)

    perm = np.concatenate([np.arange(0, H), np.arange(H, 2 * H),
                           np.arange(3 * H, 4 * H), np.arange(2 * H, 3 * H)])

    def prep_dir(wih, whh, bih, bhh):
        wihp = np.asarray(wih, np.float32)[perm] / N
        whhp = np.asarray(whh, np.float32)[perm]
        bg = (np.asarray(bih, np.float32) + np.asarray(bhh, np.float32))[perm]
        return (np.ascontiguousarray(wihp.T).astype(f16),
                np.ascontiguousarray(whhp.T).astype(f16),
                np.ascontiguousarray(bg.reshape(8, 128).T).astype(np.float32))

    lxf, lhf, bgf = prep_dir(inputs["lstm_Wih_f"], inputs["lstm_Whh_f"],
                             inputs["lstm_bih_f"], inputs["lstm_bhh_f"])
    lxb, _lhb, bgb = prep_dir(inputs["lstm_Wih_b"], inputs["lstm_Whh_b"],
                              inputs["lstm_bih_b"], inputs["lstm_bhh_b"])

    com = {
        "mixM": mixM,
        "w1": np.asarray(inputs["W1"], np.float32).astype(f16),
        "w2": np.asarray(inputs["W2"], np.float32).astype(f16),
        "w3": np.asarray(inputs["W3"], np.float32).astype(f16),
        "w4": np.asarray(inputs["W4"], np.float32).astype(f16),
        "b1": np.asarray(inputs["b1"], np.float32).reshape(64, 1),
        "b2": np.asarray(inputs["b2"], np.float32).reshape(128, 1),
        "b3": np.ascontiguousarray(
            np.asarray(inputs["b3"], np.float32).reshape(2, 128).T),
        "b4": np.ascontiguousarray(
            np.asarray(inputs["b4"], np.float32).reshape(2, 128).T),
        "lxf": lxf, "lhf": lhf, "lxb": lxb, "bgf": bgf, "bgb": bgb,
        "wfc": np.asarray(inputs["Wfc"], np.float32).astype(f16),
        "bfc": np.asarray(inputs["bfc"], np.float32).reshape(128, 1),
        "ident": np.eye(128, dtype=f16),
    }
    return [dict(com, x0=x0[c]) for c in range(NCORES)]


TRACE = False          # set by test harness to capture an NTFF profile


def kernel(**inputs) -> np.ndarray:
    if "nc" not in _CACHE:
        _CACHE["nc"] = _build_program()
    nc = _CACHE["nc"]
    in_maps = _host_prep(inputs)
    res = bass_utils.run_bass_kernel_spmd(nc, in_maps,
                                          core_ids=list(range(NCORES)),
                                          trace=TRACE)
    _CACHE["last_res"] = res
    outs = []
    for c in range(NCORES):
        o = res.results[c]["out"]                       # [128, 904]
        outs.append(o.reshape(EMB, BL, NW).transpose(1, 2, 0))
    return np.concatenate(outs, 0).astype(np.float32)   # [64, 113, 128]


if __name__ == "__main__":
    import reference
    ins = {k: np.asarray(v) for k, v in reference.setup_inputs().items()}
    out = kernel(**ins)
    print("kernel out", out.shape, out.dtype, float(np.abs(out).max()))


# revision 25
# speedup vs baseline: 1.6799x; 1.0035x over previous
"""Trainium2 Bass kernel for DeepConvGraphEncoderDownstream.

Model (per reference):
  4-layer GCN (shared dense 24x24 graph operator) applied per (batch, timestep)
  frame -> node-mean -> per sliding window (W=32, stride 2, 113 windows):
  BiLSTM(H=256) -> concat(h_fwd[-1], h_bwd[0]) @ Wfc + bfc.

Key algebraic restructurings:
  * gcn_norm folded into one dense Ahat[24,24] on host.
  * GCN runs ONCE over all 256 timesteps (the reference recomputes it ~14x
    across overlapping windows).
  * backward LSTM: only hb[:, 0] is used => exactly ONE step, no recurrence.
  * forward LSTM: all 113 windows batched into one 904-row recurrence per
    core; input transforms U precomputed from node-mean features.

Sharding: data-parallel over batch, 8 batches/core on 8 cores; output
slices are independent (no collectives).

v2 performance structure:
  * GCN processed in 16 half-chunk units (28/24 gb groups) for cross-unit
    pipelining; pools sized for ~1 unit of overlap.
  * DMA transposes round-robin over the two HWDGE rings (sync 5/6,
    scalar 1/6); x0 loads on gpsimd SWDGE.
  * PSUM->SBUF casts and bias+ReLU rotate between vector and scalar.
  * x0 packed at c=8 (6 real channels) => mix L1 is 224 cols, not 3584;
    L1->L2 transpose narrow (64 channels).
  * node-sum via single strided tensor_reduce per (kt, unit).
  * LSTM: no identity matmuls -- U+bias pre-written into PSUM by vector,
    Whh matmuls accumulate on top (start=False); activations batched per
    gate pair; elementwise state update on gpsimd.

Layouts (per core, per chunk = one local batch = 256 timesteps padded to
260 = 52 blocks * 5):
  A-layout [c_part, free=(gb, blk:128)], blk = n*5+g5 (120:128 pad),
           timestep t = 5*gb + g5.
  B-layout [blk partitions = 128, free=(gb, c)]
  A->B / B->A are single XBAR DMA-transpose instructions per c-block:
  HW semantics out[p, b, c] = in[c, b*128 + p].
  Node mixing = matmul with zero-padded stationary kron(Ahat^T, I5) [128,128].
"""

import itertools
import os
import sys
import numpy as np

try:
    import concourse.bass as bass
except ImportError:
    sys.path.insert(0, "/opt/trn_rl_repo")
    import concourse.bass as bass

import concourse.bacc as bacc
import concourse.tile as tile
from concourse import mybir
from concourse import bass_utils

F16 = mybir.dt.float16
F32 = mybir.dt.float32
AF = mybir.ActivationFunctionType
ALU = mybir.AluOpType

B, T, N, FIN = 64, 256, 24, 6
H, EMB = 256, 128
WIN = 32
NW = (T - WIN) // 2 + 1               # 113
NCORES = 8
BL = B // NCORES                      # 8
G5 = 5
GBLK = 52                             # ceil(260/5): 52*5 = 260 t-slots
TP = GBLK * G5                        # 260 padded timesteps
NCH = BL
ROWS = BL * NW                        # 904
HROWS = ROWS // 2                     # 452
FTOT = BL * TP                        # 2080 F columns
HALVES = [(0, 28), (28, 24)]          # (gb offset, gb count) per unit

PREWRITE = os.environ.get("K_PREWRITE", "1") == "1"

_CACHE = {}


def _chunks(nf, step):
    return [(i, min(step, nf - i)) for i in range(0, nf, step)]


def _kernel_body(tc, io):
    nc = tc.nc
    from contextlib import ExitStack
    ctx = ExitStack()

    cons = ctx.enter_context(tc.tile_pool(name="cons", bufs=1))
    fpool = ctx.enter_context(tc.tile_pool(name="fpool", bufs=1))

    def load_const(name, shape, dt=F16):
        t = cons.tile(shape, dt, name=name)
        nc.sync.dma_start(t[:], io[name][:])
        return t

    mixM = load_const("mixM", [128, 128])
    w1 = load_const("w1", [FIN, 64])
    w2 = load_const("w2", [64, 128])
    w3 = load_const("w3", [128, 256])
    b1 = load_const("b1", [64, 1], F32)
    b2 = load_const("b2", [128, 1], F32)
    b3 = load_const("b3", [128, 2], F32)
    b4 = load_const("b4", [128, 2], F32)
    w4k = []
    for kt in range(2):
        t = cons.tile([128, 256], F16, name=f"w4k{kt}")
        nc.sync.dma_start(t[:], io["w4"][kt * 128:(kt + 1) * 128, :])
        w4k.append(t)

    def load_ktiles(name):
        ts = []
        for kt in range(2):
            t = cons.tile([128, 1024], F16, name=f"{name}{kt}")
            nc.sync.dma_start(t[:], io[name][kt * 128:(kt + 1) * 128, :])
            ts.append(t)
        return ts

    lxf = load_ktiles("lxf")
    lhf = load_ktiles("lhf")
    lxb = load_ktiles("lxb")
    bgf = load_const("bgf", [128, 8], F32)
    bgb = load_const("bgb", [128, 8], F32)
    wfct = []
    for qt in range(4):
        t = cons.tile([128, 128], F16, name=f"wfct{qt}")
        nc.sync.dma_start(t[:], io["wfc"][qt * 128:(qt + 1) * 128, :])
        wfct.append(t)
    bfc = load_const("bfc", [128, 1], F32)
    ident = load_const("ident", [128, 128])

    F0 = fpool.tile([128, FTOT], F16, name="F0")
    F1 = fpool.tile([128, FTOT], F16, name="F1")
    Fts = [F0, F1]

    # persistent zero-padded B-layout staging tiles (pad cols stay 0)
    y1pads = [cons.tile([128, 28 * 128], F16, name=f"y1p{i}") for i in range(2)]
    y2pads = [cons.tile([128, 28 * 128], F16, name=f"y2p{i}") for i in range(2)]
    for i in range(2):
        nc.vector.memset(y1pads[i][:], 0.0)
        nc.gpsimd.memset(y2pads[i][:], 0.0)

    # engine rotations
    cast_cycle = itertools.cycle([nc.vector, nc.scalar])
    relu_cycle = itertools.cycle([nc.scalar, nc.vector])
    # concurrent XBAR transposes on the two HWDGE rings corrupt each other
    # (shared xbar S2M state) -- keep ALL transposes on the sync ring.
    tp_cycle = itertools.cycle([nc.sync])

    def cast_rot(dst, src):
        e = next(cast_cycle)
        if e is nc.scalar:
            nc.scalar.copy(dst, src)
        else:
            e.tensor_copy(dst, src)

    def relu_rot(dst, src, bias_ap):
        e = next(relu_cycle)
        if e is nc.scalar:
            nc.scalar.activation(dst, src, AF.Relu, bias=bias_ap, scale=1.0)
        else:
            e.tensor_scalar(dst, src, bias_ap, 0.0, ALU.add, ALU.max)

    def tpose(out_ap, in_ap):
        next(tp_cycle).dma_start(out_ap, in_ap, transpose=True)

    # ================= Phase 1: GCN =================
    # Units are emitted stage-interleaved (software pipeline) so pool-buffer
    # rotation reuse targets recently-freed buffers instead of coupling each
    # unit's first stage to the previous unit's last.
    with tc.tile_pool(name="gcnS", bufs=4) as gpS, \
         tc.tile_pool(name="gcnA", bufs=9) as gpA, \
         tc.tile_pool(name="gcnB", bufs=5) as gpB, \
         tc.tile_pool(name="gcnBn", bufs=3) as gpBn, \
         tc.tile_pool(name="gcnBig", bufs=2) as gpBig, \
         tc.tile_pool(name="gps", bufs=4, space="PSUM") as ps_g:

        def mix(src_ap, n_free, cast_fn):
            """node-mix src_ap [128, n_free] via mixM; cast_fn(c0, cw, ps_ap)
            moves each fp32 psum chunk into its fp16 destination."""
            for c0, cw in _chunks(n_free, 1024):
                ps = ps_g.tile([128, 1024], F32, tag="ps", name="mps")
                for s0, sw in _chunks(cw, 512):
                    nc.tensor.matmul(ps[:, s0:s0 + sw], mixM[:],
                                     src_ap[:, c0 + s0:c0 + s0 + sw],
                                     start=True, stop=True)
                cast_fn(c0, cw, ps[:, 0:cw])

        def transform(rhs_list, wslices, bias, cout, out_tiles, FH, mtr):
            """dense channel transform for output c-tile mtr; bias+ReLU fused
            in the PSUM->SBUF move."""
            nkt = len(rhs_list)
            mp = min(cout, 128)
            for f0, fw in _chunks(FH, 1024):
                ps = ps_g.tile([128, 1024], F32, tag="ps", name="tps")
                for s0, sw in _chunks(fw, 512):
                    g0 = f0 + s0
                    for kt in range(nkt):
                        w = wslices[kt]
                        wap = w[:, mtr * 128:(mtr + 1) * 128] if cout > 128 \
                            else w
                        nc.tensor.matmul(ps[0:mp, s0:s0 + sw], wap,
                                         rhs_list[kt][:, g0:g0 + sw],
                                         start=(kt == 0),
                                         stop=(kt == nkt - 1))
                relu_rot(out_tiles[mtr][0:mp, f0:f0 + fw],
                         ps[0:mp, 0:fw], bias[0:mp, mtr:mtr + 1])

        def unit_stages(uidx):
            k, hf_i = uidx // 2, uidx % 2
            g0gb, gh = HALVES[hf_i]
            FH = gh * 128

            # --- L1: packed-c8 input, mix at c=8, narrow transform 6->64
            x0t = gpS.tile([128, 28 * 8], F16, tag="x0", name="x0t")
            nc.gpsimd.dma_start(x0t[:, 0:gh * 8],
                                io["x0"][k][:, g0gb * 8:(g0gb + gh) * 8])
            y1p = y1pads[uidx % 2]

            def cast_y1(c0, cw, ps_ap):
                cast_rot(y1p[:, 0:gh * 128]
                         .rearrange("p (gb c) -> p gb c", c=128)[:, :, 0:8],
                         ps_ap.rearrange("p (gb c) -> p gb c", c=8))

            mix(x0t[:, 0:gh * 8], gh * 8, cast_y1)
            yield
            y1a = gpA.tile([128, 28 * 128], F16, tag="cA", name="y1a")
            tpose(y1a[:, 0:FH].rearrange("c (gb p) -> c gb p", p=128),
                  y1p[:, 0:FH])
            yield
            x1a = gpA.tile([128, 28 * 128], F16, tag="cA", name="x1a")
            transform([y1a[0:FIN, 0:FH]], [w1[:]], b1, 64, [x1a], FH, 0)
            yield

            # --- L2: narrow A->B (64 c), mix at 64, transform 64->128
            x2b = gpBn.tile([128, 28 * 64], F16, tag="cBn", name="x2b")
            tpose(x2b[:, 0:gh * 64].rearrange("p (gb c) -> p gb c", c=64),
                  x1a[0:64, 0:FH])
            yield
            y2p = y2pads[uidx % 2]

            def cast_y2(c0, cw, ps_ap):
                cast_rot(y2p[:, 0:gh * 128]
                         .rearrange("p (gb c) -> p gb c", c=128)
                         [:, c0 // 64:(c0 + cw) // 64, 0:64],
                         ps_ap.rearrange("p (gb c) -> p gb c", c=64))

            mix(x2b[:, 0:gh * 64], gh * 64, cast_y2)
            yield
            y2a = gpA.tile([128, 28 * 128], F16, tag="cA", name="y2a")
            tpose(y2a[:, 0:FH].rearrange("c (gb p) -> c gb p", p=128),
                  y2p[:, 0:FH])
            yield
            x2a = gpA.tile([128, 28 * 128], F16, tag="cA", name="x2a")
            transform([y2a[0:64, 0:FH]], [w2[:]], b2, 128, [x2a], FH, 0)
            yield

            # --- L3: mix@128 -> transform 128->256
            x3b = gpB.tile([128, 28 * 128], F16, tag="cB", name="x3b")
            tpose(x3b[:, 0:FH].rearrange("p (gb c) -> p gb c", c=128),
                  x2a[:, 0:FH])
            yield
            y3b = gpB.tile([128, 28 * 128], F16, tag="cB", name="y3b")

            def cast_y3(c0, cw, ps_ap):
                cast_rot(y3b[:, c0:c0 + cw], ps_ap)

            mix(x3b[:, 0:FH], FH, cast_y3)
            yield
            y3a = gpA.tile([128, 28 * 128], F16, tag="cA", name="y3a")
            tpose(y3a[:, 0:FH].rearrange("c (gb p) -> c gb p", p=128),
                  y3b[:, 0:FH])
            yield
            x3a0 = gpA.tile([128, 28 * 128], F16, tag="cA", name="x3a0")
            x3a1 = gpA.tile([128, 28 * 128], F16, tag="cA", name="x3a1")
            transform([y3a[:, 0:FH]], [w3[:]], b3, 256, [x3a0, x3a1], FH, 0)
            yield
            transform([y3a[:, 0:FH]], [w3[:]], b3, 256, [x3a0, x3a1], FH, 1)
            yield

            # --- L4: mix@256 -> transform 256->256
            x4b = gpBig.tile([128, 28 * 256], F16, tag="big", name="x4b")
            x4bv = x4b[:, 0:gh * 256].rearrange("p (gb c) -> p gb c", c=256)
            tpose(x4bv[:, :, 0:128], x3a0[:, 0:FH])
            yield
            tpose(x4bv[:, :, 128:256], x3a1[:, 0:FH])
            yield
            ylo = gpB.tile([128, 28 * 128], F16, tag="cB", name="ylo")
            yhi = gpB.tile([128, 28 * 128], F16, tag="cB", name="yhi")

            def cast_y4(c0, cw, ps_ap):
                psv = ps_ap.rearrange("p (g c) -> p g c", c=256)
                g = cw // 256
                for hf in range(2):
                    dv = (ylo, yhi)[hf][:].rearrange(
                        "p (gb c) -> p gb c", c=128)
                    cast_rot(dv[:, c0 // 256:c0 // 256 + g, :],
                             psv[:, :, hf * 128:(hf + 1) * 128])

            mix(x4b[:, 0:gh * 256], gh * 256, cast_y4)
            yield
            y4a0 = gpA.tile([128, 28 * 128], F16, tag="cA", name="y4a0")
            y4a1 = gpA.tile([128, 28 * 128], F16, tag="cA", name="y4a1")
            tpose(y4a0[:, 0:FH].rearrange("c (gb p) -> c gb p", p=128),
                  ylo[:, 0:FH])
            yield
            tpose(y4a1[:, 0:FH].rearrange("c (gb p) -> c gb p", p=128),
                  yhi[:, 0:FH])
            yield
            x4a0 = gpA.tile([128, 28 * 128], F16, tag="cA", name="x4a0")
            x4a1 = gpA.tile([128, 28 * 128], F16, tag="cA", name="x4a1")
            transform([y4a0[:, 0:FH], y4a1[:, 0:FH]],
                      [w4k[0][:], w4k[1][:]], b4, 256, [x4a0, x4a1], FH, 0)
            yield
            transform([y4a0[:, 0:FH], y4a1[:, 0:FH]],
                      [w4k[0][:], w4k[1][:]], b4, 256, [x4a0, x4a1], FH, 1)
            yield

            # node-sum into F: F[:, k*TP + t], t = 5*(g0gb+gb) + g5
            for ct, xt in enumerate((x4a0, x4a1)):
                xv = (xt[:, 0:FH]
                      .rearrange("p (gb blk) -> p gb blk", blk=128)
                      [:, :, 0:120]
                      .rearrange("p gb (n g5) -> p gb g5 n", g5=G5))
                dstv = (Fts[ct][:, k * TP + g0gb * G5:
                                k * TP + (g0gb + gh) * G5]
                        .rearrange("p (gb g5) -> p gb g5", g5=G5))
                with nc.allow_low_precision("node-sum in fp16, as baseline"):
                    nc.vector.tensor_reduce(dstv, xv, mybir.AxisListType.X,
                                            ALU.add)

        # skewed round-robin driver: admit the next unit once the newest
        # active one is SKEW stages in; emit one stage per active unit.
        SKEW = int(os.environ.get("K_SKEW", "6"))
        gens = [unit_stages(u) for u in range(NCH * 2)]
        active, nxt, prog = [], 0, {}
        while active or nxt < len(gens):
            if nxt < len(gens) and (not active or prog[active[-1]] >= SKEW):
                active.append(nxt)
                prog[nxt] = 0
                nxt += 1
            for u in list(active):
                try:
                    next(gens[u])
                    prog[u] += 1
                except StopIteration:
                    active.remove(u)

    # ================= Phase 2: U = F @ (Wih_f/24)^T =================
    # U stored par-major: col = par*(BL*130) + b*130 + kk, where the source
    # F column is b*260 + 2*kk + par.  This gives the LSTM pre-writes a
    # unit-stride inner dim.
    upool = ctx.enter_context(tc.tile_pool(name="upool", bufs=1))
    KK = TP // 2                                  # 130
    Umt = []
    with tc.tile_pool(name="ups", bufs=4, space="PSUM") as ps_u:
        for mt in range(8):
            u = upool.tile([128, FTOT], F16, name=f"U{mt}")
            uview = u[:].rearrange("p (par b kk) -> p b kk par",
                                   par=2, kk=KK)
            for b in range(BL):
                ps = ps_u.tile([128, 512], F32, tag="ups", name="ups")
                for kt in range(2):
                    nc.tensor.matmul(ps[:, 0:TP],
                                     lxf[kt][:, mt * 128:(mt + 1) * 128],
                                     Fts[kt][:, b * TP:(b + 1) * TP],
                                     start=(kt == 0), stop=(kt == 1))
                cast_rot(uview[:, b],
                         ps[:, 0:TP].rearrange("p (kk par) -> p kk par",
                                               par=2))
            Umt.append(u)

    # ================= Phase 3: forward LSTM =================
    lp = ctx.enter_context(tc.tile_pool(name="lstm", bufs=1))
    Hf = lp.tile([128, 2 * ROWS], F16, name="Hf")
    Cf = lp.tile([128, 2 * ROWS], F16, name="Cf")
    nc.vector.memset(Hf[:], 0.0)
    nc.gpsimd.memset(Cf[:], 0.0)
    gi = lp.tile([128, 2 * ROWS], F16, name="gi")
    gf = lp.tile([128, 2 * ROWS], F16, name="gf")
    go = lp.tile([128, 2 * ROWS], F16, name="go")
    tg = lp.tile([128, 2 * ROWS], F16, name="tg")
    tcl = lp.tile([128, 2 * ROWS], F16, name="tcl")
    tmp = lp.tile([128, 2 * ROWS], F16, name="tmp")
    # pair order (g, i, f, o): the c/h elementwise update interleaves with
    # the later pairs instead of forming a serial tail after all four.
    PAIRS = [(6, 7, tg, AF.Tanh), (0, 1, gi, AF.Sigmoid),
             (2, 3, gf, AF.Sigmoid), (4, 5, go, AF.Sigmoid)]

    with tc.tile_pool(name="lps", bufs=2, space="PSUM") as ps_l:
        # software pipeline over the global pair sequence: each pair's PSUM
        # pre-write is emitted right after the act that frees its buffer
        # (2 pairs earlier), so vector/scalar flow without step-tail stalls.
        PAIR_SEQ = [(s, pi) for s in range(WIN) for pi in range(4)]
        ptile = {}

        def emit_pre(P):
            # only pair 0 (g) gets a vector pre-write: its buffer frees
            # mid-previous-step so the write costs no stall.  Later pairs
            # inject U via identity matmuls at mm time -- a pre-write there
            # would stall on the pair-buffer WAR (only 2 pair tiles fit in
            # PSUM) and gate the PE stream.
            s, pi = PAIR_SEQ[P]
            if not PREWRITE or pi >= 1:
                return
            k0, par = s // 2, s % 2
            ma, mb, _, _ = PAIRS[pi]
            ps = ps_l.tile([128, 2048], F32, tag="lp", name="lp")
            for j, mt in ((0, ma), (1, mb)):
                uv = Umt[mt][:].rearrange("p (par b kk) -> p par b kk",
                                          par=2, kk=KK)
                src = (uv[:, par, :, k0:k0 + NW]
                       .rearrange("p (hh b) k -> p hh b k", hh=2))
                dst = (ps[:, j * 1024:(j + 1) * 1024]
                       .rearrange("p (hh x) -> p hh x", hh=2)
                       [:, :, 0:HROWS]
                       .rearrange("p hh (b k) -> p hh b k", k=NW))
                nc.vector.tensor_scalar(dst, src, bgf[:, mt:mt + 1],
                                        None, ALU.add)
            ptile[P] = ps

        emit_pre(0)
        emit_pre(1)
        for P, (s, pi) in enumerate(PAIR_SEQ):
            k0, par = s // 2, s % 2
            ma, mb, gdst, fn = PAIRS[pi]
            prewritten = P in ptile
            ps = ptile.pop(P) if prewritten \
                else ps_l.tile([128, 2048], F32, tag="lp", name="lp")
            for j, mt in ((0, ma), (1, mb)):
                for hh in range(2):
                    pslice = ps[:, j * 1024 + hh * 512:
                                j * 1024 + hh * 512 + HROWS]
                    b0 = hh * (BL // 2)
                    if not prewritten:
                        uv = Umt[mt][:].rearrange(
                            "p (par b kk) -> p par b kk", par=2, kk=KK)
                        nc.tensor.matmul(
                            pslice, ident[:],
                            uv[:, par, b0:b0 + BL // 2, k0:k0 + NW],
                            start=True, stop=False)
                    for kt in range(2):
                        nc.tensor.matmul(
                            pslice, lhf[kt][:, mt * 128:(mt + 1) * 128],
                            Hf[:, kt * ROWS + hh * HROWS:
                               kt * ROWS + (hh + 1) * HROWS],
                            start=False, stop=(kt == 1),
                            skip_group_check=True)
            if prewritten:
                psq = ps[:].rearrange("p (q x) -> p q x", q=4)[:, :, 0:HROWS]
                gv = gdst[:].rearrange("p (q r) -> p q r", q=4)
                nc.scalar.activation(gv, psq, fn, scale=1.0)
            else:
                gva = gdst[:].rearrange("p (m x) -> p m x", m=2)
                psa = ps[:].rearrange("p (m x) -> p m x", m=2)
                for j, mt in ((0, ma), (1, mb)):
                    nc.scalar.activation(
                        gva[:, j, :].rearrange("p (hh r) -> p hh r", hh=2),
                        psa[:, j, :].rearrange("p (hh x) -> p hh x",
                                               hh=2)[:, :, 0:HROWS],
                        fn, bias=bgf[:, mt:mt + 1], scale=1.0)
            if P + 2 < len(PAIR_SEQ):
                emit_pre(P + 2)
            if pi == 1:
                nc.gpsimd.tensor_tensor(tmp[:], gi[:], tg[:], ALU.mult)
            elif pi == 2:
                nc.vector.tensor_tensor(Cf[:], gf[:], Cf[:], ALU.mult)
                nc.vector.tensor_tensor(Cf[:], Cf[:], tmp[:], ALU.add)
                nc.scalar.activation(tcl[:], Cf[:], AF.Tanh)
            elif pi == 3:
                nc.vector.tensor_tensor(Hf[:], go[:], tcl[:], ALU.mult)

        # ===== Phase 4: backward LSTM single step (only hb[:,0] used) =====
        Hb = lp.tile([128, 2 * ROWS], F16, name="Hb")
        kb = (WIN - 2) // 2
        BPAIRS = [(0, 1, gi, AF.Sigmoid), (4, 5, go, AF.Sigmoid),
                  (6, 7, tg, AF.Tanh)]
        for ma, mb, gdst, fn in BPAIRS:
            ps = ps_l.tile([128, 2048], F32, tag="lp", name="lpb")
            for j, mt in ((0, ma), (1, mb)):
                for hh in range(2):
                    pslice = ps[:, j * 1024 + hh * 512:
                                j * 1024 + hh * 512 + HROWS]
                    b0 = hh * (BL // 2)
                    for kt in range(2):
                        fv = Fts[kt][:].rearrange("p (b k two) -> p b k two",
                                                  b=BL, two=2)
                        nc.tensor.matmul(
                            pslice, lxb[kt][:, mt * 128:(mt + 1) * 128],
                            fv[:, b0:b0 + BL // 2, kb:kb + NW, 1],
                            start=(kt == 0), stop=(kt == 1))
                psj = (ps[:, j * 1024:(j + 1) * 1024]
                       .rearrange("p (hh x) -> p hh x", hh=2)[:, :, 0:HROWS])
                gvj = (gdst[:, j * ROWS:(j + 1) * ROWS]
                       .rearrange("p (hh r) -> p hh r", hh=2))
                nc.scalar.activation(gvj, psj, fn,
                                     bias=bgb[:, mt:mt + 1], scale=1.0)
        nc.gpsimd.tensor_tensor(tmp[:], gi[:], tg[:], ALU.mult)
        nc.scalar.activation(tcl[:], tmp[:], AF.Tanh)
        nc.gpsimd.tensor_tensor(Hb[:], go[:], tcl[:], ALU.mult)

        # ===== Phase 5: FC head =====
        ps = ps_l.tile([128, 2048], F32, tag="lp", name="lpf")
        rhs4 = [Hf[:, 0:ROWS], Hf[:, ROWS:2 * ROWS],
                Hb[:, 0:ROWS], Hb[:, ROWS:2 * ROWS]]
        for hh in range(2):
            for qt in range(4):
                nc.tensor.matmul(ps[:, hh * 512:hh * 512 + HROWS],
                                 wfct[qt][:],
                                 rhs4[qt].rearrange("p (h r) -> p h r",
                                                    h=2)[:, hh, :],
                                 start=(qt == 0), stop=(qt == 3))
        ob = lp.tile([EMB, ROWS], F32, name="ob")
        obv = ob[:].rearrange("p (h r) -> p h r", h=2)
        psv = (ps[:, 0:1024].rearrange("p (h x) -> p h x", h=2)
               [:, :, 0:HROWS])
        nc.scalar.activation(obv, psv, AF.Identity,
                             bias=bfc[:, 0:1], scale=1.0)
        nc.sync.dma_start(io["out_d"][:], ob[:])

    if "fdbg0" in io:
        nc.sync.dma_start(io["fdbg0"][:], F0[:])
        nc.sync.dma_start(io["fdbg1"][:], F1[:])
    ctx.close()


def _build_program():
    nc = bacc.Bacc("TRN2", target_bir_lowering=False, debug=False,
                   num_devices=NCORES)

    def din(name, shape, dt=F16):
        return nc.dram_tensor(name, shape, dt, kind="ExternalInput").ap()

    io = dict(
        x0=din("x0", [NCH, 128, GBLK * 8]),
        mixM=din("mixM", [128, 128]),
        w1=din("w1", [FIN, 64]), w2=din("w2", [64, 128]),
        w3=din("w3", [128, 256]), w4=din("w4", [256, 256]),
        b1=din("b1", [64, 1], F32), b2=din("b2", [128, 1], F32),
        b3=din("b3", [128, 2], F32), b4=din("b4", [128, 2], F32),
        lxf=din("lxf", [256, 1024]), lhf=din("lhf", [256, 1024]),
        lxb=din("lxb", [256, 1024]),
        bgf=din("bgf", [128, 8], F32), bgb=din("bgb", [128, 8], F32),
        wfc=din("wfc", [512, 128]), bfc=din("bfc", [128, 1], F32),
        ident=din("ident", [128, 128]),
        out_d=nc.dram_tensor("out", [EMB, ROWS], F32,
                             kind="ExternalOutput").ap(),
    )
    if os.environ.get("K_FDBG", "0") == "1":
        io["fdbg0"] = nc.dram_tensor("fdbg0", [128, FTOT], F16,
                                     kind="ExternalOutput").ap()
        io["fdbg1"] = nc.dram_tensor("fdbg1", [128, FTOT], F16,
                                     kind="ExternalOutput").ap()

    with tile.TileContext(nc) as tc:
        _kernel_body(tc, io)
    nc.compile()
    return nc


def _host_prep(inputs):
    f16 = np.float16
    data = np.asarray(inputs["data"], np.float32)
    ei = np.asarray(inputs["edge_index"]).astype(np.int64)

    src = np.concatenate([ei[0], np.arange(N)])
    dst = np.concatenate([ei[1], np.arange(N)])
    deg = np.zeros(N, np.float32)
    np.add.at(deg, dst, 1.0)
    dinv = np.where(deg > 0, deg ** -0.5, 0.0).astype(np.float32)
    Ahat = np.zeros((N, N), np.float32)
    np.add.at(Ahat, (dst, src), dinv[src] * dinv[dst])
    mixM = np.zeros((128, 128), np.float32)
    mixM[0:N * G5, 0:N * G5] = np.kron(Ahat.T, np.eye(G5, dtype=np.float32))
    mixM = mixM.astype(f16)

    # x0: [core][chunk b][blk = n*5+g5 (120:128 zero)][gb*8 + c],
    # t = 5*gb+g5, channels 6:8 zero
    d = data.reshape(NCORES, BL, T, N, FIN)
    x0 = np.zeros((NCORES, BL, 128, GBLK, 8), np.float32)
    dpad = np.zeros((NCORES, BL, TP, N, FIN), np.float32)
    dpad[:, :, :T] = d
    dv = dpad.reshape(NCORES, BL, GBLK, G5, N, FIN)
    # [core, b, n, g5, gb, c]
    dv = dv.transpose(0, 1, 4, 3, 2, 5).reshape(NCORES, BL, N * G5, GBLK, FIN)
    x0[:, :, 0:N * G5, :, 0:FIN] = dv
    x0 = np.ascontiguousarray(
        x0.reshape(NCORES, BL, 128, GBLK * 8)).astype(f16)

    perm = np.concatenate([np.arange(0, H), np.arange(H, 2 * H),
                           np.arange(3 * H, 4 * H), np.arange(2 * H, 3 * H)])

    def prep_dir(wih, whh, bih, bhh):
        wihp = np.asarray(wih, np.float32)[perm] / N
        whhp = np.asarray(whh, np.float32)[perm]
        bg = (np.asarray(bih, np.float32) + np.asarray(bhh, np.float32))[perm]
        return (np.ascontiguousarray(wihp.T).astype(f16),
                np.ascontiguousarray(whhp.T).astype(f16),
                np.ascontiguousarray(bg.reshape(8, 128).T).astype(np.float32))

    lxf, lhf, bgf = prep_dir(inputs["lstm_Wih_f"], inputs["lstm_Whh_f"],
                             inputs["lstm_bih_f"], inputs["lstm_bhh_f"])
    lxb, _lhb, bgb = prep_dir(inputs["lstm_Wih_b"], inputs["lstm_Whh_b"],
                              inputs["lstm_bih_b"], inputs["lstm_bhh_b"])

    com = {
        "mixM": mixM,
        "w1": np.asarray(inputs["W1"], np.float32).astype(f16),
        "w2": np.asarray(inputs["W2"], np.float32).astype(f16),
        "w3": np.asarray(inputs["W3"], np.float32).astype(f16),
        "w4": np.asarray(inputs["W4"], np.float32).astype(f16),
        "b1": np.asarray(inputs["b1"], np.float32).reshape(64, 1),
        "b2": np.asarray(inputs["b2"], np.float32).reshape(128, 1),
        "b3": np.ascontiguousarray(
            np.asarray(inputs["b3"], np.float32).reshape(2, 128).T),
        "b4": np.ascontiguousarray(
            np.asarray(inputs["b4"], np.float32).reshape(2, 128).T),
        "lxf": lxf, "lhf": lhf, "lxb": lxb, "bgf": bgf, "bgb": bgb,
        "wfc": np.asarray(inputs["Wfc"], np.float32).astype(f16),
        "bfc": np.asarray(inputs["bfc"], np.float32).reshape(128, 1),
        "ident": np.eye(128, dtype=f16),
    }
    return [dict(com, x0=x0[c]) for c in range(NCORES)]


TRACE = False          # set by test harness to capture an NTFF profile


def kernel(**inputs) -> np.ndarray:
    if "nc" not in _CACHE:
        _CACHE["nc"] = _build_program()
    nc = _CACHE["nc"]
    in_maps = _host_prep(inputs)
    res = bass_utils.run_bass_kernel_spmd(nc, in_maps,
                                          core_ids=list(range(NCORES)),
                                          trace=TRACE)
    _CACHE["last_res"] = res
    outs = []
    for c in range(NCORES):
        o = res.results[c]["out"]                       # [128, 904]
        outs.append(o.reshape(EMB, BL, NW).transpose(1, 2, 0))
    return np.concatenate(outs, 0).astype(np.float32)   # [64, 113, 128]


if __name__ == "__main__":
    import reference
    ins = {k: np.asarray(v) for k, v in reference.setup_inputs().items()}
    out = kernel(**ins)
    print("kernel out", out.shape, out.dtype, float(np.abs(out).max()))


# revision 29
# speedup vs baseline: 1.8117x; 1.0785x over previous
"""Trainium2 Bass kernel for DeepConvGraphEncoderDownstream.

Model (per reference):
  4-layer GCN (shared dense 24x24 graph operator) applied per (batch, timestep)
  frame -> node-mean -> per sliding window (W=32, stride 2, 113 windows):
  BiLSTM(H=256) -> concat(h_fwd[-1], h_bwd[0]) @ Wfc + bfc.

Key algebraic restructurings:
  * gcn_norm folded into one dense Ahat[24,24] on host.
  * GCN runs ONCE over all 256 timesteps (the reference recomputes it ~14x
    across overlapping windows).
  * backward LSTM: only hb[:, 0] is used => exactly ONE step, no recurrence.
  * forward LSTM: all 113 windows batched into one 904-row recurrence per
    core; input transforms U precomputed from node-mean features.

Sharding: data-parallel over batch, 8 batches/core on 8 cores; output
slices are independent (no collectives).

v2 performance structure:
  * GCN processed in 16 half-chunk units (28/24 gb groups) for cross-unit
    pipelining; pools sized for ~1 unit of overlap.
  * DMA transposes round-robin over the two HWDGE rings (sync 5/6,
    scalar 1/6); x0 loads on gpsimd SWDGE.
  * PSUM->SBUF casts and bias+ReLU rotate between vector and scalar.
  * x0 packed at c=8 (6 real channels) => mix L1 is 224 cols, not 3584;
    L1->L2 transpose narrow (64 channels).
  * node-sum via single strided tensor_reduce per (kt, unit).
  * LSTM: no identity matmuls -- U+bias pre-written into PSUM by vector,
    Whh matmuls accumulate on top (start=False); activations batched per
    gate pair; elementwise state update on gpsimd.

Layouts (per core, per chunk = one local batch = 256 timesteps padded to
260 = 52 blocks * 5):
  A-layout [c_part, free=(gb, blk:128)], blk = n*5+g5 (120:128 pad),
           timestep t = 5*gb + g5.
  B-layout [blk partitions = 128, free=(gb, c)]
  A->B / B->A are single XBAR DMA-transpose instructions per c-block:
  HW semantics out[p, b, c] = in[c, b*128 + p].
  Node mixing = matmul with zero-padded stationary kron(Ahat^T, I5) [128,128].
"""

import itertools
import os
import sys
import numpy as np

try:
    import concourse.bass as bass
except ImportError:
    sys.path.insert(0, "/opt/trn_rl_repo")
    import concourse.bass as bass

import concourse.bacc as bacc
import concourse.tile as tile
from concourse import mybir
from concourse import bass_utils

F16 = mybir.dt.float16
F32 = mybir.dt.float32
AF = mybir.ActivationFunctionType
ALU = mybir.AluOpType

B, T, N, FIN = 64, 256, 24, 6
H, EMB = 256, 128
WIN = 32
NW = (T - WIN) // 2 + 1               # 113
NCORES = 8
BL = B // NCORES                      # 8
G5 = 5
GBLK = 52                             # ceil(260/5): 52*5 = 260 t-slots
TP = GBLK * G5                        # 260 padded timesteps
NCH = BL
ROWS = BL * NW                        # 904
HROWS = ROWS // 2                     # 452
FTOT = BL * TP                        # 2080 F columns
HALVES = [(0, 28), (28, 24)]          # (gb offset, gb count) per unit

PREWRITE = os.environ.get("K_PREWRITE", "1") == "1"

_CACHE = {}


def _chunks(nf, step):
    return [(i, min(step, nf - i)) for i in range(0, nf, step)]


def _kernel_body(tc, io):
    nc = tc.nc
    from contextlib import ExitStack
    ctx = ExitStack()

    cons = ctx.enter_context(tc.tile_pool(name="cons", bufs=1))
    fpool = ctx.enter_context(tc.tile_pool(name="fpool", bufs=1))

    def load_const(name, shape, dt=F16):
        t = cons.tile(shape, dt, name=name)
        nc.sync.dma_start(t[:], io[name][:])
        return t

    mixM = load_const("mixM", [128, 128])
    w1 = load_const("w1", [FIN, 64])
    w2 = load_const("w2", [64, 128])
    w3 = load_const("w3", [128, 256])
    b1 = load_const("b1", [64, 1], F32)
    b2 = load_const("b2", [128, 1], F32)
    b3 = load_const("b3", [128, 2], F32)
    b4 = load_const("b4", [128, 2], F32)
    w4k = []
    for kt in range(2):
        t = cons.tile([128, 256], F16, name=f"w4k{kt}")
        nc.sync.dma_start(t[:], io["w4"][kt * 128:(kt + 1) * 128, :])
        w4k.append(t)

    def load_ktiles(name):
        ts = []
        for kt in range(2):
            t = cons.tile([128, 1024], F16, name=f"{name}{kt}")
            nc.sync.dma_start(t[:], io[name][kt * 128:(kt + 1) * 128, :])
            ts.append(t)
        return ts

    lxf = load_ktiles("lxf")
    lhf = load_ktiles("lhf")
    lxb = load_ktiles("lxb")
    bgf = load_const("bgf", [128, 8], F32)
    bgb = load_const("bgb", [128, 8], F32)
    wfct = []
    for qt in range(4):
        t = cons.tile([128, 128], F16, name=f"wfct{qt}")
        nc.sync.dma_start(t[:], io["wfc"][qt * 128:(qt + 1) * 128, :])
        wfct.append(t)
    bfc = load_const("bfc", [128, 1], F32)
    ident = load_const("ident", [128, 128])

    F0 = fpool.tile([128, FTOT], F16, name="F0")
    F1 = fpool.tile([128, FTOT], F16, name="F1")
    Fts = [F0, F1]

    # persistent zero-padded B-layout staging tiles (pad cols stay 0)
    y1pads = [cons.tile([128, 28 * 128], F16, name=f"y1p{i}") for i in range(2)]
    y2pads = [cons.tile([128, 28 * 128], F16, name=f"y2p{i}") for i in range(2)]
    for i in range(2):
        nc.vector.memset(y1pads[i][:], 0.0)
        nc.gpsimd.memset(y2pads[i][:], 0.0)

    # engine rotations
    cast_cycle = itertools.cycle([nc.vector, nc.scalar])
    relu_cycle = itertools.cycle([nc.scalar, nc.vector])
    # concurrent XBAR transposes on the two HWDGE rings corrupt each other
    # (shared xbar S2M state) -- keep ALL transposes on the sync ring.
    tp_cycle = itertools.cycle([nc.sync])

    def cast_rot(dst, src):
        e = next(cast_cycle)
        if e is nc.scalar:
            nc.scalar.copy(dst, src)
        else:
            e.tensor_copy(dst, src)

    def relu_rot(dst, src, bias_ap):
        e = next(relu_cycle)
        if e is nc.scalar:
            nc.scalar.activation(dst, src, AF.Relu, bias=bias_ap, scale=1.0)
        else:
            e.tensor_scalar(dst, src, bias_ap, 0.0, ALU.add, ALU.max)

    def tpose(out_ap, in_ap):
        next(tp_cycle).dma_start(out_ap, in_ap, transpose=True)

    # ================= Phase 1: GCN =================
    # Units are emitted stage-interleaved (software pipeline) so pool-buffer
    # rotation reuse targets recently-freed buffers instead of coupling each
    # unit's first stage to the previous unit's last.
    with tc.tile_pool(name="gcnS", bufs=4) as gpS, \
         tc.tile_pool(name="gcnA", bufs=9) as gpA, \
         tc.tile_pool(name="gcnB", bufs=5) as gpB, \
         tc.tile_pool(name="gcnBn", bufs=3) as gpBn, \
         tc.tile_pool(name="gcnBig", bufs=2) as gpBig, \
         tc.tile_pool(name="gps", bufs=4, space="PSUM") as ps_g:

        def mix(src_ap, n_free, cast_fn):
            """node-mix src_ap [128, n_free] via mixM; cast_fn(c0, cw, ps_ap)
            moves each fp32 psum chunk into its fp16 destination."""
            for c0, cw in _chunks(n_free, 1024):
                ps = ps_g.tile([128, 1024], F32, tag="ps", name="mps")
                for s0, sw in _chunks(cw, 512):
                    nc.tensor.matmul(ps[:, s0:s0 + sw], mixM[:],
                                     src_ap[:, c0 + s0:c0 + s0 + sw],
                                     start=True, stop=True)
                cast_fn(c0, cw, ps[:, 0:cw])

        def transform(rhs_list, wslices, bias, cout, out_tiles, FH, mtr):
            """dense channel transform for output c-tile mtr; bias+ReLU fused
            in the PSUM->SBUF move."""
            nkt = len(rhs_list)
            mp = min(cout, 128)
            for f0, fw in _chunks(FH, 1024):
                ps = ps_g.tile([128, 1024], F32, tag="ps", name="tps")
                for s0, sw in _chunks(fw, 512):
                    g0 = f0 + s0
                    for kt in range(nkt):
                        w = wslices[kt]
                        wap = w[:, mtr * 128:(mtr + 1) * 128] if cout > 128 \
                            else w
                        nc.tensor.matmul(ps[0:mp, s0:s0 + sw], wap,
                                         rhs_list[kt][:, g0:g0 + sw],
                                         start=(kt == 0),
                                         stop=(kt == nkt - 1))
                relu_rot(out_tiles[mtr][0:mp, f0:f0 + fw],
                         ps[0:mp, 0:fw], bias[0:mp, mtr:mtr + 1])

        def unit_stages(uidx):
            k, hf_i = uidx // 2, uidx % 2
            g0gb, gh = HALVES[hf_i]
            FH = gh * 128

            # --- L1: packed-c8 input, mix at c=8, narrow transform 6->64
            x0t = gpS.tile([128, 28 * 8], F16, tag="x0", name="x0t")
            nc.gpsimd.dma_start(x0t[:, 0:gh * 8],
                                io["x0"][k][:, g0gb * 8:(g0gb + gh) * 8])
            y1p = y1pads[uidx % 2]

            def cast_y1(c0, cw, ps_ap):
                cast_rot(y1p[:, 0:gh * 128]
                         .rearrange("p (gb c) -> p gb c", c=128)[:, :, 0:8],
                         ps_ap.rearrange("p (gb c) -> p gb c", c=8))

            mix(x0t[:, 0:gh * 8], gh * 8, cast_y1)
            yield
            y1a = gpA.tile([128, 28 * 128], F16, tag="cA", name="y1a")
            tpose(y1a[:, 0:FH].rearrange("c (gb p) -> c gb p", p=128),
                  y1p[:, 0:FH])
            yield
            x1a = gpA.tile([128, 28 * 128], F16, tag="cA", name="x1a")
            transform([y1a[0:FIN, 0:FH]], [w1[:]], b1, 64, [x1a], FH, 0)
            yield

            # --- L2: narrow A->B (64 c), mix at 64, transform 64->128
            x2b = gpBn.tile([128, 28 * 64], F16, tag="cBn", name="x2b")
            tpose(x2b[:, 0:gh * 64].rearrange("p (gb c) -> p gb c", c=64),
                  x1a[0:64, 0:FH])
            yield
            y2p = y2pads[uidx % 2]

            def cast_y2(c0, cw, ps_ap):
                cast_rot(y2p[:, 0:gh * 128]
                         .rearrange("p (gb c) -> p gb c", c=128)
                         [:, c0 // 64:(c0 + cw) // 64, 0:64],
                         ps_ap.rearrange("p (gb c) -> p gb c", c=64))

            mix(x2b[:, 0:gh * 64], gh * 64, cast_y2)
            yield
            y2a = gpA.tile([128, 28 * 128], F16, tag="cA", name="y2a")
            tpose(y2a[:, 0:FH].rearrange("c (gb p) -> c gb p", p=128),
                  y2p[:, 0:FH])
            yield
            x2a = gpA.tile([128, 28 * 128], F16, tag="cA", name="x2a")
            transform([y2a[0:64, 0:FH]], [w2[:]], b2, 128, [x2a], FH, 0)
            yield

            # --- L3: mix@128 -> transform 128->256
            x3b = gpB.tile([128, 28 * 128], F16, tag="cB", name="x3b")
            tpose(x3b[:, 0:FH].rearrange("p (gb c) -> p gb c", c=128),
                  x2a[:, 0:FH])
            yield
            y3b = gpB.tile([128, 28 * 128], F16, tag="cB", name="y3b")

            def cast_y3(c0, cw, ps_ap):
                cast_rot(y3b[:, c0:c0 + cw], ps_ap)

            mix(x3b[:, 0:FH], FH, cast_y3)
            yield
            y3a = gpA.tile([128, 28 * 128], F16, tag="cA", name="y3a")
            tpose(y3a[:, 0:FH].rearrange("c (gb p) -> c gb p", p=128),
                  y3b[:, 0:FH])
            yield
            x3a0 = gpA.tile([128, 28 * 128], F16, tag="cA", name="x3a0")
            x3a1 = gpA.tile([128, 28 * 128], F16, tag="cA", name="x3a1")
            transform([y3a[:, 0:FH]], [w3[:]], b3, 256, [x3a0, x3a1], FH, 0)
            yield
            transform([y3a[:, 0:FH]], [w3[:]], b3, 256, [x3a0, x3a1], FH, 1)
            yield

            # --- L4: mix@256 -> transform 256->256
            x4b = gpBig.tile([128, 28 * 256], F16, tag="big", name="x4b")
            x4bv = x4b[:, 0:gh * 256].rearrange("p (gb c) -> p gb c", c=256)
            tpose(x4bv[:, :, 0:128], x3a0[:, 0:FH])
            yield
            tpose(x4bv[:, :, 128:256], x3a1[:, 0:FH])
            yield
            ylo = gpB.tile([128, 28 * 128], F16, tag="cB", name="ylo")
            yhi = gpB.tile([128, 28 * 128], F16, tag="cB", name="yhi")

            def cast_y4(c0, cw, ps_ap):
                psv = ps_ap.rearrange("p (g c) -> p g c", c=256)
                g = cw // 256
                for hf in range(2):
                    dv = (ylo, yhi)[hf][:].rearrange(
                        "p (gb c) -> p gb c", c=128)
                    cast_rot(dv[:, c0 // 256:c0 // 256 + g, :],
                             psv[:, :, hf * 128:(hf + 1) * 128])

            mix(x4b[:, 0:gh * 256], gh * 256, cast_y4)
            yield
            y4a0 = gpA.tile([128, 28 * 128], F16, tag="cA", name="y4a0")
            y4a1 = gpA.tile([128, 28 * 128], F16, tag="cA", name="y4a1")
            tpose(y4a0[:, 0:FH].rearrange("c (gb p) -> c gb p", p=128),
                  ylo[:, 0:FH])
            yield
            tpose(y4a1[:, 0:FH].rearrange("c (gb p) -> c gb p", p=128),
                  yhi[:, 0:FH])
            yield
            x4a0 = gpA.tile([128, 28 * 128], F16, tag="cA", name="x4a0")
            x4a1 = gpA.tile([128, 28 * 128], F16, tag="cA", name="x4a1")
            transform([y4a0[:, 0:FH], y4a1[:, 0:FH]],
                      [w4k[0][:], w4k[1][:]], b4, 256, [x4a0, x4a1], FH, 0)
            yield
            transform([y4a0[:, 0:FH], y4a1[:, 0:FH]],
                      [w4k[0][:], w4k[1][:]], b4, 256, [x4a0, x4a1], FH, 1)
            yield

            # node-sum into F: F[:, k*TP + t], t = 5*(g0gb+gb) + g5
            for ct, xt in enumerate((x4a0, x4a1)):
                xv = (xt[:, 0:FH]
                      .rearrange("p (gb blk) -> p gb blk", blk=128)
                      [:, :, 0:120]
                      .rearrange("p gb (n g5) -> p gb g5 n", g5=G5))
                dstv = (Fts[ct][:, k * TP + g0gb * G5:
                                k * TP + (g0gb + gh) * G5]
                        .rearrange("p (gb g5) -> p gb g5", g5=G5))
                with nc.allow_low_precision("node-sum in fp16, as baseline"):
                    nc.vector.tensor_reduce(dstv, xv, mybir.AxisListType.X,
                                            ALU.add)

        # skewed round-robin driver: admit the next unit once the newest
        # active one is SKEW stages in; emit one stage per active unit.
        SKEW = int(os.environ.get("K_SKEW", "6"))
        gens = [unit_stages(u) for u in range(NCH * 2)]
        active, nxt, prog = [], 0, {}
        while active or nxt < len(gens):
            if nxt < len(gens) and (not active or prog[active[-1]] >= SKEW):
                active.append(nxt)
                prog[nxt] = 0
                nxt += 1
            for u in list(active):
                try:
                    next(gens[u])
                    prog[u] += 1
                except StopIteration:
                    active.remove(u)

    # ================= Phase 2: U = F @ (Wih_f/24)^T =================
    # U stored par-major: col = par*(BL*130) + b*130 + kk, where the source
    # F column is b*260 + 2*kk + par.  This gives the LSTM pre-writes a
    # unit-stride inner dim.
    upool = ctx.enter_context(tc.tile_pool(name="upool", bufs=1))
    KK = TP // 2                                  # 130
    Umt = []
    with tc.tile_pool(name="ups", bufs=4, space="PSUM") as ps_u:
        for mt in range(8):
            u = upool.tile([128, FTOT], F16, name=f"U{mt}")
            uview = u[:].rearrange("p (par b kk) -> p b kk par",
                                   par=2, kk=KK)
            for b in range(BL):
                ps = ps_u.tile([128, 512], F32, tag="ups", name="ups")
                for kt in range(2):
                    nc.tensor.matmul(ps[:, 0:TP],
                                     lxf[kt][:, mt * 128:(mt + 1) * 128],
                                     Fts[kt][:, b * TP:(b + 1) * TP],
                                     start=(kt == 0), stop=(kt == 1))
                # fold the forward gate bias into U here so LSTM acts and
                # ident-injections need no per-mt bias
                src = ps[:, 0:TP].rearrange("p (kk par) -> p kk par", par=2)
                dst = uview[:, b]
                e = next(cast_cycle)
                if e is nc.scalar:
                    nc.scalar.activation(dst, src, AF.Identity,
                                         bias=bgf[:, mt:mt + 1], scale=1.0)
                else:
                    e.tensor_scalar(dst, src, bgf[:, mt:mt + 1],
                                    None, ALU.add)
            Umt.append(u)

    # ================= Phase 3: forward LSTM =================
    lp = ctx.enter_context(tc.tile_pool(name="lstm", bufs=1))
    Hf = lp.tile([128, 2 * ROWS], F16, name="Hf")
    Cf = lp.tile([128, 2 * ROWS], F16, name="Cf")
    nc.vector.memset(Hf[:], 0.0)
    nc.gpsimd.memset(Cf[:], 0.0)
    Cf2 = lp.tile([128, 2 * ROWS], F16, name="Cf2")
    gi = lp.tile([128, 2 * ROWS], F16, name="gi")
    gf = lp.tile([128, 2 * ROWS], F16, name="gf")
    go = lp.tile([128, 2 * ROWS], F16, name="go")
    tg = lp.tile([128, 2 * ROWS], F16, name="tg")
    tcl = lp.tile([128, 2 * ROWS], F16, name="tcl")
    tmp = lp.tile([128, 2 * ROWS], F16, name="tmp")
    tmp2 = lp.tile([128, 2 * ROWS], F16, name="tmp2")
    # pair order (g, i, f, o): the c/h elementwise update interleaves with
    # the later pairs instead of forming a serial tail after all four.
    PAIRS = [(6, 7, tg, AF.Tanh), (0, 1, gi, AF.Sigmoid),
             (2, 3, gf, AF.Sigmoid), (4, 5, go, AF.Sigmoid)]

    with tc.tile_pool(name="lps", bufs=2, space="PSUM") as ps_l:
        # software pipeline over the global pair sequence: each pair's PSUM
        # pre-write is emitted right after the act that frees its buffer
        # (2 pairs earlier), so vector/scalar flow without step-tail stalls.
        PAIR_SEQ = [(s, pi) for s in range(WIN) for pi in range(4)]
        ptile = {}

        def emit_pre(P):
            # only pair 0 (g) gets a vector pre-write: its buffer frees
            # mid-previous-step so the write costs no stall.  Later pairs
            # inject U via identity matmuls at mm time -- a pre-write there
            # would stall on the pair-buffer WAR (only 2 pair tiles fit in
            # PSUM) and gate the PE stream.
            s, pi = PAIR_SEQ[P]
            if not PREWRITE or pi >= 1:
                return
            k0, par = s // 2, s % 2
            ma, mb, _, _ = PAIRS[pi]
            ps = ps_l.tile([128, 2048], F32, tag="lp", name="lp")
            for j, mt in ((0, ma), (1, mb)):
                uv = Umt[mt][:].rearrange("p (par b kk) -> p par b kk",
                                          par=2, kk=KK)
                src = (uv[:, par, :, k0:k0 + NW]
                       .rearrange("p (hh b) k -> p hh b k", hh=2))
                dst = (ps[:, j * 1024:(j + 1) * 1024]
                       .rearrange("p (hh x) -> p hh x", hh=2)
                       [:, :, 0:HROWS]
                       .rearrange("p hh (b k) -> p hh b k", k=NW))
                nc.vector.tensor_copy(dst, src)
            ptile[P] = ps

        emit_pre(0)
        emit_pre(1)
        Cst = [Cf, Cf2]          # ping-pong: out-of-place DVE runs 2x rate
        for P, (s, pi) in enumerate(PAIR_SEQ):
            k0, par = s // 2, s % 2
            ma, mb, gdst, fn = PAIRS[pi]
            Ccur, Cnxt = Cst[s % 2], Cst[1 - s % 2]
            prewritten = P in ptile
            ps = ptile.pop(P) if prewritten \
                else ps_l.tile([128, 2048], F32, tag="lp", name="lp")
            if not prewritten:
                # inject U (+folded bias) via PE before the Whh accumulation
                for j, mt in ((0, ma), (1, mb)):
                    uv = Umt[mt][:].rearrange(
                        "p (par b kk) -> p par b kk", par=2, kk=KK)
                    for hh in range(2):
                        b0 = hh * (BL // 2)
                        nc.tensor.matmul(
                            ps[:, j * 1024 + hh * 512:
                               j * 1024 + hh * 512 + HROWS],
                            ident[:],
                            uv[:, par, b0:b0 + BL // 2, k0:k0 + NW],
                            start=True, stop=False)
            # kt-outer so next step's kt0 mms only need the Hf0 half
            for kt in range(2):
                for j, mt in ((0, ma), (1, mb)):
                    for hh in range(2):
                        nc.tensor.matmul(
                            ps[:, j * 1024 + hh * 512:
                               j * 1024 + hh * 512 + HROWS],
                            lhf[kt][:, mt * 128:(mt + 1) * 128],
                            Hf[:, kt * ROWS + hh * HROWS:
                               kt * ROWS + (hh + 1) * HROWS],
                            start=False, stop=(kt == 1),
                            skip_group_check=True)
            if P + 2 < len(PAIR_SEQ):
                emit_pre(P + 2)
            if pi < 3:
                psq = ps[:].rearrange("p (q x) -> p q x", q=4)[:, :, 0:HROWS]
                gv = gdst[:].rearrange("p (q r) -> p q r", q=4)
                nc.scalar.activation(gv, psq, fn, scale=1.0)
            if pi == 1:
                for j in range(2):
                    sl = slice(j * ROWS, (j + 1) * ROWS)
                    nc.gpsimd.tensor_tensor(tmp[:, sl], gi[:, sl],
                                            tg[:, sl], ALU.mult)
            elif pi == 2:
                for j in range(2):
                    sl = slice(j * ROWS, (j + 1) * ROWS)
                    nc.vector.tensor_tensor(tmp2[:, sl], gf[:, sl],
                                            Ccur[:, sl], ALU.mult)
                    nc.vector.tensor_tensor(Cnxt[:, sl], tmp2[:, sl],
                                            tmp[:, sl], ALU.add)
            elif pi == 3:
                # per-half act -> tanh(C) -> Hf so next step's kt0 matmuls
                # start as soon as half 0 is ready
                for j in range(2):
                    sl = slice(j * ROWS, (j + 1) * ROWS)
                    psj = (ps[:, j * 1024:(j + 1) * 1024]
                           .rearrange("p (hh x) -> p hh x", hh=2)
                           [:, :, 0:HROWS])
                    gvj = (go[:, sl].rearrange("p (hh r) -> p hh r", hh=2))
                    nc.scalar.activation(gvj, psj, fn, scale=1.0)
                    nc.scalar.activation(tcl[:, sl], Cnxt[:, sl], AF.Tanh)
                    nc.vector.tensor_tensor(Hf[:, sl], go[:, sl],
                                            tcl[:, sl], ALU.mult)

        # ===== Phase 4: backward LSTM single step (only hb[:,0] used) =====
        Hb = lp.tile([128, 2 * ROWS], F16, name="Hb")
        kb = (WIN - 2) // 2
        BPAIRS = [(0, 1, gi, AF.Sigmoid), (4, 5, go, AF.Sigmoid),
                  (6, 7, tg, AF.Tanh)]
        for ma, mb, gdst, fn in BPAIRS:
            ps = ps_l.tile([128, 2048], F32, tag="lp", name="lpb")
            for j, mt in ((0, ma), (1, mb)):
                for hh in range(2):
                    pslice = ps[:, j * 1024 + hh * 512:
                                j * 1024 + hh * 512 + HROWS]
                    b0 = hh * (BL // 2)
                    for kt in range(2):
                        fv = Fts[kt][:].rearrange("p (b k two) -> p b k two",
                                                  b=BL, two=2)
                        nc.tensor.matmul(
                            pslice, lxb[kt][:, mt * 128:(mt + 1) * 128],
                            fv[:, b0:b0 + BL // 2, kb:kb + NW, 1],
                            start=(kt == 0), stop=(kt == 1))
                psj = (ps[:, j * 1024:(j + 1) * 1024]
                       .rearrange("p (hh x) -> p hh x", hh=2)[:, :, 0:HROWS])
                gvj = (gdst[:, j * ROWS:(j + 1) * ROWS]
                       .rearrange("p (hh r) -> p hh r", hh=2))
                nc.scalar.activation(gvj, psj, fn,
                                     bias=bgb[:, mt:mt + 1], scale=1.0)
        nc.gpsimd.tensor_tensor(tmp[:], gi[:], tg[:], ALU.mult)
        nc.scalar.activation(tcl[:], tmp[:], AF.Tanh)
        nc.gpsimd.tensor_tensor(Hb[:], go[:], tcl[:], ALU.mult)

        # ===== Phase 5: FC head =====
        ps = ps_l.tile([128, 2048], F32, tag="lp", name="lpf")
        rhs4 = [Hf[:, 0:ROWS], Hf[:, ROWS:2 * ROWS],
                Hb[:, 0:ROWS], Hb[:, ROWS:2 * ROWS]]
        for hh in range(2):
            for qt in range(4):
                nc.tensor.matmul(ps[:, hh * 512:hh * 512 + HROWS],
                                 wfct[qt][:],
                                 rhs4[qt].rearrange("p (h r) -> p h r",
                                                    h=2)[:, hh, :],
                                 start=(qt == 0), stop=(qt == 3))
        ob = lp.tile([EMB, ROWS], F32, name="ob")
        obv = ob[:].rearrange("p (h r) -> p h r", h=2)
        psv = (ps[:, 0:1024].rearrange("p (h x) -> p h x", h=2)
               [:, :, 0:HROWS])
        nc.scalar.activation(obv, psv, AF.Identity,
                             bias=bfc[:, 0:1], scale=1.0)
        nc.sync.dma_start(io["out_d"][:], ob[:])

    if "fdbg0" in io:
        nc.sync.dma_start(io["fdbg0"][:], F0[:])
        nc.sync.dma_start(io["fdbg1"][:], F1[:])
    ctx.close()


def _build_program():
    nc = bacc.Bacc("TRN2", target_bir_lowering=False, debug=False,
                   num_devices=NCORES)

    def din(name, shape, dt=F16):
        return nc.dram_tensor(name, shape, dt, kind="ExternalInput").ap()

    io = dict(
        x0=din("x0", [NCH, 128, GBLK * 8]),
        mixM=din("mixM", [128, 128]),
        w1=din("w1", [FIN, 64]), w2=din("w2", [64, 128]),
        w3=din("w3", [128, 256]), w4=din("w4", [256, 256]),
        b1=din("b1", [64, 1], F32), b2=din("b2", [128, 1], F32),
        b3=din("b3", [128, 2], F32), b4=din("b4", [128, 2], F32),
        lxf=din("lxf", [256, 1024]), lhf=din("lhf", [256, 1024]),
        lxb=din("lxb", [256, 1024]),
        bgf=din("bgf", [128, 8], F32), bgb=din("bgb", [128, 8], F32),
        wfc=din("wfc", [512, 128]), bfc=din("bfc", [128, 1], F32),
        ident=din("ident", [128, 128]),
        out_d=nc.dram_tensor("out", [EMB, ROWS], F32,
                             kind="ExternalOutput").ap(),
    )
    if os.environ.get("K_FDBG", "0") == "1":
        io["fdbg0"] = nc.dram_tensor("fdbg0", [128, FTOT], F16,
                                     kind="ExternalOutput").ap()
        io["fdbg1"] = nc.dram_tensor("fdbg1", [128, FTOT], F16,
                                     kind="ExternalOutput").ap()

    with tile.TileContext(nc) as tc:
        _kernel_body(tc, io)
    nc.compile()
    return nc


def _host_prep(inputs):
    f16 = np.float16
    data = np.asarray(inputs["data"], np.float32)
    ei = np.asarray(inputs["edge_index"]).astype(np.int64)

    src = np.concatenate([ei[0], np.arange(N)])
    dst = np.concatenate([ei[1], np.arange(N)])
    deg = np.zeros(N, np.float32)
    np.add.at(deg, dst, 1.0)
    dinv = np.where(deg > 0, deg ** -0.5, 0.0).astype(np.float32)
    Ahat = np.zeros((N, N), np.float32)
    np.add.at(Ahat, (dst, src), dinv[src] * dinv[dst])
    mixM = np.zeros((128, 128), np.float32)
    mixM[0:N * G5, 0:N * G5] = np.kron(Ahat.T, np.eye(G5, dtype=np.float32))
    mixM = mixM.astype(f16)

    # x0: [core][chunk b][blk = n*5+g5 (120:128 zero)][gb*8 + c],
    # t = 5*gb+g5, channels 6:8 zero
    d = data.reshape(NCORES, BL, T, N, FIN)
    x0 = np.zeros((NCORES, BL, 128, GBLK, 8), np.float32)
    dpad = np.zeros((NCORES, BL, TP, N, FIN), np.float32)
    dpad[:, :, :T] = d
    dv = dpad.reshape(NCORES, BL, GBLK, G5, N, FIN)
    # [core, b, n, g5, gb, c]
    dv = dv.transpose(0, 1, 4, 3, 2, 5).reshape(NCORES, BL, N * G5, GBLK, FIN)
    x0[:, :, 0:N * G5, :, 0:FIN] = dv
    x0 = np.ascontiguousarray(
        x0.reshape(NCORES, BL, 128, GBLK * 8)).astype(f16)

    perm = np.concatenate([np.arange(0, H), np.arange(H, 2 * H),
                           np.arange(3 * H, 4 * H), np.arange(2 * H, 3 * H)])

    def prep_dir(wih, whh, bih, bhh):
        wihp = np.asarray(wih, np.float32)[perm] / N
        whhp = np.asarray(whh, np.float32)[perm]
        bg = (np.asarray(bih, np.float32) + np.asarray(bhh, np.float32))[perm]
        return (np.ascontiguousarray(wihp.T).astype(f16),
                np.ascontiguousarray(whhp.T).astype(f16),
                np.ascontiguousarray(bg.reshape(8, 128).T).astype(np.float32))

    lxf, lhf, bgf = prep_dir(inputs["lstm_Wih_f"], inputs["lstm_Whh_f"],
                             inputs["lstm_bih_f"], inputs["lstm_bhh_f"])
    lxb, _lhb, bgb = prep_dir(inputs["lstm_Wih_b"], inputs["lstm_Whh_b"],
                              inputs["lstm_bih_b"], inputs["lstm_bhh_b"])

    com = {
        "mixM": mixM,
        "w1": np.asarray(inputs["W1"], np.float32).astype(f16),
        "w2": np.asarray(inputs["W2"], np.float32).astype(f16),
        "w3": np.asarray(inputs["W3"], np.float32).astype(f16),
        "w4": np.asarray(inputs["W4"], np.float32).astype(f16),
        "b1": np.asarray(inputs["b1"], np.float32).reshape(64, 1),
        "b2": np.asarray(inputs["b2"], np.float32).reshape(128, 1),
        "b3": np.ascontiguousarray(
            np.asarray(inputs["b3"], np.float32).reshape(2, 128).T),
        "b4": np.ascontiguousarray(
            np.asarray(inputs["b4"], np.float32).reshape(2, 128).T),
        "lxf": lxf, "lhf": lhf, "lxb": lxb, "bgf": bgf, "bgb": bgb,
        "wfc": np.asarray(inputs["Wfc"], np.float32).astype(f16),
        "bfc": np.asarray(inputs["bfc"], np.float32).reshape(128, 1),
        "ident": np.eye(128, dtype=f16),
    }
    return [dict(com, x0=x0[c]) for c in range(NCORES)]


TRACE = False          # set by test harness to capture an NTFF profile


def kernel(**inputs) -> np.ndarray:
    if "nc" not in _CACHE:
        _CACHE["nc"] = _build_program()
    nc = _CACHE["nc"]
    in_maps = _host_prep(inputs)
    res = bass_utils.run_bass_kernel_spmd(nc, in_maps,
                                          core_ids=list(range(NCORES)),
                                          trace=TRACE)
    _CACHE["last_res"] = res
    outs = []
    for c in range(NCORES):
        o = res.results[c]["out"]                       # [128, 904]
        outs.append(o.reshape(EMB, BL, NW).transpose(1, 2, 0))
    return np.concatenate(outs, 0).astype(np.float32)   # [64, 113, 128]


if __name__ == "__main__":
    import reference
    ins = {k: np.asarray(v) for k, v in reference.setup_inputs().items()}
    out = kernel(**ins)
    print("kernel out", out.shape, out.dtype, float(np.abs(out).max()))


# revision 37
# speedup vs baseline: 1.9291x; 1.0648x over previous
"""Trainium2 Bass kernel for DeepConvGraphEncoderDownstream.

Model (per reference):
  4-layer GCN (shared dense 24x24 graph operator) applied per (batch, timestep)
  frame -> node-mean -> per sliding window (W=32, stride 2, 113 windows):
  BiLSTM(H=256) -> concat(h_fwd[-1], h_bwd[0]) @ Wfc + bfc.

Key algebraic restructurings:
  * gcn_norm folded into one dense Ahat[24,24] on host.
  * GCN runs ONCE over all 256 timesteps (the reference recomputes it ~14x
    across overlapping windows).
  * backward LSTM: only hb[:, 0] is used => exactly ONE step, no recurrence.
  * forward LSTM: all 113 windows batched into one 904-row recurrence per
    core; input transforms U precomputed from node-mean features.

Sharding: data-parallel over batch, 8 batches/core on 8 cores; output
slices are independent (no collectives).

v2 performance structure:
  * GCN processed in 16 half-chunk units (28/24 gb groups) for cross-unit
    pipelining; pools sized for ~1 unit of overlap.
  * DMA transposes round-robin over the two HWDGE rings (sync 5/6,
    scalar 1/6); x0 loads on gpsimd SWDGE.
  * PSUM->SBUF casts and bias+ReLU rotate between vector and scalar.
  * x0 packed at c=8 (6 real channels) => mix L1 is 224 cols, not 3584;
    L1->L2 transpose narrow (64 channels).
  * node-sum via single strided tensor_reduce per (kt, unit).
  * LSTM: no identity matmuls -- U+bias pre-written into PSUM by vector,
    Whh matmuls accumulate on top (start=False); activations batched per
    gate pair; elementwise state update on gpsimd.

Layouts (per core, per chunk = one local batch = 256 timesteps padded to
260 = 52 blocks * 5):
  A-layout [c_part, free=(gb, blk:128)], blk = n*5+g5 (120:128 pad),
           timestep t = 5*gb + g5.
  B-layout [blk partitions = 128, free=(gb, c)]
  A->B / B->A are single XBAR DMA-transpose instructions per c-block:
  HW semantics out[p, b, c] = in[c, b*128 + p].
  Node mixing = matmul with zero-padded stationary kron(Ahat^T, I5) [128,128].
"""

import itertools
import os
import sys
import numpy as np

try:
    import concourse.bass as bass
except ImportError:
    sys.path.insert(0, "/opt/trn_rl_repo")
    import concourse.bass as bass

import concourse.bacc as bacc
import concourse.tile as tile
from concourse import mybir
from concourse import bass_utils

F16 = mybir.dt.float16
F32 = mybir.dt.float32
AF = mybir.ActivationFunctionType
ALU = mybir.AluOpType

B, T, N, FIN = 64, 256, 24, 6
H, EMB = 256, 128
WIN = 32
NW = (T - WIN) // 2 + 1               # 113
NCORES = 8
BL = B // NCORES                      # 8
G5 = 5
GBLK = 52                             # ceil(260/5): 52*5 = 260 t-slots
TP = GBLK * G5                        # 260 padded timesteps
NCH = BL
ROWS = BL * NW                        # 904
HROWS = ROWS // 2                     # 452
FTOT = BL * TP                        # 2080 F columns
HALVES = [(0, 28), (28, 24)]          # (gb offset, gb count) per unit

PREWRITE = os.environ.get("K_PREWRITE", "1") == "1"

_CACHE = {}


def _chunks(nf, step):
    return [(i, min(step, nf - i)) for i in range(0, nf, step)]


def _kernel_body(tc, io):
    nc = tc.nc
    from contextlib import ExitStack
    ctx = ExitStack()

    cons = ctx.enter_context(tc.tile_pool(name="cons", bufs=1))
    fpool = ctx.enter_context(tc.tile_pool(name="fpool", bufs=1))

    def load_const(name, shape, dt=F16):
        t = cons.tile(shape, dt, name=name)
        nc.sync.dma_start(t[:], io[name][:])
        return t

    mixM = load_const("mixM", [128, 128])
    w1 = load_const("w1", [FIN, 64])
    w2 = load_const("w2", [64, 128])
    w3 = load_const("w3", [128, 256])
    b1n = load_const("b1n", [128, 28 * 64])   # -b1 bcast over (gb, c)
    b1p = load_const("b1p", [128, 28 * 64])   # +b1 bcast
    b2 = load_const("b2", [128, 1], F32)
    b3 = load_const("b3", [128, 2], F32)
    b4 = load_const("b4", [128, 2], F32)
    w4k = []
    for kt in range(2):
        t = cons.tile([128, 256], F16, name=f"w4k{kt}")
        nc.sync.dma_start(t[:], io["w4"][kt * 128:(kt + 1) * 128, :])
        w4k.append(t)

    def load_ktiles(name):
        ts = []
        for kt in range(2):
            t = cons.tile([128, 1024], F16, name=f"{name}{kt}")
            nc.sync.dma_start(t[:], io[name][kt * 128:(kt + 1) * 128, :])
            ts.append(t)
        return ts

    lxf = load_ktiles("lxf")
    lhf = load_ktiles("lhf")
    lxb = load_ktiles("lxb")
    bgf = load_const("bgf", [128, 8], F32)
    bgb = load_const("bgb", [128, 8], F32)
    wfct = []
    for qt in range(4):
        t = cons.tile([128, 128], F16, name=f"wfct{qt}")
        nc.sync.dma_start(t[:], io["wfc"][qt * 128:(qt + 1) * 128, :])
        wfct.append(t)
    bfc = load_const("bfc", [128, 1], F32)
    ident = load_const("ident", [128, 128])

    F0 = fpool.tile([128, FTOT], F16, name="F0")
    F1 = fpool.tile([128, FTOT], F16, name="F1")
    Fts = [F0, F1]
    KK = TP // 2                                  # 130
    Umt = [fpool.tile([128, FTOT], F16, name=f"U{mt}") for mt in range(8)]

    # persistent zero-padded B-layout staging tiles (pad cols stay 0)
    y2pads = [cons.tile([128, 28 * 128], F16, name=f"y2p{i}") for i in range(2)]
    for i in range(2):
        nc.gpsimd.memset(y2pads[i][:], 0.0)

    # engine rotations
    cast_cycle = itertools.cycle([nc.vector, nc.scalar])
    relu_cycle = itertools.cycle([nc.scalar, nc.vector])
    # concurrent XBAR transposes on the two HWDGE rings corrupt each other
    # (shared xbar S2M state) -- keep ALL transposes on the sync ring.
    tp_cycle = itertools.cycle([nc.sync])

    def cast_rot(dst, src):
        e = next(cast_cycle)
        if e is nc.scalar:
            nc.scalar.copy(dst, src)
        else:
            e.tensor_copy(dst, src)

    def relu_rot(dst, src, bias_ap):
        e = next(relu_cycle)
        if e is nc.scalar:
            nc.scalar.activation(dst, src, AF.Relu, bias=bias_ap, scale=1.0)
        else:
            e.tensor_scalar(dst, src, bias_ap, 0.0, ALU.add, ALU.max)

    def tpose(out_ap, in_ap):
        next(tp_cycle).dma_start(out_ap, in_ap, transpose=True)

    # ================= Phase 1: GCN =================
    # Units are emitted stage-interleaved (software pipeline) so pool-buffer
    # rotation reuse targets recently-freed buffers instead of coupling each
    # unit's first stage to the previous unit's last.
    with tc.tile_pool(name="gcnS", bufs=2) as gpS, \
         tc.tile_pool(name="gcnA", bufs=8) as gpA, \
         tc.tile_pool(name="gcnB", bufs=6) as gpB, \
         tc.tile_pool(name="gcnBn", bufs=3) as gpBn, \
         tc.tile_pool(name="gps", bufs=4, space="PSUM") as ps_g:

        def mix(src_ap, n_free, cast_fn):
            """node-mix src_ap [128, n_free] via mixM; cast_fn(c0, cw, ps_ap)
            moves each fp32 psum chunk into its fp16 destination."""
            for c0, cw in _chunks(n_free, 1024):
                ps = ps_g.tile([128, 1024], F32, tag="ps", name="mps")
                for s0, sw in _chunks(cw, 512):
                    nc.tensor.matmul(ps[:, s0:s0 + sw], mixM[:],
                                     src_ap[:, c0 + s0:c0 + s0 + sw],
                                     start=True, stop=True)
                cast_fn(c0, cw, ps[:, 0:cw])

        def transform(rhs_list, wslices, bias, cout, out_tiles, FH, mtr):
            """dense channel transform for output c-tile mtr; bias+ReLU
            (or a plain cast when bias is None) fused in the PSUM->SBUF
            move."""
            nkt = len(rhs_list)
            mp = min(cout, 128)
            for f0, fw in _chunks(FH, 1024):
                ps = ps_g.tile([128, 1024], F32, tag="ps", name="tps")
                for s0, sw in _chunks(fw, 512):
                    g0 = f0 + s0
                    for kt in range(nkt):
                        w = wslices[kt]
                        wap = w[:, mtr * 128:(mtr + 1) * 128] if cout > 128 \
                            else w
                        nc.tensor.matmul(ps[0:mp, s0:s0 + sw], wap,
                                         rhs_list[kt][:, g0:g0 + sw],
                                         start=(kt == 0),
                                         stop=(kt == nkt - 1))
                if bias is None:
                    cast_rot(out_tiles[mtr][0:mp, f0:f0 + fw],
                             ps[0:mp, 0:fw])
                else:
                    relu_rot(out_tiles[mtr][0:mp, f0:f0 + fw],
                             ps[0:mp, 0:fw], bias[0:mp, mtr:mtr + 1])

        def unit_stages(uidx):
            k, hf_i = uidx // 2, uidx % 2
            g0gb, gh = HALVES[hf_i]
            FH = gh * 128

            # --- L1 transform-first: h1 = x0 @ W1 straight from the
            # A-layout HBM input (no B-layout input, no T1 transpose)
            x0a = gpS.tile([FIN, 28 * 128], F16, tag="x0", name="x0a")
            nc.gpsimd.dma_start(x0a[:, 0:FH],
                                io["x0"][k][:, g0gb * 128:(g0gb + gh) * 128])
            yield
            h1a = gpA.tile([128, 28 * 128], F16, tag="cA", name="h1a")
            transform([x0a[:, 0:FH]], [w1[:]], None, 64, [h1a], FH, 0)
            yield
            h1b = gpBn.tile([128, 28 * 64], F16, tag="cBn", name="h1b")
            tpose(h1b[:, 0:gh * 64].rearrange("p (gb c) -> p gb c", c=64),
                  h1a[0:64, 0:FH])
            yield

            # mix1 at c=64 in B-layout; bias+ReLU via max(x,-b)+b with
            # channel-broadcast bias tiles (bias varies along free dim here)
            z1b = gpBn.tile([128, 28 * 64], F16, tag="cBn", name="z1b")

            def cast_z1(c0, cw, ps_ap):
                nc.vector.scalar_tensor_tensor(
                    z1b[:, c0:c0 + cw], ps_ap, 0.0, b1n[:, c0:c0 + cw],
                    ALU.bypass, ALU.max)

            mix(h1b[:, 0:gh * 64], gh * 64, cast_z1)
            nc.gpsimd.tensor_tensor(z1b[:, 0:gh * 64], z1b[:, 0:gh * 64],
                                    b1p[:, 0:gh * 64], ALU.add)
            yield

            # --- L2: mix at 64 directly on z1b, then transform 64->128
            y2p = y2pads[uidx % 2]

            def cast_y2(c0, cw, ps_ap):
                cast_rot(y2p[:, 0:gh * 128]
                         .rearrange("p (gb c) -> p gb c", c=128)
                         [:, c0 // 64:(c0 + cw) // 64, 0:64],
                         ps_ap.rearrange("p (gb c) -> p gb c", c=64))

            mix(z1b[:, 0:gh * 64], gh * 64, cast_y2)
            yield
            y2a = gpA.tile([128, 28 * 128], F16, tag="cA", name="y2a")
            tpose(y2a[:, 0:FH].rearrange("c (gb p) -> c gb p", p=128),
                  y2p[:, 0:FH])
            yield
            x2a = gpA.tile([128, 28 * 128], F16, tag="cA", name="x2a")
            transform([y2a[0:64, 0:FH]], [w2[:]], b2, 128, [x2a], FH, 0)
            yield

            # --- L3: mix@128 -> transform 128->256
            x3b = gpB.tile([128, 28 * 128], F16, tag="cB", name="x3b")
            tpose(x3b[:, 0:FH].rearrange("p (gb c) -> p gb c", c=128),
                  x2a[:, 0:FH])
            yield
            y3b = gpB.tile([128, 28 * 128], F16, tag="cB", name="y3b")

            def cast_y3(c0, cw, ps_ap):
                cast_rot(y3b[:, c0:c0 + cw], ps_ap)

            mix(x3b[:, 0:FH], FH, cast_y3)
            yield
            y3a = gpA.tile([128, 28 * 128], F16, tag="cA", name="y3a")
            tpose(y3a[:, 0:FH].rearrange("c (gb p) -> c gb p", p=128),
                  y3b[:, 0:FH])
            yield
            x3a0 = gpA.tile([128, 28 * 128], F16, tag="cA", name="x3a0")
            x3a1 = gpA.tile([128, 28 * 128], F16, tag="cA", name="x3a1")
            transform([y3a[:, 0:FH]], [w3[:]], b3, 256, [x3a0, x3a1], FH, 0)
            yield
            transform([y3a[:, 0:FH]], [w3[:]], b3, 256, [x3a0, x3a1], FH, 1)
            yield

            # --- L4: mix@256 -> transform 256->256; the two c-halves stay
            # in separate B-tiles (node mix is independent per channel)
            x4b0 = gpB.tile([128, 28 * 128], F16, tag="cB", name="x4b0")
            tpose(x4b0[:, 0:FH].rearrange("p (gb c) -> p gb c", c=128),
                  x3a0[:, 0:FH])
            yield
            x4b1 = gpB.tile([128, 28 * 128], F16, tag="cB", name="x4b1")
            tpose(x4b1[:, 0:FH].rearrange("p (gb c) -> p gb c", c=128),
                  x3a1[:, 0:FH])
            yield
            ylo = gpB.tile([128, 28 * 128], F16, tag="cB", name="ylo")

            def cast_ylo(c0, cw, ps_ap):
                cast_rot(ylo[:, c0:c0 + cw], ps_ap)

            mix(x4b0[:, 0:FH], FH, cast_ylo)
            yield
            yhi = gpB.tile([128, 28 * 128], F16, tag="cB", name="yhi")

            def cast_yhi(c0, cw, ps_ap):
                cast_rot(yhi[:, c0:c0 + cw], ps_ap)

            mix(x4b1[:, 0:FH], FH, cast_yhi)
            yield
            y4a0 = gpA.tile([128, 28 * 128], F16, tag="cA", name="y4a0")
            y4a1 = gpA.tile([128, 28 * 128], F16, tag="cA", name="y4a1")
            tpose(y4a0[:, 0:FH].rearrange("c (gb p) -> c gb p", p=128),
                  ylo[:, 0:FH])
            yield
            tpose(y4a1[:, 0:FH].rearrange("c (gb p) -> c gb p", p=128),
                  yhi[:, 0:FH])
            yield
            x4a0 = gpA.tile([128, 28 * 128], F16, tag="cA", name="x4a0")
            x4a1 = gpA.tile([128, 28 * 128], F16, tag="cA", name="x4a1")
            transform([y4a0[:, 0:FH], y4a1[:, 0:FH]],
                      [w4k[0][:], w4k[1][:]], b4, 256, [x4a0, x4a1], FH, 0)
            yield
            transform([y4a0[:, 0:FH], y4a1[:, 0:FH]],
                      [w4k[0][:], w4k[1][:]], b4, 256, [x4a0, x4a1], FH, 1)
            yield

            # node-sum into F: F[:, k*TP + t], t = 5*(g0gb+gb) + g5
            for ct, xt in enumerate((x4a0, x4a1)):
                xv = (xt[:, 0:FH]
                      .rearrange("p (gb blk) -> p gb blk", blk=128)
                      [:, :, 0:120]
                      .rearrange("p gb (n g5) -> p gb g5 n", g5=G5))
                dstv = (Fts[ct][:, k * TP + g0gb * G5:
                                k * TP + (g0gb + gh) * G5]
                        .rearrange("p (gb g5) -> p gb g5", g5=G5))
                with nc.allow_low_precision("node-sum in fp16, as baseline"):
                    nc.vector.tensor_reduce(dstv, xv, mybir.AxisListType.X,
                                            ALU.add)

            if hf_i == 1:
                # fold this chunk's U slice (with gate bias) into the
                # pipeline: U[:, k] = lxf^T @ F[:, k] + bgf, par-major
                for half in range(2):
                    yield
                    for mt in range(half * 4, half * 4 + 4):
                        u = Umt[mt]
                        uview = u[:].rearrange(
                            "p (par b kk) -> p b kk par", par=2, kk=KK)
                        ps = ps_g.tile([128, 1024], F32, tag="ps",
                                       name="ups")
                        for kt in range(2):
                            nc.tensor.matmul(
                                ps[:, 0:TP],
                                lxf[kt][:, mt * 128:(mt + 1) * 128],
                                Fts[kt][:, k * TP:(k + 1) * TP],
                                start=(kt == 0), stop=(kt == 1))
                        src = ps[:, 0:TP].rearrange(
                            "p (kk par) -> p kk par", par=2)
                        dst = uview[:, k]
                        e = next(cast_cycle)
                        if e is nc.scalar:
                            nc.scalar.activation(dst, src, AF.Identity,
                                                 bias=bgf[:, mt:mt + 1],
                                                 scale=1.0)
                        else:
                            e.tensor_scalar(dst, src, bgf[:, mt:mt + 1],
                                            None, ALU.add)

        # skewed round-robin driver: admit the next unit once the newest
        # active one is SKEW stages in; emit one stage per active unit.
        SKEW = int(os.environ.get("K_SKEW", "6"))
        gens = [unit_stages(u) for u in range(NCH * 2)]
        active, nxt, prog = [], 0, {}
        while active or nxt < len(gens):
            if nxt < len(gens) and (not active or prog[active[-1]] >= SKEW):
                active.append(nxt)
                prog[nxt] = 0
                nxt += 1
            for u in list(active):
                try:
                    next(gens[u])
                    prog[u] += 1
                except StopIteration:
                    active.remove(u)

    # ================= Phase 3: forward LSTM =================
    # (U was computed per-chunk inside the GCN pipeline, par-major with the
    # forward gate bias folded in: col = par*(BL*KK) + b*KK + kk, source F
    # column b*260 + 2*kk + par.)
    lp = ctx.enter_context(tc.tile_pool(name="lstm", bufs=1))
    Hf = lp.tile([128, 2 * ROWS], F16, name="Hf")
    Cf = lp.tile([128, 2 * ROWS], F16, name="Cf")
    nc.vector.memset(Hf[:], 0.0)
    nc.gpsimd.memset(Cf[:], 0.0)
    Cf2 = lp.tile([128, 2 * ROWS], F16, name="Cf2")
    gi = lp.tile([128, 2 * ROWS], F16, name="gi")
    gf = lp.tile([128, 2 * ROWS], F16, name="gf")
    go = lp.tile([128, 2 * ROWS], F16, name="go")
    tg = lp.tile([128, 2 * ROWS], F16, name="tg")
    tcl = lp.tile([128, 2 * ROWS], F16, name="tcl")
    tmp = lp.tile([128, 2 * ROWS], F16, name="tmp")
    tmp2 = lp.tile([128, 2 * ROWS], F16, name="tmp2")
    # pair order (g, i, f, o): the c/h elementwise update interleaves with
    # the later pairs instead of forming a serial tail after all four.
    PAIRS = [(6, 7, tg, AF.Tanh), (0, 1, gi, AF.Sigmoid),
             (2, 3, gf, AF.Sigmoid), (4, 5, go, AF.Sigmoid)]

    with tc.tile_pool(name="lps", bufs=2, space="PSUM") as ps_l:
        # software pipeline over the global pair sequence: each pair's PSUM
        # pre-write is emitted right after the act that frees its buffer
        # (2 pairs earlier), so vector/scalar flow without step-tail stalls.
        PAIR_SEQ = [(s, pi) for s in range(WIN) for pi in range(4)]
        ptile = {}

        def emit_pre(P):
            # only pair 0 (g) gets a vector pre-write: its buffer frees
            # mid-previous-step so the write costs no stall.  Later pairs
            # inject U via identity matmuls at mm time -- a pre-write there
            # would stall on the pair-buffer WAR (only 2 pair tiles fit in
            # PSUM) and gate the PE stream.
            s, pi = PAIR_SEQ[P]
            if not PREWRITE or pi >= 1:
                return
            k0, par = s // 2, s % 2
            ma, mb, _, _ = PAIRS[pi]
            ps = ps_l.tile([128, 2048], F32, tag="lp", name="lp")
            for j, mt in ((0, ma), (1, mb)):
                uv = Umt[mt][:].rearrange("p (par b kk) -> p par b kk",
                                          par=2, kk=KK)
                src = (uv[:, par, :, k0:k0 + NW]
                       .rearrange("p (hh b) k -> p hh b k", hh=2))
                dst = (ps[:, j * 1024:(j + 1) * 1024]
                       .rearrange("p (hh x) -> p hh x", hh=2)
                       [:, :, 0:HROWS]
                       .rearrange("p hh (b k) -> p hh b k", k=NW))
                nc.vector.tensor_copy(dst, src)
            ptile[P] = ps

        emit_pre(0)
        emit_pre(1)
        Cst = [Cf, Cf2]          # ping-pong: out-of-place DVE runs 2x rate
        for P, (s, pi) in enumerate(PAIR_SEQ):
            k0, par = s // 2, s % 2
            ma, mb, gdst, fn = PAIRS[pi]
            Ccur, Cnxt = Cst[s % 2], Cst[1 - s % 2]
            prewritten = P in ptile
            ps = ptile.pop(P) if prewritten \
                else ps_l.tile([128, 2048], F32, tag="lp", name="lp")
            if not prewritten:
                # inject U (+folded bias) via PE before the Whh accumulation
                for j, mt in ((0, ma), (1, mb)):
                    uv = Umt[mt][:].rearrange(
                        "p (par b kk) -> p par b kk", par=2, kk=KK)
                    for hh in range(2):
                        b0 = hh * (BL // 2)
                        nc.tensor.matmul(
                            ps[:, j * 1024 + hh * 512:
                               j * 1024 + hh * 512 + HROWS],
                            ident[:],
                            uv[:, par, b0:b0 + BL // 2, k0:k0 + NW],
                            start=True, stop=False)
            # kt-outer so next step's kt0 mms only need the Hf0 half
            for kt in range(2):
                for j, mt in ((0, ma), (1, mb)):
                    for hh in range(2):
                        nc.tensor.matmul(
                            ps[:, j * 1024 + hh * 512:
                               j * 1024 + hh * 512 + HROWS],
                            lhf[kt][:, mt * 128:(mt + 1) * 128],
                            Hf[:, kt * ROWS + hh * HROWS:
                               kt * ROWS + (hh + 1) * HROWS],
                            start=False, stop=(kt == 1),
                            skip_group_check=True)
            if P + 2 < len(PAIR_SEQ):
                emit_pre(P + 2)
            if pi < 3:
                psq = ps[:].rearrange("p (q x) -> p q x", q=4)[:, :, 0:HROWS]
                gv = gdst[:].rearrange("p (q r) -> p q r", q=4)
                nc.scalar.activation(gv, psq, fn, scale=1.0)
            if pi == 1:
                for j in range(2):
                    sl = slice(j * ROWS, (j + 1) * ROWS)
                    nc.gpsimd.tensor_tensor(tmp[:, sl], gi[:, sl],
                                            tg[:, sl], ALU.mult)
            elif pi == 2:
                for j in range(2):
                    sl = slice(j * ROWS, (j + 1) * ROWS)
                    nc.vector.tensor_tensor(tmp2[:, sl], gf[:, sl],
                                            Ccur[:, sl], ALU.mult)
                    nc.vector.tensor_tensor(Cnxt[:, sl], tmp2[:, sl],
                                            tmp[:, sl], ALU.add)
            elif pi == 3:
                # per-half act -> tanh(C) -> Hf so next step's kt0 matmuls
                # start as soon as half 0 is ready
                for j in range(2):
                    sl = slice(j * ROWS, (j + 1) * ROWS)
                    psj = (ps[:, j * 1024:(j + 1) * 1024]
                           .rearrange("p (hh x) -> p hh x", hh=2)
                           [:, :, 0:HROWS])
                    gvj = (go[:, sl].rearrange("p (hh r) -> p hh r", hh=2))
                    nc.scalar.activation(gvj, psj, fn, scale=1.0)
                    nc.scalar.activation(tcl[:, sl], Cnxt[:, sl], AF.Tanh)
                    nc.vector.tensor_tensor(Hf[:, sl], go[:, sl],
                                            tcl[:, sl], ALU.mult)

        # ===== Phase 4: backward LSTM single step (only hb[:,0] used) =====
        Hb = lp.tile([128, 2 * ROWS], F16, name="Hb")
        kb = (WIN - 2) // 2
        BPAIRS = [(0, 1, gi, AF.Sigmoid), (4, 5, go, AF.Sigmoid),
                  (6, 7, tg, AF.Tanh)]
        for ma, mb, gdst, fn in BPAIRS:
            ps = ps_l.tile([128, 2048], F32, tag="lp", name="lpb")
            for j, mt in ((0, ma), (1, mb)):
                for hh in range(2):
                    pslice = ps[:, j * 1024 + hh * 512:
                                j * 1024 + hh * 512 + HROWS]
                    b0 = hh * (BL // 2)
                    for kt in range(2):
                        fv = Fts[kt][:].rearrange("p (b k two) -> p b k two",
                                                  b=BL, two=2)
                        nc.tensor.matmul(
                            pslice, lxb[kt][:, mt * 128:(mt + 1) * 128],
                            fv[:, b0:b0 + BL // 2, kb:kb + NW, 1],
                            start=(kt == 0), stop=(kt == 1))
                psj = (ps[:, j * 1024:(j + 1) * 1024]
                       .rearrange("p (hh x) -> p hh x", hh=2)[:, :, 0:HROWS])
                gvj = (gdst[:, j * ROWS:(j + 1) * ROWS]
                       .rearrange("p (hh r) -> p hh r", hh=2))
                nc.scalar.activation(gvj, psj, fn,
                                     bias=bgb[:, mt:mt + 1], scale=1.0)
        nc.gpsimd.tensor_tensor(tmp[:], gi[:], tg[:], ALU.mult)
        nc.scalar.activation(tcl[:], tmp[:], AF.Tanh)
        nc.gpsimd.tensor_tensor(Hb[:], go[:], tcl[:], ALU.mult)

        # ===== Phase 5: FC head =====
        ps = ps_l.tile([128, 2048], F32, tag="lp", name="lpf")
        rhs4 = [Hf[:, 0:ROWS], Hf[:, ROWS:2 * ROWS],
                Hb[:, 0:ROWS], Hb[:, ROWS:2 * ROWS]]
        for hh in range(2):
            for qt in range(4):
                nc.tensor.matmul(ps[:, hh * 512:hh * 512 + HROWS],
                                 wfct[qt][:],
                                 rhs4[qt].rearrange("p (h r) -> p h r",
                                                    h=2)[:, hh, :],
                                 start=(qt == 0), stop=(qt == 3))
        ob = lp.tile([EMB, ROWS], F32, name="ob")
        obv = ob[:].rearrange("p (h r) -> p h r", h=2)
        psv = (ps[:, 0:1024].rearrange("p (h x) -> p h x", h=2)
               [:, :, 0:HROWS])
        nc.scalar.activation(obv, psv, AF.Identity,
                             bias=bfc[:, 0:1], scale=1.0)
        nc.sync.dma_start(io["out_d"][:], ob[:])

    if "fdbg0" in io:
        nc.sync.dma_start(io["fdbg0"][:], F0[:])
        nc.sync.dma_start(io["fdbg1"][:], F1[:])
    ctx.close()


def _build_program():
    nc = bacc.Bacc("TRN2", target_bir_lowering=False, debug=False,
                   num_devices=NCORES)

    def din(name, shape, dt=F16):
        return nc.dram_tensor(name, shape, dt, kind="ExternalInput").ap()

    io = dict(
        x0=din("x0", [NCH, FIN, GBLK * 128]),
        mixM=din("mixM", [128, 128]),
        w1=din("w1", [FIN, 64]), w2=din("w2", [64, 128]),
        w3=din("w3", [128, 256]), w4=din("w4", [256, 256]),
        b1n=din("b1n", [128, 28 * 64]), b1p=din("b1p", [128, 28 * 64]),
        b2=din("b2", [128, 1], F32),
        b3=din("b3", [128, 2], F32), b4=din("b4", [128, 2], F32),
        lxf=din("lxf", [256, 1024]), lhf=din("lhf", [256, 1024]),
        lxb=din("lxb", [256, 1024]),
        bgf=din("bgf", [128, 8], F32), bgb=din("bgb", [128, 8], F32),
        wfc=din("wfc", [512, 128]), bfc=din("bfc", [128, 1], F32),
        ident=din("ident", [128, 128]),
        out_d=nc.dram_tensor("out", [EMB, ROWS], F32,
                             kind="ExternalOutput").ap(),
    )
    if os.environ.get("K_FDBG", "0") == "1":
        io["fdbg0"] = nc.dram_tensor("fdbg0", [128, FTOT], F16,
                                     kind="ExternalOutput").ap()
        io["fdbg1"] = nc.dram_tensor("fdbg1", [128, FTOT], F16,
                                     kind="ExternalOutput").ap()

    with tile.TileContext(nc) as tc:
        _kernel_body(tc, io)
    nc.compile()
    return nc


def _host_prep(inputs):
    f16 = np.float16
    data = np.asarray(inputs["data"], np.float32)
    ei = np.asarray(inputs["edge_index"]).astype(np.int64)

    src = np.concatenate([ei[0], np.arange(N)])
    dst = np.concatenate([ei[1], np.arange(N)])
    deg = np.zeros(N, np.float32)
    np.add.at(deg, dst, 1.0)
    dinv = np.where(deg > 0, deg ** -0.5, 0.0).astype(np.float32)
    Ahat = np.zeros((N, N), np.float32)
    np.add.at(Ahat, (dst, src), dinv[src] * dinv[dst])
    mixM = np.zeros((128, 128), np.float32)
    mixM[0:N * G5, 0:N * G5] = np.kron(Ahat.T, np.eye(G5, dtype=np.float32))
    mixM = mixM.astype(f16)

    # x0 in A-layout: [core][chunk b][c (6)][gb*128 + blk], blk = n*5+g5
    # (120:128 zero), t = 5*gb+g5
    d = data.reshape(NCORES, BL, T, N, FIN)
    dpad = np.zeros((NCORES, BL, TP, N, FIN), np.float32)
    dpad[:, :, :T] = d
    dv = dpad.reshape(NCORES, BL, GBLK, G5, N, FIN)
    # -> [core, b, c, gb, n*g5]
    dv = (dv.transpose(0, 1, 5, 2, 4, 3)
          .reshape(NCORES, BL, FIN, GBLK, N * G5))
    x0 = np.zeros((NCORES, BL, FIN, GBLK, 128), np.float32)
    x0[:, :, :, :, 0:N * G5] = dv
    x0 = np.ascontiguousarray(
        x0.reshape(NCORES, BL, FIN, GBLK * 128)).astype(f16)

    perm = np.concatenate([np.arange(0, H), np.arange(H, 2 * H),
                           np.arange(3 * H, 4 * H), np.arange(2 * H, 3 * H)])

    def prep_dir(wih, whh, bih, bhh):
        wihp = np.asarray(wih, np.float32)[perm] / N
        whhp = np.asarray(whh, np.float32)[perm]
        bg = (np.asarray(bih, np.float32) + np.asarray(bhh, np.float32))[perm]
        return (np.ascontiguousarray(wihp.T).astype(f16),
                np.ascontiguousarray(whhp.T).astype(f16),
                np.ascontiguousarray(bg.reshape(8, 128).T).astype(np.float32))

    lxf, lhf, bgf = prep_dir(inputs["lstm_Wih_f"], inputs["lstm_Whh_f"],
                             inputs["lstm_bih_f"], inputs["lstm_bhh_f"])
    lxb, _lhb, bgb = prep_dir(inputs["lstm_Wih_b"], inputs["lstm_Whh_b"],
                              inputs["lstm_bih_b"], inputs["lstm_bhh_b"])

    b1v = np.asarray(inputs["b1"], np.float32)
    com = {
        "mixM": mixM,
        "w1": np.asarray(inputs["W1"], np.float32).astype(f16),
        "w2": np.asarray(inputs["W2"], np.float32).astype(f16),
        "w3": np.asarray(inputs["W3"], np.float32).astype(f16),
        "w4": np.asarray(inputs["W4"], np.float32).astype(f16),
        "b1n": np.ascontiguousarray(
            np.tile(-b1v, (128, 28))).astype(f16),
        "b1p": np.ascontiguousarray(
            np.tile(b1v, (128, 28))).astype(f16),
        "b2": np.asarray(inputs["b2"], np.float32).reshape(128, 1),
        "b3": np.ascontiguousarray(
            np.asarray(inputs["b3"], np.float32).reshape(2, 128).T),
        "b4": np.ascontiguousarray(
            np.asarray(inputs["b4"], np.float32).reshape(2, 128).T),
        "lxf": lxf, "lhf": lhf, "lxb": lxb, "bgf": bgf, "bgb": bgb,
        "wfc": np.asarray(inputs["Wfc"], np.float32).astype(f16),
        "bfc": np.asarray(inputs["bfc"], np.float32).reshape(128, 1),
        "ident": np.eye(128, dtype=f16),
    }
    return [dict(com, x0=x0[c]) for c in range(NCORES)]


TRACE = False          # set by test harness to capture an NTFF profile


def kernel(**inputs) -> np.ndarray:
    if "nc" not in _CACHE:
        _CACHE["nc"] = _build_program()
    nc = _CACHE["nc"]
    in_maps = _host_prep(inputs)
    res = bass_utils.run_bass_kernel_spmd(nc, in_maps,
                                          core_ids=list(range(NCORES)),
                                          trace=TRACE)
    _CACHE["last_res"] = res
    outs = []
    for c in range(NCORES):
        o = res.results[c]["out"]                       # [128, 904]
        outs.append(o.reshape(EMB, BL, NW).transpose(1, 2, 0))
    return np.concatenate(outs, 0).astype(np.float32)   # [64, 113, 128]


if __name__ == "__main__":
    import reference
    ins = {k: np.asarray(v) for k, v in reference.setup_inputs().items()}
    out = kernel(**ins)
    print("kernel out", out.shape, out.dtype, float(np.abs(out).max()))
